# revision 1
# baseline (speedup 1.0000x reference)
"""Trainium2 Bass kernel for nn_DecoderAttention (dual-key tree decoder attention).

Sharding: data-parallel over batch B=8, one batch element per NeuronCore.

Per-core computation (B-slice), all fp32:
  q = target @ Wq + bq                     [T,F]   (kept transposed, duplicated on 128 partitions)
  k/v (node, leaf) = x @ {Wk,Wv} + b       (kept transposed [F, *] via PE-transposed inputs)
  logits = leaf @ Wagg + bagg              [L,1]   (fused mult+reduce on DVE from natural leaf)
  Aqn/Aql softmaxes are computed unnormalized (exp, no max-subtraction: |scores/8| <~ 1.2)
  out_pre = (En^T @ [nh|1])/Z1 + (El^T @ [v|1])/Z2 + root/3
  out = softmax_F(out_pre)                 [T,F]
The tree interpolation's root term commutes through the suffix-mean and the
attention average (softmax weights sum to 1), so root/3 is added once at the end.
Suffix cumsum over L: per-128-chunk triangular matmuls (batched 4 chunks / matmul);
the cross-chunk carries are folded into the LAST ROW of each interp chunk before
the in-chunk suffix (row 127 participates in every suffix sum of its chunk).
"""

import os
import sys

import numpy as np

for _p in ("/opt/trn_rl_repo", "/root/.axon_site/_ro/trn_rl_repo"):
    if os.path.isdir(_p) and _p not in sys.path:
        sys.path.insert(0, _p)

import concourse.bass as bass
import concourse.tile as tile
from concourse import bacc
from concourse import mybir
from concourse.bass_utils import run_bass_kernel_spmd
from concourse.masks import make_identity, make_lower_triangular

FP = mybir.dt.float32
AF = mybir.ActivationFunctionType
OP = mybir.AluOpType
AX = mybir.AxisListType

B, T, N, L, D, F = 8, 1024, 512, 4096, 512, 64
BR = L // N          # 8 leaves per node
NC = L // 128        # 32 leaf chunks of 128
ND = D // 128        # 4 contraction chunks
SCALE = 1.0 / float(np.sqrt(F))


def _bcast_ap(ap, parts=128):
    """Partition-broadcast read AP (DRAM sources only)."""
    dims = list(ap.ap)
    if dims and dims[0][1] == 1:
        dims = dims[1:]
    return bass.AP(tensor=ap.tensor, offset=ap.offset, ap=[[0, parts]] + dims)


def _rep_ap(ap, rep):
    """Append a step-0 innermost free dim (read each element `rep` times)."""
    return bass.AP(tensor=ap.tensor, offset=ap.offset, ap=list(ap.ap) + [[0, rep]])


def build_nc():
    nc = bacc.Bacc("TRN2", target_bir_lowering=False, debug=False)

    d_root = nc.dram_tensor("root", [1, F], FP, kind="ExternalInput")
    d_node = nc.dram_tensor("node", [N, D], FP, kind="ExternalInput")
    d_leaf = nc.dram_tensor("leaf", [L, D], FP, kind="ExternalInput")
    d_target = nc.dram_tensor("target", [T, D], FP, kind="ExternalInput")
    d_wq = nc.dram_tensor("Wq", [D, F], FP, kind="ExternalInput")
    d_bq = nc.dram_tensor("bq", [F], FP, kind="ExternalInput")
    d_wk = nc.dram_tensor("Wk", [D, F], FP, kind="ExternalInput")
    d_bk = nc.dram_tensor("bk", [F], FP, kind="ExternalInput")
    d_wv = nc.dram_tensor("Wv", [D, F], FP, kind="ExternalInput")
    d_bv = nc.dram_tensor("bv", [F], FP, kind="ExternalInput")
    d_wagg = nc.dram_tensor("Wagg", [D, 1], FP, kind="ExternalInput")
    d_bagg = nc.dram_tensor("bagg", [1], FP, kind="ExternalInput")
    d_out = nc.dram_tensor("out", [T, F], FP, kind="ExternalOutput")

    with tile.TileContext(nc) as tc:
        _emit(nc, tc, d_root, d_node, d_leaf, d_target, d_wq, d_bq, d_wk, d_bk,
              d_wv, d_bv, d_wagg, d_bagg, d_out)
    nc.compile()
    return nc


def _emit(nc, tc, d_root, d_node, d_leaf, d_target, d_wq, d_bq, d_wk, d_bk,
          d_wv, d_bv, d_wagg, d_bagg, d_out):
    from contextlib import ExitStack

    with ExitStack() as ctx:
        consts = ctx.enter_context(tc.tile_pool(name="consts", bufs=1))
        big = ctx.enter_context(tc.tile_pool(name="big", bufs=1))
        lnat = ctx.enter_context(tc.tile_pool(name="lnat", bufs=3))
        ltp = ctx.enter_context(tc.tile_pool(name="ltp", bufs=2))
        work = ctx.enter_context(tc.tile_pool(name="work", bufs=2))
        epool = ctx.enter_context(tc.tile_pool(name="epool", bufs=3))
        ptr = ctx.enter_context(tc.tile_pool(name="ptr", bufs=2, space="PSUM"))
        pmm = ctx.enter_context(tc.tile_pool(name="pmm", bufs=4, space="PSUM"))
        pacc = ctx.enter_context(tc.tile_pool(name="pacc", bufs=2, space="PSUM"))

        # ---------------- constants ----------------
        ident = consts.tile([128, 128], FP)
        make_identity(nc, ident[:])
        tri128 = consts.tile([128, 128], FP)      # [m,l]=1 iff l<=m  (suffix-sum lhsT)
        make_lower_triangular(nc, tri128[:], val=1.0, diag=True)
        tri32s = consts.tile([32, 32], FP)        # [k,c]=1 iff k>c   (carry)
        make_lower_triangular(nc, tri32s[:], val=1.0, diag=False)

        # G[m,j] = 1 iff m//8 == j  (leaf->node group indicator), GT transposed
        G = consts.tile([128, 16], FP)
        nc.gpsimd.memset(G[:], 1.0)
        nc.gpsimd.affine_select(out=G[:], in_=G[:], compare_op=OP.is_ge, fill=0.0,
                                base=0, pattern=[[-BR, 16]], channel_multiplier=1)
        nc.gpsimd.affine_select(out=G[:], in_=G[:], compare_op=OP.is_ge, fill=0.0,
                                base=BR - 1, pattern=[[BR, 16]], channel_multiplier=-1)
        GT = consts.tile([16, 128], FP)
        nc.gpsimd.memset(GT[:], 1.0)
        nc.gpsimd.affine_select(out=GT[:], in_=GT[:], compare_op=OP.is_ge, fill=0.0,
                                base=0, pattern=[[1, 128]], channel_multiplier=-BR)
        nc.gpsimd.affine_select(out=GT[:], in_=GT[:], compare_op=OP.is_ge, fill=0.0,
                                base=BR - 1, pattern=[[-1, 128]], channel_multiplier=BR)

        onesP = consts.tile([128, 64], FP)
        nc.gpsimd.memset(onesP[:], 1.0)

        # 1 / (3 * (L - l)) with l = 128*c + p   -> [128, 32]
        cnt3 = consts.tile([128, NC], FP)
        nc.gpsimd.iota(cnt3[:], pattern=[[-3 * 128, NC]], base=3 * L,
                       channel_multiplier=-3, allow_small_or_imprecise_dtypes=True)
        inv3 = consts.tile([128, NC], FP)
        nc.vector.reciprocal(inv3[:], cnt3[:])

        # ---------------- weights / biases ----------------
        w_kv = consts.tile([128, ND, 128], FP)     # cols 0:64 Wk, 64:128 Wv per d-chunk
        w_qq = consts.tile([128, ND, 128], FP)     # Wq duplicated
        wk_raw = consts.tile([128, ND, F], FP)
        wv_raw = consts.tile([128, ND, F], FP)
        wq_raw = consts.tile([128, ND, F], FP)
        nc.sync.dma_start(wk_raw[:], d_wk[:].rearrange("(j p) f -> p j f", p=128))
        nc.sync.dma_start(wv_raw[:], d_wv[:].rearrange("(j p) f -> p j f", p=128))
        nc.sync.dma_start(wq_raw[:], d_wq[:].rearrange("(j p) f -> p j f", p=128))
        for dc in range(ND):
            nc.vector.tensor_copy(w_kv[:, dc, 0:F], wk_raw[:, dc, :])
            nc.vector.tensor_copy(w_kv[:, dc, F:128], wv_raw[:, dc, :])
            nc.vector.tensor_copy(w_qq[:, dc, 0:F], wq_raw[:, dc, :])
            nc.vector.tensor_copy(w_qq[:, dc, F:128], wq_raw[:, dc, :])

        wagg_bc = consts.tile([128, D], FP)        # Wagg broadcast down partitions
        nc.gpsimd.dma_start(wagg_bc[:], _bcast_ap(d_wagg[:, 0:1].rearrange("d o -> (d o)")))

        bias_q = consts.tile([128, 1], FP)
        bias_k = consts.tile([128, 1], FP)
        bias_v = consts.tile([128, 1], FP)
        bq2 = d_bq[:].rearrange("(f o) -> f o", o=1)
        bk2 = d_bk[:].rearrange("(f o) -> f o", o=1)
        bv2 = d_bv[:].rearrange("(f o) -> f o", o=1)
        nc.gpsimd.dma_start(bias_q[0:F, :], bq2)
        nc.gpsimd.dma_start(bias_q[F:128, :], bq2)
        nc.gpsimd.dma_start(bias_k[0:F, :], bk2)
        nc.gpsimd.dma_start(bias_k[F:128, :], bk2)
        nc.gpsimd.dma_start(bias_v[0:F, :], bv2)
        bagg_b = consts.tile([128, 1], FP)
        nc.gpsimd.dma_start(bagg_b[:], _bcast_ap(d_bagg[:]))

        # rootT3 = root^T / 3   [64, 1]
        root_row = consts.tile([1, F], FP)
        nc.sync.dma_start(root_row[:], d_root[:])
        rt_ps = ptr.tile([F, 1], FP, tag="tp")
        nc.tensor.transpose(rt_ps[:], root_row[:], ident[0:1, 0:1])
        rootT3 = consts.tile([F, 1], FP)
        nc.scalar.activation(out=rootT3[:], in_=rt_ps[:], func=AF.Copy, scale=1.0 / 3.0)

        # ---------------- target -> qdual [128, 1024] ----------------
        targT = big.tile([128, ND, T], FP)
        for ib in range(T // 512):
            tn = lnat.tile([128, 4, D], FP, tag="xnat")
            nc.sync.dma_start(tn[:], d_target[ib * 512:(ib + 1) * 512, :]
                              .rearrange("(j p) d -> p j d", p=128))
            for j in range(4):
                i = 4 * ib + j
                tp = ptr.tile([128, 512], FP, tag="tp")
                for dc in range(ND):
                    nc.tensor.transpose(tp[:, dc * 128:(dc + 1) * 128],
                                        tn[:, j, dc * 128:(dc + 1) * 128], ident[:])
                nc.vector.tensor_copy(
                    targT[:, 0:ND, i * 128:(i + 1) * 128],
                    tp[:].rearrange("p (dc b) -> p dc b", b=128))
        qdual = big.tile([128, T], FP)
        for h in range(2):
            q_ps = pmm.tile([128, 512], FP, tag="mm")
            for dc in range(ND):
                nc.tensor.matmul(q_ps[:], w_qq[:, dc, :],
                                 targT[:, dc, h * 512:(h + 1) * 512],
                                 start=(dc == 0), stop=(dc == ND - 1))
            nc.scalar.activation(out=qdual[:, h * 512:(h + 1) * 512], in_=q_ps[:],
                                 func=AF.Identity, bias=bias_q[:])

        # ---------------- node -> kTn_dual [128, 256], node_vT [64, 512] ----------------
        nodeT = big.tile([128, ND, N], FP)
        nn = lnat.tile([128, 4, D], FP, tag="xnat")
        nc.sync.dma_start(nn[:], d_node[:].rearrange("(j p) d -> p j d", p=128))
        for i in range(N // 128):
            tp = ptr.tile([128, 512], FP, tag="tp")
            for dc in range(ND):
                nc.tensor.transpose(tp[:, dc * 128:(dc + 1) * 128],
                                    nn[:, i, dc * 128:(dc + 1) * 128], ident[:])
            nc.vector.tensor_copy(nodeT[:, 0:ND, i * 128:(i + 1) * 128],
                                  tp[:].rearrange("p (dc b) -> p dc b", b=128))
        kTn_dual = big.tile([128, 256], FP)
        node_vT = big.tile([64, N], FP)
        kvn_ps = pmm.tile([128, 512], FP, tag="mm")
        for dc in range(ND):
            nc.tensor.matmul(kvn_ps[:], w_kv[:, dc, :], nodeT[:, dc, :],
                             start=(dc == 0), stop=(dc == ND - 1))
        for b in range(4):
            ro, co = (b % 2) * 64, (b // 2) * 128
            nc.scalar.activation(out=kTn_dual[ro:ro + 64, co:co + 128],
                                 in_=kvn_ps[0:64, b * 128:(b + 1) * 128],
                                 func=AF.Identity, bias=bias_k[ro:ro + 64, :])
        nc.scalar.activation(out=node_vT[:], in_=kvn_ps[64:128, :],
                             func=AF.Identity, bias=bias_v[0:64, :])

        # ---------------- leaf: kTdual, tile12 (vT + interpT), logits ----------------
        kTdual = big.tile([128, L // 2], FP)   # 512-chunk i -> rows (i%2)*64, cols (i//2)*512
        tile12 = big.tile([128, L], FP)        # rows 0:64 leaf_vT, rows 64:128 interp'T
        logits_nat = big.tile([128, NC], FP)
        for i in range(L // 512):
            leafT = ltp.tile([128, ND, 512], FP)
            ln = lnat.tile([128, 4, D], FP, tag="xnat")
            nc.sync.dma_start(ln[:], d_leaf[i * 512:(i + 1) * 512, :]
                              .rearrange("(j p) d -> p j d", p=128))
            for j in range(4):
                c = 4 * i + j
                # logits chunk: product on (otherwise idle) gpsimd, row-sum on DVE.
                # (tensor_tensor_reduce would fuse these but crashes the device.)
                prod = work.tile([128, D], FP, tag="prod")
                nc.gpsimd.tensor_tensor(out=prod[:], in0=ln[:, j, :], in1=wagg_bc[:],
                                        op=OP.mult)
                nc.vector.tensor_reduce(out=logits_nat[:, c:c + 1], in_=prod[:],
                                        axis=AX.X, op=OP.add)
                tp = ptr.tile([128, 512], FP, tag="tp")
                for dc in range(ND):
                    nc.tensor.transpose(tp[:, dc * 128:(dc + 1) * 128],
                                        ln[:, j, dc * 128:(dc + 1) * 128], ident[:])
                nc.vector.tensor_copy(leafT[:, 0:ND, j * 128:(j + 1) * 128],
                                      tp[:].rearrange("p (dc b) -> p dc b", b=128))
            kv_ps = pmm.tile([128, 512], FP, tag="mm")
            for dc in range(ND):
                nc.tensor.matmul(kv_ps[:], w_kv[:, dc, :], leafT[:, dc, :],
                                 start=(dc == 0), stop=(dc == ND - 1))
            ro, co = (i % 2) * 64, (i // 2) * 512
            nc.scalar.activation(out=kTdual[ro:ro + 64, co:co + 512],
                                 in_=kv_ps[0:64, :], func=AF.Identity,
                                 bias=bias_k[ro:ro + 64, :])
            sl = slice(i * 512, (i + 1) * 512)
            nc.scalar.activation(out=tile12[0:64, sl],
                                 in_=kv_ps[64:128, :], func=AF.Identity,
                                 bias=bias_v[0:64, :])
            # interp'T = leaf_vT + node_vT replicated 8x along l (no root, no /3)
            base = node_vT[0:64, 64 * i:64 * (i + 1)]
            nc.vector.tensor_tensor(
                out=tile12[64:128, sl].rearrange("f (n c) -> f n c", c=BR),
                in0=tile12[0:64, sl].rearrange("f (n c) -> f n c", c=BR),
                in1=_rep_ap(base, BR), op=OP.add)

        # chunk totals -> carries, folded into last row of each interp chunk
        totT = work.tile([64, NC], FP, tag="tot")
        nc.vector.tensor_reduce(out=totT[:],
                                in_=tile12[64:128, :].rearrange("f (c m) -> f c m", m=128),
                                axis=AX.X, op=OP.add)
        tot_ps = ptr.tile([NC, 64], FP, tag="tp")
        nc.tensor.transpose(tot_ps[:], totT[:], ident[0:64, 0:64])
        totals = work.tile([NC, 64], FP, tag="tot")
        nc.scalar.activation(out=totals[:], in_=tot_ps[:], func=AF.Copy)
        carrT_ps = ptr.tile([64, NC], FP, tag="tp")
        nc.tensor.matmul(carrT_ps[:], totals[:], tri32s[:], start=True, stop=True)
        # interpT[f, 128c+127] += carryT[f, c]  (row 127 is in every suffix sum)
        last_rows = tile12[64:128, 127::128]
        nc.vector.tensor_tensor(out=last_rows, in0=last_rows, in1=carrT_ps[:], op=OP.add)

        # vnat/interp natural via one [128,128] transpose per chunk:
        # out cols 0:64 = leaf_v chunk, cols 64:128 = interp chunk
        comb = big.tile([128, NC, 129], FP)    # [v(64) | ones | interp(64)]
        nc.vector.memset(comb[:, :, 64:65], 1.0)
        for c in range(NC):
            tp = ptr.tile([128, 512], FP, tag="tp")
            nc.tensor.transpose(tp[:, 0:128], tile12[:, c * 128:(c + 1) * 128], ident[:])
            nc.vector.tensor_copy(comb[:, c, 0:64], tp[:, 0:64])
            nc.vector.tensor_copy(comb[:, c, 65:129], tp[:, 64:128])

        # ---------------- group-softmax weights (batched over all 32 chunks) -------
        e_all = work.tile([128, NC], FP, tag="e_all")
        nc.scalar.activation(out=e_all[:], in_=logits_nat[:], func=AF.Exp, bias=bagg_b[:])
        s_ps = pmm.tile([16, NC], FP, tag="mm")
        nc.tensor.matmul(s_ps[:], G[:], e_all[:], start=True, stop=True)
        sinv = work.tile([16, NC], FP, tag="sinv")
        nc.vector.reciprocal(sinv[:], s_ps[:])
        r_ps = pmm.tile([128, NC], FP, tag="mm")
        nc.tensor.matmul(r_ps[:], GT[:], sinv[:], start=True, stop=True)
        w_all = work.tile([128, NC], FP, tag="w_all")
        nc.vector.tensor_tensor(out=w_all[:], in0=e_all[:], in1=r_ps[:], op=OP.mult)

        # ---------------- suffix-mean (4 chunks per matmul) + node_hat ----------------
        nh_nat = big.tile([128, 4, 65], FP)
        nc.vector.memset(nh_nat[:, :, 64:65], 1.0)
        wbd_pp = big.tile([128, 2, 128], FP)
        nc.vector.memset(wbd_pp[:], 0.0)
        for c4 in range(NC // 4):
            sfx_ps = pmm.tile([128, 4, 64], FP, tag="mm")
            nc.tensor.matmul(sfx_ps[:], tri128[:], comb[:, 4 * c4:4 * c4 + 4, 65:129],
                             start=True, stop=True)
            upw4 = work.tile([128, 4, 64], FP, tag="upw")
            nc.vector.tensor_tensor(out=upw4[:], in0=sfx_ps[:],
                                    in1=_rep_ap(inv3[:, 4 * c4:4 * c4 + 4], 64),
                                    op=OP.mult)
            for jc in range(4):
                c = 4 * c4 + jc
                bo = 16 * (c % 8)
                wsl = wbd_pp[:, c % 2, :]
                nc.vector.tensor_scalar(out=wsl[:, bo:bo + 16],
                                        in0=G[:], scalar1=w_all[:, c:c + 1],
                                        scalar2=None, op0=OP.mult)
                if c % 8 == 0:
                    nh_ps = pmm.tile([128, 64], FP, tag="mm", name=f"nh_ps{c // 8}")
                nc.tensor.matmul(nh_ps[:], wsl, upw4[:, jc, :], start=(c % 8 == 0),
                                 stop=(c % 8 == 7), skip_group_check=True)
                # restore the slot to all-zeros for its next use
                nc.vector.memset(wsl[:, bo:bo + 16], 0.0)
                if c % 8 == 7:
                    nc.scalar.activation(out=nh_nat[:, c // 8, 0:64], in_=nh_ps[:],
                                         func=AF.Copy)

        # ---------------- node attention -> o1 [65, 1024] ----------------
        o1_sb = big.tile([65, T], FP)
        for h in range(2):
            o1_ps = pacc.tile([65, 512], FP, tag="oacc")
            for ct in range(2):
                for half in range(2):
                    ro = half * 64
                    b = 2 * ct + half
                    st = pmm.tile([128, 512], FP, tag="mm")
                    nc.tensor.matmul(st[:], kTn_dual[ro:ro + 64, ct * 128:(ct + 1) * 128],
                                     qdual[ro:ro + 64, h * 512:(h + 1) * 512],
                                     start=True, stop=True)
                    en = epool.tile([128, 512], FP, tag="en")
                    nc.scalar.activation(out=en[:], in_=st[:], func=AF.Exp, scale=SCALE)
                    nc.tensor.matmul(o1_ps[:], nh_nat[:, b, :], en[:],
                                     start=(b == 0), stop=(b == 3),
                                     skip_group_check=True)
            nc.scalar.activation(out=o1_sb[:, h * 512:(h + 1) * 512], in_=o1_ps[:],
                                 func=AF.Copy)

        # ---------------- leaf attention -> o2_sb [65, 1024] ----------------
        o2_sb = big.tile([65, T], FP)
        o2_ps = [pacc.tile([65, 512], FP, tag="oacc", name=f"o2_ps{h}") for h in range(2)]
        for ct in range(16):
            blocks = (8 * (ct // 4) + ct % 4, 8 * (ct // 4) + ct % 4 + 4)
            for h in range(2):
                for half in range(2):
                    ro = half * 64
                    b = blocks[half]
                    st = pmm.tile([128, 512], FP, tag="mm")
                    nc.tensor.matmul(st[:], kTdual[ro:ro + 64, ct * 128:(ct + 1) * 128],
                                     qdual[ro:ro + 64, h * 512:(h + 1) * 512],
                                     start=True, stop=True)
                    el = epool.tile([128, 512], FP, tag="el")
                    nc.scalar.activation(out=el[:], in_=st[:], func=AF.Exp, scale=SCALE)
                    nc.tensor.matmul(o2_ps[h][:], comb[:, b, 0:65], el[:],
                                     start=(ct == 0 and half == 0),
                                     stop=(ct == 15 and half == 1),
                                     skip_group_check=True)
        for h in range(2):
            nc.scalar.activation(out=o2_sb[:, h * 512:(h + 1) * 512], in_=o2_ps[h][:],
                                 func=AF.Copy)

        # ---------------- combine + final softmax over F ----------------
        fs1 = work.tile([65, T], FP, tag="fs")
        fs2 = work.tile([65, T], FP, tag="fs")
        nc.vector.reciprocal(fs1[64:65, :], o1_sb[64:65, :])
        nc.vector.reciprocal(fs2[64:65, :], o2_sb[64:65, :])
        outT = big.tile([64, T], FP)
        for h in range(2):
            hs = slice(h * 512, (h + 1) * 512)
            b1 = pmm.tile([64, 512], FP, tag="mm")
            nc.tensor.matmul(b1[:], onesP[64:65, 0:64], fs1[64:65, hs], start=True, stop=True)
            b2 = pmm.tile([64, 512], FP, tag="mm")
            nc.tensor.matmul(b2[:], onesP[64:65, 0:64], fs2[64:65, hs], start=True, stop=True)
            x1 = work.tile([64, 512], FP, tag="x1")
            nc.vector.tensor_tensor(out=x1[:], in0=o1_sb[0:64, hs], in1=b1[:], op=OP.mult)
            x2 = work.tile([64, 512], FP, tag="x2")
            nc.vector.tensor_tensor(out=x2[:], in0=o2_sb[0:64, hs], in1=b2[:], op=OP.mult)
            s12 = work.tile([64, 512], FP, tag="s12")
            nc.vector.tensor_tensor(out=s12[:], in0=x1[:], in1=x2[:], op=OP.add)
            pre = work.tile([64, 512], FP, tag="pre")
            nc.vector.tensor_scalar(out=pre[:], in0=s12[:], scalar1=rootT3[:],
                                    scalar2=None, op0=OP.add)
            e3 = work.tile([64, 512], FP, tag="e3")
            nc.scalar.activation(out=e3[:], in_=pre[:], func=AF.Exp)
            z3 = pmm.tile([1, 512], FP, tag="mm")
            nc.tensor.matmul(z3[:], onesP[0:64, 0:1], e3[:], start=True, stop=True)
            nc.vector.reciprocal(fs1[0:1, hs], z3[:])
            b3 = pmm.tile([64, 512], FP, tag="mm")
            nc.tensor.matmul(b3[:], onesP[0:1, 0:64], fs1[0:1, hs], start=True, stop=True)
            nc.vector.tensor_tensor(out=outT[:, hs], in0=e3[:], in1=b3[:], op=OP.mult)

        onat = big.tile([128, T // 128, F], FP)
        for k2 in range(T // 256):
            op_ = ptr.tile([128, 512], FP, tag="tp")
            for k in (2 * k2, 2 * k2 + 1):
                nc.tensor.transpose(op_[:, (k % 2) * 64:(k % 2) * 64 + 64],
                                    outT[:, k * 128:(k + 1) * 128], ident[0:64, 0:64])
            nc.vector.tensor_copy(onat[:, 2 * k2:2 * k2 + 2, :]
                                  .rearrange("p k f -> p (k f)"), op_[:, 0:128])
        nc.sync.dma_start(d_out[:].rearrange("(k p) f -> p k f", p=128), onat[:])


_NC_CACHE = None


def kernel(**inputs):
    global _NC_CACHE
    if _NC_CACHE is None:
        _NC_CACHE = build_nc()
    nc = _NC_CACHE
    shared = {k: np.ascontiguousarray(np.asarray(inputs[k], dtype=np.float32))
              for k in ("Wq", "bq", "Wk", "bk", "Wv", "bv", "Wagg", "bagg")}
    in_maps = []
    for b in range(B):
        m = dict(shared)
        m["root"] = np.ascontiguousarray(np.asarray(inputs["root"][b], dtype=np.float32))
        m["node"] = np.ascontiguousarray(np.asarray(inputs["node"][b], dtype=np.float32))
        m["leaf"] = np.ascontiguousarray(np.asarray(inputs["leaf"][b], dtype=np.float32))
        m["target"] = np.ascontiguousarray(np.asarray(inputs["target"][b], dtype=np.float32))
        in_maps.append(m)
    res = run_bass_kernel_spmd(nc, in_maps, core_ids=list(range(B)))
    return np.stack([r["out"] for r in res.results], axis=0)



# revision 32
# speedup vs baseline: 2.2452x; 2.2452x over previous
"""Trainium2 Bass kernel for nn_DecoderAttention (dual-key tree decoder attention).

Sharding: data-parallel over batch B=8, one batch element per NeuronCore.

Per-core computation (B-slice), fp32 data with fp32r (split-bf16) matmuls:
  q = target @ Wq + bq                     [T,F]   (kept transposed, duplicated on 128 partitions)
  k/v (node, leaf) = x @ {Wk,Wv} + b       (kept transposed [F, *] via PE-transposed inputs)
  logits = leaf @ Wagg + bagg              [L,1]   (PE: leafT-chunk stationary x Wagg column)
  Aqn/Aql softmaxes are computed unnormalized (exp, no max-subtraction: |scores/8| <~ 1.2)
  out_pre = (En^T @ [nh|1])/Z1 + (El^T @ [v|1])/Z2 + root/3
  out = softmax_F(out_pre)                 [T,F]
The tree interpolation's root term commutes through the suffix-mean and the
attention average (softmax weights sum to 1), so root/3 is added once at the end.
Suffix cumsum over L: per-128-chunk triangular matmuls (batched 4 chunks / matmul);
the cross-chunk carries are folded into the LAST ROW of each interp chunk before
the in-chunk suffix (row 127 participates in every suffix sum of its chunk).

Schedule: the leaf-attention score/exp/accumulate stream (the dominant
Activation-engine load) is software-pipelined into the leaf projection loop —
each 512-leaf chunk queues the 4 (score, exp, accumulate) units it unlocks and
drains the queue one chunk behind, so the Activation engine runs continuously
while PE fills its gaps with the next chunk's transposes/projections. Only the
node path (suffix mean -> node_hat -> node attention) waits for the full leaf
pass (it needs the global suffix carries); node attention is interleaved into
the suffix loop, and the final combine/softmax is pipelined in T/4 slices.
PSUM budget (8 banks): pmm 2x[128,1024] + ptr 2x[128,512] + pacc 1x[65,1024],
with the pacc buffer sequenced o2 -> logits -> o1 -> z3.
"""

import os
import sys
from collections import deque

BISECT = set(os.environ.get("KBISECT", "").split(",")) - {""}

import numpy as np

for _p in ("/opt/trn_rl_repo", "/root/.axon_site/_ro/trn_rl_repo"):
    if os.path.isdir(_p) and _p not in sys.path:
        sys.path.insert(0, _p)

import concourse.bass as bass
import concourse.tile as tile
from concourse import bacc
from concourse import mybir
from concourse.bass_utils import run_bass_kernel_spmd
from concourse.masks import make_identity, make_lower_triangular

FP = mybir.dt.float32
FR = mybir.dt.float32r
BF = mybir.dt.bfloat16
AF = mybir.ActivationFunctionType
OP = mybir.AluOpType
AX = mybir.AxisListType

B, T, N, L, D, F = 8, 1024, 512, 4096, 512, 64
BR = L // N          # 8 leaves per node
NC = L // 128        # 32 leaf chunks of 128
ND = D // 128        # 4 contraction chunks
TQ = T // 4          # final-stage pipeline slice
SCALE = 1.0 / float(np.sqrt(F))


def R(ap):
    """View an fp32 AP as float32r (full-rate PE matmuls, split-bf16 numerics)."""
    return ap.bitcast(FR)


def _rep_ap(ap, rep):
    """Append a step-0 innermost free dim (read each element `rep` times)."""
    return bass.AP(tensor=ap.tensor, offset=ap.offset, ap=list(ap.ap) + [[0, rep]])


def build_nc():
    nc = bacc.Bacc("TRN2", target_bir_lowering=False, debug=False)

    d_root = nc.dram_tensor("root", [1, F], FP, kind="ExternalInput")
    d_node = nc.dram_tensor("node", [N, D], FP, kind="ExternalInput")
    d_leaf = nc.dram_tensor("leaf", [L, D], FP, kind="ExternalInput")
    d_target = nc.dram_tensor("target", [T, D], FP, kind="ExternalInput")
    d_wq = nc.dram_tensor("Wq", [D, F], FP, kind="ExternalInput")
    d_bq = nc.dram_tensor("bq", [F], FP, kind="ExternalInput")
    d_wk = nc.dram_tensor("Wk", [D, F], FP, kind="ExternalInput")
    d_bk = nc.dram_tensor("bk", [F], FP, kind="ExternalInput")
    d_wv = nc.dram_tensor("Wv", [D, F], FP, kind="ExternalInput")
    d_bv = nc.dram_tensor("bv", [F], FP, kind="ExternalInput")
    d_wagg = nc.dram_tensor("Wagg", [D, 1], FP, kind="ExternalInput")
    d_bagg = nc.dram_tensor("bagg", [1], FP, kind="ExternalInput")
    d_out = nc.dram_tensor("out", [T, F], FP, kind="ExternalOutput")

    with tile.TileContext(nc) as tc:
        _emit(nc, tc, d_root, d_node, d_leaf, d_target, d_wq, d_bq, d_wk, d_bk,
              d_wv, d_bv, d_wagg, d_bagg, d_out)
    nc.compile()
    return nc


def _emit(nc, tc, d_root, d_node, d_leaf, d_target, d_wq, d_bq, d_wk, d_bk,
          d_wv, d_bv, d_wagg, d_bagg, d_out):
    from contextlib import ExitStack

    with ExitStack() as ctx:
        ctx.enter_context(nc.allow_low_precision(
            reason="fp32r rounding on matmul operands is intentional"))
        consts = ctx.enter_context(tc.tile_pool(name="consts", bufs=1))
        big = ctx.enter_context(tc.tile_pool(name="big", bufs=1))
        lnat = ctx.enter_context(tc.tile_pool(name="lnat", bufs=2))
        ltp = ctx.enter_context(tc.tile_pool(name="ltp", bufs=2))
        work = ctx.enter_context(tc.tile_pool(name="work", bufs=2))
        fin = ctx.enter_context(tc.tile_pool(name="fin", bufs=1))
        epool = ctx.enter_context(tc.tile_pool(name="epool", bufs=3))
        ptr = ctx.enter_context(tc.tile_pool(name="ptr", bufs=2, space="PSUM"))
        pmm = ctx.enter_context(tc.tile_pool(name="pmm", bufs=2, space="PSUM"))
        pacc = ctx.enter_context(tc.tile_pool(name="pacc", bufs=1, space="PSUM"))

        # ---- identity only (gates all transposes); other consts deferred ----
        ident = consts.tile([128, 128], FP)
        make_identity(nc, ident[:])
        identR = consts.tile([128, 128], FP)   # fp32r-rounded copy for R-transposes
        nc.vector.tensor_copy(R(identR[:]), ident[:])

        # ---------------- stage A: target -> qdual [128, 1024] ----------------
        tns = []
        for ib in range(T // 512):
            tn = lnat.tile([128, 4, D], FP, tag="xnat")
            nc.sync.dma_start(R(tn[:]), R(d_target[ib * 512:(ib + 1) * 512, :]
                              .rearrange("(j p) d -> p j d", p=128)))
            tns.append(tn)

        # weights / biases, queued behind the first input loads
        w_kv = consts.tile([128, ND, 128], FP)     # cols 0:64 Wk, 64:128 Wv per d-chunk
        w_qq = consts.tile([128, ND, 128], FP)     # Wq duplicated
        nc.sync.dma_start(R(w_qq[:, :, 0:F]), R(d_wq[:].rearrange("(j p) f -> p j f", p=128)))
        nc.sync.dma_start(R(w_qq[:, :, F:128]), R(d_wq[:].rearrange("(j p) f -> p j f", p=128)))
        nc.sync.dma_start(R(w_kv[:, :, 0:F]), R(d_wk[:].rearrange("(j p) f -> p j f", p=128)))
        nc.sync.dma_start(R(w_kv[:, :, F:128]), R(d_wv[:].rearrange("(j p) f -> p j f", p=128)))
        wagg_raw = consts.tile([128, ND], FP)
        nc.sync.dma_start(wagg_raw[:], d_wagg[:].rearrange("(j p) o -> p (j o)", p=128))
        bias_q = consts.tile([128, 1], FP)
        bias_k = consts.tile([128, 1], FP)
        bias_v = consts.tile([64, 1], FP)
        bq2 = d_bq[:].rearrange("(f o) -> f o", o=1)
        bk2 = d_bk[:].rearrange("(f o) -> f o", o=1)
        nc.sync.dma_start(bias_q[0:F, :], bq2)
        nc.sync.dma_start(bias_q[F:128, :], bq2)
        nc.sync.dma_start(bias_k[0:F, :], bk2)
        nc.sync.dma_start(bias_k[F:128, :], bk2)
        nc.sync.dma_start(bias_v[:], d_bv[:].rearrange("(f o) -> f o", o=1))
        bagg_b = consts.tile([128, 1], FP)
        _ba = d_bagg[:]
        nc.gpsimd.dma_start(bagg_b[:], bass.AP(tensor=_ba.tensor, offset=_ba.offset,
                                               ap=[[0, 128], [1, 1]]))
        root_row = consts.tile([1, F], FP)
        nc.sync.dma_start(root_row[:], d_root[:])

        q_ps = pmm.tile([128, T], FP, tag="mm", name="q_ps")
        for ib in range(T // 512):
            targT = ltp.tile([128, ND, 512], FP, tag="lt")
            for j in range(4):
                tp = ptr.tile([128, 512], FP, tag="tp")
                for dc in range(ND):
                    nc.tensor.transpose(R(tp[:, dc * 128:(dc + 1) * 128]),
                                        R(tns[ib][:, j, dc * 128:(dc + 1) * 128]),
                                        R(identR[:]))
                nc.vector.tensor_copy(R(targT[:, 0:ND, j * 128:(j + 1) * 128]),
                                      R(tp[:].rearrange("p (dc b) -> p dc b", b=128)))
            for dc in range(ND):
                nc.tensor.matmul(q_ps[:, ib * 512:(ib + 1) * 512], R(w_qq[:, dc, :]),
                                 R(targT[:, dc, :]), start=(dc == 0), stop=(dc == ND - 1),
                                 skip_group_check=True)
        qdual = big.tile([128, T], FP)
        nc.scalar.activation(out=R(qdual[:]), in_=q_ps[:], func=AF.Identity, bias=bias_q[:])

        # ---------------- stage B: node -> kTn_dual, node_vT ----------------
        nn = lnat.tile([128, 4, D], FP, tag="xnat")
        nc.sync.dma_start(R(nn[:]), R(d_node[:].rearrange("(j p) d -> p j d", p=128)))
        nodeT = ltp.tile([128, ND, 512], FP, tag="lt")
        for i in range(N // 128):
            tp = ptr.tile([128, 512], FP, tag="tp")
            for dc in range(ND):
                nc.tensor.transpose(R(tp[:, dc * 128:(dc + 1) * 128]),
                                    R(nn[:, i, dc * 128:(dc + 1) * 128]), R(identR[:]))
            nc.vector.tensor_copy(R(nodeT[:, 0:ND, i * 128:(i + 1) * 128]),
                                  R(tp[:].rearrange("p (dc b) -> p dc b", b=128)))
        kTn_dual = big.tile([128, 256], FP)
        node_vT = big.tile([64, N], FP)            # node_v^T + bias_v (bias pre-folded)
        kvn_ps = pmm.tile([128, 512], FP, tag="mm", name="kvn_ps")
        for dc in range(ND):
            nc.tensor.matmul(kvn_ps[:], R(w_kv[:, dc, :]), R(nodeT[:, dc, :]),
                             start=(dc == 0), stop=(dc == ND - 1))
        for b in range(4):
            ro, co = (b % 2) * 64, (b // 2) * 128
            nc.scalar.activation(out=R(kTn_dual[ro:ro + 64, co:co + 128]),
                                 in_=kvn_ps[0:64, b * 128:(b + 1) * 128],
                                 func=AF.Identity, bias=bias_k[ro:ro + 64, :])
        nc.vector.tensor_scalar(out=node_vT[:], in0=kvn_ps[64:128, :],
                                scalar1=bias_v[:], scalar2=None, op0=OP.add)

        # remaining constants (needed only after the leaf pass); pool/DVE slot
        # these behind stage B's work
        tri_raw = work.tile([128, 128], FP, tag="traw")
        make_lower_triangular(nc, tri_raw[:], val=1.0, diag=True)
        tri128 = consts.tile([128, 128], FP)      # [m,l]=1 iff l<=m  (suffix lhsT)
        nc.vector.tensor_copy(R(tri128[:]), tri_raw[:])
        tri32s = consts.tile([32, 32], FP)        # [k,c]=1 iff k>c   (carry)
        make_lower_triangular(nc, tri32s[:], val=1.0, diag=False)
        G = consts.tile([128, 16], FP)            # G[m,j] = 1 iff m//8 == j
        nc.gpsimd.memset(G[:], 1.0)
        nc.gpsimd.affine_select(out=G[:], in_=G[:], compare_op=OP.is_ge, fill=0.0,
                                base=0, pattern=[[-BR, 16]], channel_multiplier=1)
        nc.gpsimd.affine_select(out=G[:], in_=G[:], compare_op=OP.is_ge, fill=0.0,
                                base=BR - 1, pattern=[[BR, 16]], channel_multiplier=-1)
        GT = consts.tile([16, 128], FP)
        nc.gpsimd.memset(GT[:], 1.0)
        nc.gpsimd.affine_select(out=GT[:], in_=GT[:], compare_op=OP.is_ge, fill=0.0,
                                base=0, pattern=[[1, 128]], channel_multiplier=-BR)
        nc.gpsimd.affine_select(out=GT[:], in_=GT[:], compare_op=OP.is_ge, fill=0.0,
                                base=BR - 1, pattern=[[-1, 128]], channel_multiplier=BR)
        ones_raw = consts.tile([128, 1], FP)
        nc.gpsimd.memset(ones_raw[:], 1.0)
        onesP = consts.tile([128, 64], FP)
        nc.vector.tensor_copy(R(onesP[:]), bass.AP(tensor=ones_raw[:].tensor,
                                                   offset=ones_raw[:].offset,
                                                   ap=[[1, 128], [0, 64]]))
        cnt3 = consts.tile([128, NC], FP)         # 3 * (L - l), l = 128*c + p
        nc.gpsimd.iota(cnt3[:], pattern=[[-3 * 128, NC]], base=3 * L,
                       channel_multiplier=-3, allow_small_or_imprecise_dtypes=True)
        inv3 = consts.tile([128, NC], FP)
        nc.vector.reciprocal(inv3[:], cnt3[:])

        # ------- stage C + F: leaf pass with pipelined leaf attention -------
        # kTdual: 512-chunk i -> rows (i%2)*64, cols (i//2)*512
        kTdual = big.tile([128, L // 2], FP)
        leafT = big.tile([128, ND, L], FP)     # persistent
        lgn = big.tile([128, NC], FP)          # leaf logits, natural layout
        tile12i = big.tile([64, L], FP)        # interp' = leaf_v + node_rep
        vcomb = big.tile([128, NC, 65], FP)    # [v(64) | ones] per 128-leaf chunk
        nc.vector.tensor_copy(R(vcomb[:, :, 64:65]),
                              bass.AP(tensor=ones_raw[:].tensor,
                                      offset=ones_raw[:].offset,
                                      ap=[[1, 128], [0, NC], [1, 1]]))
        o2_ps = pacc.tile([65, T], FP, tag="acc", name="o2_ps")
        totT = big.tile([64, NC], FP)          # per-chunk interp totals (for carries)
        att_q = deque()          # (ct, half) score work not yet emitted
        acc_q = deque()          # (el, b2) exp'd scores awaiting accumulate
        el_state = {"done": 0}

        def emit_score(ct, half):
            if "noatt" in BISECT:
                return
            ro2 = half * 64
            b2 = 8 * (ct // 4) + ct % 4 + 4 * half
            st = pmm.tile([128, T], FP, tag="mm")
            for h in range(2):
                hs = slice(h * 512, (h + 1) * 512)
                nc.tensor.matmul(st[:, hs],
                                 R(kTdual[ro2:ro2 + 64, ct * 128:(ct + 1) * 128]),
                                 R(qdual[ro2:ro2 + 64, hs]), start=True, stop=True,
                                 skip_group_check=True)
            el = epool.tile([128, T], FP, tag="el")
            nc.scalar.activation(out=R(el[:]), in_=st[:], func=AF.Exp, scale=SCALE)
            acc_q.append((el, b2))

        def emit_acc():
            if "noatt" in BISECT:
                if acc_q:
                    acc_q.popleft()
                return
            el, b2 = acc_q.popleft()
            for h in range(2):
                hs = slice(h * 512, (h + 1) * 512)
                nc.tensor.matmul(o2_ps[:, hs], R(vcomb[:, b2, 0:65]), R(el[:, hs]),
                                 start=(el_state["done"] == 0),
                                 stop=(el_state["done"] == 31),
                                 skip_group_check=True)
            el_state["done"] += 1

        for i in range(L // 512):
            ln = lnat.tile([128, 4, D], FP, tag="xnat")
            nc.sync.dma_start(R(ln[:]), R(d_leaf[i * 512:(i + 1) * 512, :]
                              .rearrange("(j p) d -> p j d", p=128)))
            for j in range(4):
                tp = ptr.tile([128, 512], FP, tag="tp")
                for dc in range(ND):
                    nc.tensor.transpose(R(tp[:, dc * 128:(dc + 1) * 128]),
                                        R(ln[:, j, dc * 128:(dc + 1) * 128]), R(identR[:]))
                nc.vector.tensor_copy(
                    R(leafT[:, 0:ND, (4 * i + j) * 128:(4 * i + j + 1) * 128]),
                    R(tp[:].rearrange("p (dc b) -> p dc b", b=128)))
                if att_q:
                    emit_score(*att_q.popleft())
                if len(acc_q) >= 2:
                    emit_acc()
            kv_ps = pmm.tile([128, 528], FP, tag="mm")
            for dc in range(ND):
                nc.tensor.matmul(kv_ps[:, 0:512], R(w_kv[:, dc, :]),
                                 R(leafT[:, dc, i * 512:(i + 1) * 512]),
                                 start=(dc == 0), stop=(dc == ND - 1),
                                 skip_group_check=True)
            if "nolg" not in BISECT:
                # one single (non-accumulating) matmul per (column, d-chunk);
                # the 4 d-chunk partials are summed on DVE below
                for cj in range(4):
                    c = 4 * i + cj
                    for dc in range(ND):
                        nc.tensor.matmul(kv_ps[:, 512 + 4 * cj + dc:513 + 4 * cj + dc],
                                         leafT[:, dc, c * 128:(c + 1) * 128],
                                         wagg_raw[:, dc:dc + 1],
                                         start=True, stop=True,
                                         skip_group_check=True)
            # drain kv_ps promptly: it shares the PSUM rotation with the scores
            ro, co = (i % 2) * 64, (i // 2) * 512
            sl = slice(i * 512, (i + 1) * 512)
            nc.scalar.activation(out=R(kTdual[ro:ro + 64, co:co + 512]),
                                 in_=kv_ps[0:64, 0:512], func=AF.Identity,
                                 bias=bias_k[ro:ro + 64, :])
            t12v = work.tile([64, 512], FP, tag="t12v")
            nc.vector.tensor_scalar(out=t12v[:], in0=kv_ps[64:128, 0:512],
                                    scalar1=bias_v[:], scalar2=None, op0=OP.add)
            if "nolg" in BISECT:
                nc.vector.memset(lgn[:, 4 * i:4 * i + 4], 0.0)
            else:
                nc.vector.tensor_reduce(
                    out=lgn[:, 4 * i:4 * i + 4],
                    in_=kv_ps[:, 512:528].rearrange("p (c d) -> p c d", d=4),
                    axis=AX.X, op=OP.add)
            while acc_q:
                emit_acc()
            # interp' = leaf_v + node_vT' replicated 8x along l
            nc.gpsimd.tensor_tensor(
                out=tile12i[:, sl].rearrange("f (n c) -> f n c", c=BR),
                in0=t12v[:].rearrange("f (n c) -> f n c", c=BR),
                in1=_rep_ap(node_vT[0:64, 64 * i:64 * (i + 1)], BR), op=OP.add)
            # v back to natural for the attention lhsT: 4x [64,128]->[128,64]
            tpv = ptr.tile([128, 512], FP, tag="tp")
            for c4 in range(4):
                nc.tensor.transpose(tpv[:, c4 * 64:(c4 + 1) * 64],
                                    t12v[:, c4 * 128:(c4 + 1) * 128],
                                    ident[0:64, 0:64])
            nc.vector.tensor_copy(
                R(vcomb[:, 4 * i:4 * i + 4, 0:64]),
                tpv[:, 0:256].rearrange("p (c f) -> p c f", f=64))
            nc.vector.tensor_reduce(out=totT[:, 4 * i:4 * i + 4],
                                    in_=tile12i[:, sl].rearrange("f (c m) -> f c m", m=128),
                                    axis=AX.X, op=OP.add)
            # queue the 4 leaf-attention units this chunk unlocks
            g, half = i // 2, i % 2
            for ct in range(4 * g, 4 * g + 4):
                att_q.append((ct, half))

        # ---- flush remaining attention; carries run concurrently on DVE/PE ----
        while att_q or acc_q:
            if att_q:
                emit_score(*att_q.popleft())
            if acc_q:
                emit_acc()
        tot_ps = ptr.tile([NC, 64], FP, tag="tp")
        nc.tensor.transpose(tot_ps[:], totT[:], ident[0:64, 0:64])
        totals = work.tile([NC, 64], FP, tag="tot")
        nc.scalar.activation(out=totals[:], in_=tot_ps[:], func=AF.Copy)
        carrT_ps = ptr.tile([64, NC], FP, tag="tp")
        nc.tensor.matmul(carrT_ps[:], totals[:], tri32s[:], start=True, stop=True)
        # interpT[f, 128c+127] += carryT[f, c]  (row 127 is in every suffix sum)
        last_rows = tile12i[:, 127::128]
        nc.vector.tensor_tensor(out=last_rows, in0=last_rows, in1=carrT_ps[:], op=OP.add)
        rt_ps = ptr.tile([F, 1], FP, tag="tp")
        nc.tensor.transpose(rt_ps[:], root_row[:], ident[0:1, 0:1])
        rootT3 = consts.tile([F, 1], FP)
        nc.scalar.activation(out=rootT3[:], in_=rt_ps[:], func=AF.Copy, scale=1.0 / 3.0)

        o2_sb = big.tile([65, T], FP)
        if "noatt" in BISECT:
            nc.vector.memset(o2_sb[:], 1.0)
        else:
            nc.vector.tensor_copy(o2_sb[:], o2_ps[:])
        fsr = fin.tile([65, T], FP, tag="fsr")  # rows 0/32/64: 1/Z1, 1/Z2, 1/Z3
        nc.vector.reciprocal(R(fsr[32:33, :]), o2_sb[64:65, :])

        # ---------------- logits -> group-softmax weights ----------------
        e_all = work.tile([128, NC], FP, tag="e_all")
        nc.scalar.activation(out=e_all[:], in_=lgn[:], func=AF.Exp, bias=bagg_b[:])
        s_ps = pmm.tile([16, NC], FP, tag="mm", name="s_ps")
        nc.tensor.matmul(s_ps[:], G[:], e_all[:], start=True, stop=True)
        sinv = work.tile([16, NC], FP, tag="sinv")
        nc.vector.reciprocal(sinv[:], s_ps[:])
        r_ps = pmm.tile([128, NC], FP, tag="mm", name="r_ps")
        nc.tensor.matmul(r_ps[:], GT[:], sinv[:], start=True, stop=True)
        w_all = work.tile([128, NC], FP, tag="w_all")
        nc.vector.tensor_tensor(out=w_all[:], in0=e_all[:], in1=r_ps[:], op=OP.mult)

        # ------- suffix-mean + node_hat, with node attention interleaved -------
        nh_nat = big.tile([128, 4, 65], FP)
        nc.vector.tensor_copy(R(nh_nat[:, :, 64:65]),
                              bass.AP(tensor=ones_raw[:].tensor,
                                      offset=ones_raw[:].offset,
                                      ap=[[1, 128], [0, 4], [1, 1]]))
        wblk = big.tile([128, 8, 16], FP)      # per-chunk G-masked weights, rotating
        o1_ps = pacc.tile([65, T], FP, tag="acc", name="o1_ps")

        def emit_node_unit(b):
            ro, co = (b % 2) * 64, (b // 2) * 128
            st = pmm.tile([128, T], FP, tag="mm")
            for h in range(2):
                hs = slice(h * 512, (h + 1) * 512)
                nc.tensor.matmul(st[:, hs], R(kTn_dual[ro:ro + 64, co:co + 128]),
                                 R(qdual[ro:ro + 64, hs]), start=True, stop=True,
                                 skip_group_check=True)
            en = epool.tile([128, T], FP, tag="el")
            nc.scalar.activation(out=R(en[:]), in_=st[:], func=AF.Exp, scale=SCALE)
            for h in range(2):
                hs = slice(h * 512, (h + 1) * 512)
                nc.tensor.matmul(o1_ps[:, hs], R(nh_nat[:, b, 0:65]), R(en[:, hs]),
                                 start=(b == 0), stop=(b == 3), skip_group_check=True)

        for g in range(NC // 8):
            # interp chunks back to natural: 8x [64,128]->[128,64]
            tpi = ptr.tile([128, 512], FP, tag="tp")
            for jc in range(8):
                c = 8 * g + jc
                nc.tensor.transpose(tpi[:, jc * 64:(jc + 1) * 64],
                                    tile12i[:, c * 128:(c + 1) * 128],
                                    ident[0:64, 0:64])
            icomb = work.tile([128, 8, 64], FP, tag="icomb")
            nc.vector.tensor_copy(R(icomb[:].rearrange("p c f -> p (c f)")), tpi[:])
            sfx_ps = pmm.tile([128, 8, 64], FP, tag="mm")
            nc.tensor.matmul(sfx_ps[:], R(tri128[:]), R(icomb[:]), start=True, stop=True,
                             skip_group_check=True)
            upw4 = work.tile([128, 8, 64], FP, tag="upw")
            nc.vector.tensor_tensor(out=R(upw4[:]), in0=sfx_ps[:],
                                    in1=_rep_ap(inv3[:, 8 * g:8 * g + 8], 64),
                                    op=OP.mult)
            # nh^T[f, 16-block c] = upw_c^T @ (G * w_all[:,c]) - disjoint out slices
            for jc in range(8):
                c = 8 * g + jc
                nc.gpsimd.tensor_scalar(out=R(wblk[:, jc, :]), in0=G[:],
                                         scalar1=w_all[:, c:c + 1],
                                         scalar2=None, op0=OP.mult)
            nhT_ps = pmm.tile([64, 128], FP, tag="mm")
            for jc in range(8):
                nc.tensor.matmul(nhT_ps[0:64, 16 * jc:16 * jc + 16], R(upw4[:, jc, :]),
                                 R(wblk[:, jc, :]), start=True, stop=True,
                                 skip_group_check=True)
            nhT_sb = work.tile([64, 128], FP, tag="nhT")
            nc.scalar.activation(out=R(nhT_sb[:]), in_=nhT_ps[:], func=AF.Copy)
            nhn_ps = pmm.tile([128, 64], FP, tag="mm")
            nc.tensor.transpose(R(nhn_ps[:]), R(nhT_sb[:]), R(identR[0:64, 0:64]))
            nc.vector.tensor_copy(R(nh_nat[:, g, 0:64]), R(nhn_ps[:]))
            emit_node_unit(g)
        # ------- combine + final softmax over F, pipelined in T/4 slices -------
        b2 = pmm.tile([64, T], FP, tag="mm", name="b2")
        for h in range(2):
            hs = slice(h * 512, (h + 1) * 512)
            nc.tensor.matmul(b2[:, hs], R(onesP[32:33, 0:64]), R(fsr[32:33, hs]),
                             start=True, stop=True, skip_group_check=True)
        nc.vector.reciprocal(R(fsr[0:1, :]), o1_ps[64:65, :])
        b1 = pmm.tile([64, T], FP, tag="mm", name="b1")
        for h in range(2):
            hs = slice(h * 512, (h + 1) * 512)
            nc.tensor.matmul(b1[:, hs], R(onesP[0:1, 0:64]), R(fsr[0:1, hs]),
                             start=True, stop=True, skip_group_check=True)
        o1_sb = big.tile([64, T], FP)
        x1 = fin.tile([64, T], FP, tag="x1")
        x2 = fin.tile([64, T], FP, tag="x2")
        s12 = fin.tile([64, T], FP, tag="s12")
        e3 = fin.tile([64, T], FP, tag="e3")
        outT = big.tile([64, T], FP)
        onat = big.tile([128, T // 128, F], FP)
        for hq in range(4):
            q = slice(hq * TQ, (hq + 1) * TQ)
            nc.vector.tensor_tensor(out=x2[:, q], in0=o2_sb[0:64, q], in1=b2[:, q],
                                    op=OP.mult)
            nc.vector.tensor_copy(o1_sb[:, q], o1_ps[0:64, q])
            nc.vector.tensor_tensor(out=x1[:, q], in0=o1_sb[:, q], in1=b1[:, q],
                                    op=OP.mult)
            nc.gpsimd.tensor_tensor(out=s12[:, q], in0=x1[:, q], in1=x2[:, q],
                                    op=OP.add)
            nc.scalar.activation(out=R(e3[:, q]), in_=s12[:, q], func=AF.Exp,
                                 bias=rootT3[:])
            z3 = ptr.tile([1, TQ], FP, tag="tp")
            nc.tensor.matmul(z3[:], R(onesP[0:64, 0:1]), R(e3[:, q]),
                             start=True, stop=True, skip_group_check=True)
            nc.vector.reciprocal(R(fsr[64:65, q]), z3[:])
            b3 = ptr.tile([64, TQ], FP, tag="tp")
            nc.tensor.matmul(b3[:], R(onesP[64:65, 0:64]), R(fsr[64:65, q]),
                             start=True, stop=True, skip_group_check=True)
            nc.vector.tensor_tensor(out=outT[:, q], in0=e3[:, q], in1=b3[:], op=OP.mult)
            op_ = ptr.tile([128, 512], FP, tag="tp")
            for k in (2 * hq, 2 * hq + 1):
                nc.tensor.transpose(op_[:, (k % 2) * 64:(k % 2) * 64 + 64],
                                    outT[:, k * 128:(k + 1) * 128],
                                    ident[0:64, 0:64])
            nc.vector.tensor_copy(onat[:, 2 * hq:2 * hq + 2, :]
                                  .rearrange("p k f -> p (k f)"), op_[:, 0:128])
            if hq % 2 == 1:
                nc.sync.dma_start(
                    d_out[(hq - 1) * 256:(hq + 1) * 256, :]
                    .rearrange("(k p) f -> p k f", p=128),
                    onat[:, 2 * (hq - 1):2 * (hq + 1), :])


_NC_CACHE = None


def kernel(**inputs):
    global _NC_CACHE
    if _NC_CACHE is None:
        _NC_CACHE = build_nc()
    nc = _NC_CACHE
    shared = {k: np.ascontiguousarray(np.asarray(inputs[k], dtype=np.float32))
              for k in ("Wq", "bq", "Wk", "bk", "Wv", "bv", "Wagg", "bagg")}
    in_maps = []
    for b in range(B):
        m = dict(shared)
        m["root"] = np.ascontiguousarray(np.asarray(inputs["root"][b], dtype=np.float32))
        m["node"] = np.ascontiguousarray(np.asarray(inputs["node"][b], dtype=np.float32))
        m["leaf"] = np.ascontiguousarray(np.asarray(inputs["leaf"][b], dtype=np.float32))
        m["target"] = np.ascontiguousarray(np.asarray(inputs["target"][b], dtype=np.float32))
        in_maps.append(m)
    res = run_bass_kernel_spmd(nc, in_maps, core_ids=list(range(B)))
    return np.stack([r["out"] for r in res.results], axis=0)


# revision 52
# speedup vs baseline: 2.3395x; 1.0420x over previous
"""Trainium2 Bass kernel for nn_DecoderAttention (dual-key tree decoder attention).

Sharding: data-parallel over batch B=8, one batch element per NeuronCore.

Per-core computation (B-slice), fp32 data with fp32r (split-bf16) matmuls:
  q = target @ Wq + bq                     [T,F]   (kept transposed, duplicated on 128 partitions)
  k/v (node, leaf) = x @ {Wk,Wv} + b       (kept transposed [F, *] via PE-transposed inputs)
  logits = leaf @ Wagg + bagg              [L,1]   (PE: leafT-chunk stationary x Wagg column)
  Aqn/Aql softmaxes are computed unnormalized (exp, no max-subtraction: |scores/8| <~ 1.2)
  out_pre = (En^T @ [nh|1])/Z1 + (El^T @ [v|1])/Z2 + root/3
  out = softmax_F(out_pre)                 [T,F]
The tree interpolation's root term commutes through the suffix-mean and the
attention average (softmax weights sum to 1), so root/3 is added once at the end.
Suffix cumsum over L: per-128-chunk triangular matmuls (batched 4 chunks / matmul);
the cross-chunk carries are folded into the LAST ROW of each interp chunk before
the in-chunk suffix (row 127 participates in every suffix sum of its chunk).

Schedule: the leaf-attention score/exp/accumulate stream (the dominant
Activation-engine load) is software-pipelined into the leaf projection loop —
each 512-leaf chunk queues the 4 (score, exp, accumulate) units it unlocks and
drains the queue one chunk behind, so the Activation engine runs continuously
while PE fills its gaps with the next chunk's transposes/projections. Only the
node path (suffix mean -> node_hat -> node attention) waits for the full leaf
pass (it needs the global suffix carries); node attention is interleaved into
the suffix loop, and the final combine/softmax is pipelined in T/4 slices.
PSUM budget (8 banks): pmm 2x[128,1024] + ptr 2x[128,512] + pacc 1x[65,1024],
with the pacc buffer sequenced o2 -> logits -> o1 -> z3.
"""

import os
import sys
from collections import deque

import numpy as np

for _p in ("/opt/trn_rl_repo", "/root/.axon_site/_ro/trn_rl_repo"):
    if os.path.isdir(_p) and _p not in sys.path:
        sys.path.insert(0, _p)

import concourse.bass as bass
import concourse.tile as tile
from concourse import bacc
from concourse import mybir
from concourse.bass_utils import run_bass_kernel_spmd
from concourse.masks import make_identity, make_lower_triangular

FP = mybir.dt.float32
FR = mybir.dt.float32r
BF = mybir.dt.bfloat16
AF = mybir.ActivationFunctionType
OP = mybir.AluOpType
AX = mybir.AxisListType

B, T, N, L, D, F = 8, 1024, 512, 4096, 512, 64
BR = L // N          # 8 leaves per node
NC = L // 128        # 32 leaf chunks of 128
ND = D // 128        # 4 contraction chunks
TQ = T // 4          # final-stage pipeline slice
SCALE = 1.0 / float(np.sqrt(F))


def R(ap):
    """View an fp32 AP as float32r (full-rate PE matmuls, split-bf16 numerics)."""
    return ap.bitcast(FR)


def _rep_ap(ap, rep):
    """Append a step-0 innermost free dim (read each element `rep` times)."""
    return bass.AP(tensor=ap.tensor, offset=ap.offset, ap=list(ap.ap) + [[0, rep]])


def build_nc():
    nc = bacc.Bacc("TRN2", target_bir_lowering=False, debug=False)

    d_root = nc.dram_tensor("root", [1, F], FP, kind="ExternalInput")
    d_node = nc.dram_tensor("node", [N, D], FP, kind="ExternalInput")
    d_leaf = nc.dram_tensor("leaf", [L, D], FP, kind="ExternalInput")
    d_target = nc.dram_tensor("target", [T, D], FP, kind="ExternalInput")
    d_wq = nc.dram_tensor("Wq", [D, F], FP, kind="ExternalInput")
    d_bq = nc.dram_tensor("bq", [F], FP, kind="ExternalInput")
    d_wk = nc.dram_tensor("Wk", [D, F], FP, kind="ExternalInput")
    d_bk = nc.dram_tensor("bk", [F], FP, kind="ExternalInput")
    d_wv = nc.dram_tensor("Wv", [D, F], FP, kind="ExternalInput")
    d_bv = nc.dram_tensor("bv", [F], FP, kind="ExternalInput")
    d_wagg = nc.dram_tensor("Wagg", [D, 1], FP, kind="ExternalInput")
    d_bagg = nc.dram_tensor("bagg", [1], FP, kind="ExternalInput")
    d_out = nc.dram_tensor("out", [T, F], FP, kind="ExternalOutput")

    with tile.TileContext(nc) as tc:
        _emit(nc, tc, d_root, d_node, d_leaf, d_target, d_wq, d_bq, d_wk, d_bk,
              d_wv, d_bv, d_wagg, d_bagg, d_out)
    nc.compile()
    return nc


def _emit(nc, tc, d_root, d_node, d_leaf, d_target, d_wq, d_bq, d_wk, d_bk,
          d_wv, d_bv, d_wagg, d_bagg, d_out):
    from contextlib import ExitStack

    with ExitStack() as ctx:
        ctx.enter_context(nc.allow_low_precision(
            reason="fp32r rounding on matmul operands is intentional"))
        consts = ctx.enter_context(tc.tile_pool(name="consts", bufs=1))
        big = ctx.enter_context(tc.tile_pool(name="big", bufs=1))
        lnat = ctx.enter_context(tc.tile_pool(name="lnat", bufs=3))
        ltp = ctx.enter_context(tc.tile_pool(name="ltp", bufs=2))
        work = ctx.enter_context(tc.tile_pool(name="work", bufs=2))
        fin = ctx.enter_context(tc.tile_pool(name="fin", bufs=1))
        epool = ctx.enter_context(tc.tile_pool(name="epool", bufs=3))
        ptr = ctx.enter_context(tc.tile_pool(name="ptr", bufs=2, space="PSUM"))
        pmm = ctx.enter_context(tc.tile_pool(name="pmm", bufs=2, space="PSUM"))
        pacc = ctx.enter_context(tc.tile_pool(name="pacc", bufs=1, space="PSUM"))

        # ---- identity only (gates all transposes); other consts deferred ----
        ident = consts.tile([128, 128], FP)
        make_identity(nc, ident[:])
        identR = consts.tile([128, 128], FP)   # fp32r-rounded copy for R-transposes
        nc.vector.tensor_copy(R(identR[:]), ident[:])

        # ---------------- stage A: target -> qdual [128, 1024] ----------------
        tns = []
        tn = lnat.tile([128, 4, D], FP, tag="xnat")
        nc.sync.dma_start(R(tn[:]), R(d_target[0:512, :]
                          .rearrange("(j p) d -> p j d", p=128)))
        tns.append(tn)

        # weights / biases; w_qq queued early (gates qdual), w_kv after leaf0
        w_kv = consts.tile([128, ND, 128], FP)     # cols 0:64 Wk, 64:128 Wv per d-chunk
        w_qq = consts.tile([128, ND, 128], FP)     # Wq duplicated
        wagg_raw = consts.tile([128, ND], FP)
        bias_q = consts.tile([128, 1], FP)
        bias_k = consts.tile([128, 1], FP)
        bias_v = consts.tile([64, 1], FP)
        bq2 = d_bq[:].rearrange("(f o) -> f o", o=1)
        bk2 = d_bk[:].rearrange("(f o) -> f o", o=1)
        nc.sync.dma_start(bias_q[0:F, :], bq2)
        nc.sync.dma_start(bias_q[F:128, :], bq2)
        nc.sync.dma_start(bias_k[0:F, :], bk2)
        nc.sync.dma_start(bias_k[F:128, :], bk2)
        nc.sync.dma_start(bias_v[:], d_bv[:].rearrange("(f o) -> f o", o=1))
        bagg_b = consts.tile([128, 1], FP)
        _ba = d_bagg[:]
        nc.gpsimd.dma_start(bagg_b[:], bass.AP(tensor=_ba.tensor, offset=_ba.offset,
                                               ap=[[0, 128], [1, 1]]))
        root_row = consts.tile([1, F], FP)
        nc.sync.dma_start(root_row[:], d_root[:])

        ln_pre = deque()

        def prefetch_leaf(i):
            ln = lnat.tile([128, 4, D], FP, tag="xnat")
            nc.sync.dma_start(R(ln[:]), R(d_leaf[i * 512:(i + 1) * 512, :]
                              .rearrange("(j p) d -> p j d", p=128)))
            ln_pre.append(ln)

        nc.sync.dma_start(R(w_qq[:, :, 0:F]), R(d_wq[:].rearrange("(j p) f -> p j f", p=128)))
        nc.sync.dma_start(R(w_qq[:, :, F:128]), R(d_wq[:].rearrange("(j p) f -> p j f", p=128)))
        tn = lnat.tile([128, 4, D], FP, tag="xnat")
        nc.sync.dma_start(R(tn[:]), R(d_target[512:1024, :]
                          .rearrange("(j p) d -> p j d", p=128)))
        tns.append(tn)
        prefetch_leaf(0)
        nc.sync.dma_start(R(w_kv[:, :, 0:F]), R(d_wk[:].rearrange("(j p) f -> p j f", p=128)))
        nc.sync.dma_start(R(w_kv[:, :, F:128]), R(d_wv[:].rearrange("(j p) f -> p j f", p=128)))
        nc.sync.dma_start(wagg_raw[:], d_wagg[:].rearrange("(j p) o -> p (j o)", p=128))
        q_ps = pmm.tile([128, T], FP, tag="mm", name="q_ps")
        for ib in range(T // 512):
            targT = ltp.tile([128, ND, 512], FP, tag="lt")
            for j in range(4):
                tp = ptr.tile([128, 512], FP, tag="tp")
                for dc in range(ND):
                    nc.tensor.transpose(R(tp[:, dc * 128:(dc + 1) * 128]),
                                        R(tns[ib][:, j, dc * 128:(dc + 1) * 128]),
                                        R(identR[:]))
                if j % 2 == 0:
                    nc.vector.tensor_copy(R(targT[:, 0:ND, j * 128:(j + 1) * 128]),
                                          R(tp[:].rearrange("p (dc b) -> p dc b", b=128)))
                else:
                    nc.scalar.activation(
                        out=R(targT[:, 0:ND, j * 128:(j + 1) * 128]),
                        in_=R(tp[:].rearrange("p (dc b) -> p dc b", b=128)),
                        func=AF.Copy)
            for dc in range(ND):
                nc.tensor.matmul(q_ps[:, ib * 512:(ib + 1) * 512], R(w_qq[:, dc, :]),
                                 R(targT[:, dc, :]), start=(dc == 0), stop=(dc == ND - 1),
                                 skip_group_check=True)
        qdual = big.tile([128, T], FP)
        nc.scalar.activation(out=R(qdual[:]), in_=q_ps[:], func=AF.Identity, bias=bias_q[:])

        # ---------------- stage B: node -> kTn_dual, node_vT ----------------
        prefetch_leaf(1)
        nn = lnat.tile([128, 4, D], FP, tag="xnat")
        nc.sync.dma_start(R(nn[:]), R(d_node[:].rearrange("(j p) d -> p j d", p=128)))
        nodeT = ltp.tile([128, ND, 512], FP, tag="lt")
        for i in range(N // 128):
            tp = ptr.tile([128, 512], FP, tag="tp")
            for dc in range(ND):
                nc.tensor.transpose(R(tp[:, dc * 128:(dc + 1) * 128]),
                                    R(nn[:, i, dc * 128:(dc + 1) * 128]), R(identR[:]))
            if i % 2 == 0:
                nc.vector.tensor_copy(R(nodeT[:, 0:ND, i * 128:(i + 1) * 128]),
                                      R(tp[:].rearrange("p (dc b) -> p dc b", b=128)))
            else:
                nc.scalar.activation(
                    out=R(nodeT[:, 0:ND, i * 128:(i + 1) * 128]),
                    in_=R(tp[:].rearrange("p (dc b) -> p dc b", b=128)),
                    func=AF.Copy)
        kTn_dual = big.tile([128, 256], FP)
        node_vT = big.tile([64, N], FP)            # node_v^T + bias_v (bias pre-folded)
        kvn_ps = pmm.tile([128, 512], FP, tag="mm", name="kvn_ps")
        for dc in range(ND):
            nc.tensor.matmul(kvn_ps[:], R(w_kv[:, dc, :]), R(nodeT[:, dc, :]),
                             start=(dc == 0), stop=(dc == ND - 1))
        for b in range(4):
            ro, co = (b % 2) * 64, (b // 2) * 128
            nc.scalar.activation(out=R(kTn_dual[ro:ro + 64, co:co + 128]),
                                 in_=kvn_ps[0:64, b * 128:(b + 1) * 128],
                                 func=AF.Identity, bias=bias_k[ro:ro + 64, :])
        nc.vector.tensor_scalar(out=node_vT[:], in0=kvn_ps[64:128, :],
                                scalar1=bias_v[:], scalar2=None, op0=OP.add)

        # remaining constants (needed only after the leaf pass); pool/DVE slot
        # these behind stage B's work
        tri_raw = work.tile([128, 128], FP, tag="traw")
        make_lower_triangular(nc, tri_raw[:], val=1.0, diag=True)
        tri128 = consts.tile([128, 128], FP)      # [m,l]=1 iff l<=m  (suffix lhsT)
        nc.vector.tensor_copy(R(tri128[:]), tri_raw[:])
        tri32s = consts.tile([32, 32], FP)        # [k,c]=1 iff k>c   (carry)
        make_lower_triangular(nc, tri32s[:], val=1.0, diag=False)
        G = consts.tile([128, 16], FP)            # G[m,j] = 1 iff m//8 == j
        nc.gpsimd.memset(G[:], 1.0)
        nc.gpsimd.affine_select(out=G[:], in_=G[:], compare_op=OP.is_ge, fill=0.0,
                                base=0, pattern=[[-BR, 16]], channel_multiplier=1)
        nc.gpsimd.affine_select(out=G[:], in_=G[:], compare_op=OP.is_ge, fill=0.0,
                                base=BR - 1, pattern=[[BR, 16]], channel_multiplier=-1)
        GT = consts.tile([16, 128], FP)
        nc.gpsimd.memset(GT[:], 1.0)
        nc.gpsimd.affine_select(out=GT[:], in_=GT[:], compare_op=OP.is_ge, fill=0.0,
                                base=0, pattern=[[1, 128]], channel_multiplier=-BR)
        nc.gpsimd.affine_select(out=GT[:], in_=GT[:], compare_op=OP.is_ge, fill=0.0,
                                base=BR - 1, pattern=[[-1, 128]], channel_multiplier=BR)
        ones_raw = consts.tile([128, 1], FP)
        nc.gpsimd.memset(ones_raw[:], 1.0)
        onesP = consts.tile([128, 64], FP)
        nc.vector.tensor_copy(R(onesP[:]), bass.AP(tensor=ones_raw[:].tensor,
                                                   offset=ones_raw[:].offset,
                                                   ap=[[1, 128], [0, 64]]))
        cnt3 = consts.tile([128, NC], FP)         # 3 * (L - l), l = 128*c + p
        nc.gpsimd.iota(cnt3[:], pattern=[[-3 * 128, NC]], base=3 * L,
                       channel_multiplier=-3, allow_small_or_imprecise_dtypes=True)
        inv3 = consts.tile([128, NC], FP)
        nc.vector.reciprocal(inv3[:], cnt3[:])

        # ------- stage C + F: leaf pass with pipelined leaf attention -------
        # kTdual: 512-chunk i -> rows (i%2)*64, cols (i//2)*512
        kTdual = big.tile([128, L // 2], FP)
        leafT = big.tile([128, ND, L], FP)     # persistent
        lgn = big.tile([128, NC], FP)          # leaf logits, natural layout
        tile12i = big.tile([64, L], FP)        # interp' = leaf_v + node_rep
        vcomb = big.tile([128, NC, 65], BF)    # [v(64) | ones] per 128-leaf chunk
        nc.vector.tensor_copy(vcomb[:, :, 64:65],
                              bass.AP(tensor=ones_raw[:].tensor,
                                      offset=ones_raw[:].offset,
                                      ap=[[1, 128], [0, NC], [1, 1]]))
        o2_ps = pacc.tile([65, T], FP, tag="acc", name="o2_ps")
        totT = big.tile([64, NC], FP)          # per-chunk interp totals (for carries)
        att_q = deque()          # (ct, half) score work not yet emitted
        acc_q = deque()          # (el, b2) exp'd scores awaiting accumulate
        el_state = {"done": 0}

        def emit_score(ct, half):
            ro2 = half * 64
            b2 = 8 * (ct // 4) + ct % 4 + 4 * half
            st = pmm.tile([128, T], FP, tag="mm")
            for h in range(2):
                hs = slice(h * 512, (h + 1) * 512)
                nc.tensor.matmul(st[:, hs],
                                 R(kTdual[ro2:ro2 + 64, ct * 128:(ct + 1) * 128]),
                                 R(qdual[ro2:ro2 + 64, hs]), start=True, stop=True,
                                 skip_group_check=True)
            el = epool.tile([128, T], BF, tag="el")
            nc.scalar.activation(out=el[:], in_=st[:], func=AF.Exp, scale=SCALE)
            acc_q.append((el, b2))

        def emit_acc():
            el, b2 = acc_q.popleft()
            for h in range(2):
                hs = slice(h * 512, (h + 1) * 512)
                nc.tensor.matmul(o2_ps[:, hs], vcomb[:, b2, 0:65], el[:, hs],
                                 start=(el_state["done"] == 0),
                                 stop=(el_state["done"] == 31),
                                 skip_group_check=True)
            el_state["done"] += 1

        for i in range(L // 512):
            ln = ln_pre.popleft() if ln_pre else None
            if ln is None:
                ln = lnat.tile([128, 4, D], FP, tag="xnat")
                nc.sync.dma_start(R(ln[:]), R(d_leaf[i * 512:(i + 1) * 512, :]
                                  .rearrange("(j p) d -> p j d", p=128)))
            if i + 2 < L // 512:
                prefetch_leaf(i + 2)
            for j in range(4):
                tp = ptr.tile([128, 512], FP, tag="tp")
                for dc in range(ND):
                    nc.tensor.transpose(R(tp[:, dc * 128:(dc + 1) * 128]),
                                        R(ln[:, j, dc * 128:(dc + 1) * 128]), R(identR[:]))
                if j % 2 == 0:
                    nc.vector.tensor_copy(
                        R(leafT[:, 0:ND, (4 * i + j) * 128:(4 * i + j + 1) * 128]),
                        R(tp[:].rearrange("p (dc b) -> p dc b", b=128)))
                else:
                    nc.scalar.activation(
                        out=R(leafT[:, 0:ND, (4 * i + j) * 128:(4 * i + j + 1) * 128]),
                        in_=R(tp[:].rearrange("p (dc b) -> p dc b", b=128)),
                        func=AF.Copy)
                if att_q:
                    emit_score(*att_q.popleft())
                if j >= 2 and len(acc_q) >= 3:
                    emit_acc()
            kv_ps = pmm.tile([128, 528], FP, tag="mm")
            for dc in range(ND):
                nc.tensor.matmul(kv_ps[:, 0:512], R(w_kv[:, dc, :]),
                                 R(leafT[:, dc, i * 512:(i + 1) * 512]),
                                 start=(dc == 0), stop=(dc == ND - 1),
                                 skip_group_check=True)
            # logits: one single (non-accumulating) matmul per (column, d-chunk);
            # nested accumulation groups wedge the device, so the 4 d-chunk
            # partials land in separate columns and are summed on DVE below
            for cj in range(4):
                c = 4 * i + cj
                for dc in range(ND):
                    nc.tensor.matmul(kv_ps[:, 512 + 4 * cj + dc:513 + 4 * cj + dc],
                                     leafT[:, dc, c * 128:(c + 1) * 128],
                                     wagg_raw[:, dc:dc + 1],
                                     start=True, stop=True,
                                     skip_group_check=True)
            # drain kv_ps promptly: it shares the PSUM rotation with the scores
            ro, co = (i % 2) * 64, (i // 2) * 512
            sl = slice(i * 512, (i + 1) * 512)
            nc.vector.tensor_scalar(out=R(kTdual[ro:ro + 64, co:co + 512]),
                                    in0=kv_ps[0:64, 0:512], scalar1=bias_k[ro:ro + 64, :],
                                    scalar2=None, op0=OP.add)
            t12v = work.tile([64, 512], FP, tag="t12v")
            nc.vector.tensor_scalar(out=t12v[:], in0=kv_ps[64:128, 0:512],
                                    scalar1=bias_v[:], scalar2=None, op0=OP.add)
            nc.vector.tensor_reduce(
                out=lgn[:, 4 * i:4 * i + 4],
                in_=kv_ps[:, 512:528].rearrange("p (c d) -> p c d", d=4),
                axis=AX.X, op=OP.add)
            while acc_q:
                emit_acc()
            # interp' = leaf_v + node_vT' replicated 8x along l
            nc.gpsimd.tensor_tensor(
                out=tile12i[:, sl].rearrange("f (n c) -> f n c", c=BR),
                in0=t12v[:].rearrange("f (n c) -> f n c", c=BR),
                in1=_rep_ap(node_vT[0:64, 64 * i:64 * (i + 1)], BR), op=OP.add)
            # v back to natural for the attention lhsT: 4x [64,128]->[128,64]
            tpv = ptr.tile([128, 512], FP, tag="tp")
            for c4 in range(4):
                nc.tensor.transpose(tpv[:, c4 * 64:(c4 + 1) * 64],
                                    t12v[:, c4 * 128:(c4 + 1) * 128],
                                    ident[0:64, 0:64])
            nc.vector.tensor_copy(
                vcomb[:, 4 * i:4 * i + 4, 0:64],
                tpv[:, 0:256].rearrange("p (c f) -> p c f", f=64))
            nc.vector.tensor_reduce(out=totT[:, 4 * i:4 * i + 4],
                                    in_=tile12i[:, sl].rearrange("f (c m) -> f c m", m=128),
                                    axis=AX.X, op=OP.add)
            # queue the 4 leaf-attention units this chunk unlocks
            g, half = i // 2, i % 2
            for ct in range(4 * g, 4 * g + 4):
                att_q.append((ct, half))

        node_en = {}

        def emit_node_score(b):
            ro, co = (b % 2) * 64, (b // 2) * 128
            st = pmm.tile([128, T], FP, tag="mm")
            for h in range(2):
                hs = slice(h * 512, (h + 1) * 512)
                nc.tensor.matmul(st[:, hs], R(kTn_dual[ro:ro + 64, co:co + 128]),
                                 R(qdual[ro:ro + 64, hs]), start=True, stop=True,
                                 skip_group_check=True)
            en = epool.tile([128, T], BF, tag="el")
            nc.scalar.activation(out=en[:], in_=st[:], func=AF.Exp, scale=SCALE)
            node_en[b] = en

        def emit_node_acc(b):
            en = node_en.pop(b)
            for h in range(2):
                hs = slice(h * 512, (h + 1) * 512)
                nc.tensor.matmul(o1_ps[:, hs], nh_nat[:, b, 0:65], en[:, hs],
                                 start=(b == 0), stop=(b == 3), skip_group_check=True)

        # ---- flush remaining attention; carries run concurrently on DVE/PE ----
        while att_q or acc_q:
            if att_q:
                emit_score(*att_q.popleft())
            if acc_q:
                emit_acc()
        emit_node_score(0)
        tot_ps = ptr.tile([NC, 64], FP, tag="tp")
        nc.tensor.transpose(tot_ps[:], totT[:], ident[0:64, 0:64])
        totals = work.tile([NC, 64], FP, tag="tot")
        nc.scalar.activation(out=totals[:], in_=tot_ps[:], func=AF.Copy)
        carrT_ps = ptr.tile([64, NC], FP, tag="tp")
        nc.tensor.matmul(carrT_ps[:], totals[:], tri32s[:], start=True, stop=True)
        # interpT[f, 128c+127] += carryT[f, c]  (row 127 is in every suffix sum)
        last_rows = tile12i[:, 127::128]
        nc.vector.tensor_tensor(out=last_rows, in0=last_rows, in1=carrT_ps[:], op=OP.add)
        rt_ps = ptr.tile([F, 1], FP, tag="tp")
        nc.tensor.transpose(rt_ps[:], root_row[:], ident[0:1, 0:1])
        rootT3 = consts.tile([F, 1], FP)
        nc.scalar.activation(out=rootT3[:], in_=rt_ps[:], func=AF.Copy, scale=1.0 / 3.0)
        # ---------------- logits -> group-softmax weights ----------------
        e_all = work.tile([128, NC], FP, tag="e_all")
        nc.scalar.activation(out=e_all[:], in_=lgn[:], func=AF.Exp, bias=bagg_b[:])
        s_ps = pmm.tile([16, NC], FP, tag="mm", name="s_ps")
        nc.tensor.matmul(s_ps[:], G[:], e_all[:], start=True, stop=True)
        sinv = work.tile([16, NC], FP, tag="sinv")
        nc.vector.reciprocal(sinv[:], s_ps[:])
        r_ps = pmm.tile([128, NC], FP, tag="mm", name="r_ps")
        nc.tensor.matmul(r_ps[:], GT[:], sinv[:], start=True, stop=True)
        w_all = work.tile([128, NC], FP, tag="w_all")
        nc.vector.tensor_tensor(out=w_all[:], in0=e_all[:], in1=r_ps[:], op=OP.mult)

        o2_sb = big.tile([65, T], FP)
        nc.vector.tensor_copy(o2_sb[:], o2_ps[:])
        fsr = fin.tile([65, T], FP, tag="fsr")  # rows 0/32/64: 1/Z1, 1/Z2, 1/Z3
        nc.vector.reciprocal(R(fsr[32:33, :]), o2_sb[64:65, :])


        # o2-dependent half of the final combine runs during the node phase
        b2 = pmm.tile([64, T], FP, tag="mm", name="b2")
        for h in range(2):
            hs = slice(h * 512, (h + 1) * 512)
            nc.tensor.matmul(b2[:, hs], R(onesP[32:33, 0:64]), R(fsr[32:33, hs]),
                             start=True, stop=True, skip_group_check=True)
        x2 = fin.tile([64, T], FP, tag="x2")
        for h in range(2):
            hs = slice(h * 512, (h + 1) * 512)
            nc.vector.tensor_tensor(out=x2[:, hs], in0=o2_sb[0:64, hs],
                                    in1=b2[:, hs], op=OP.mult)

        # ------- suffix-mean + node_hat, with node attention interleaved -------
        nh_nat = big.tile([128, 4, 65], BF)
        nc.vector.tensor_copy(nh_nat[:, :, 64:65],
                              bass.AP(tensor=ones_raw[:].tensor,
                                      offset=ones_raw[:].offset,
                                      ap=[[1, 128], [0, 4], [1, 1]]))
        wblk = big.tile([128, 8, 16], FP)      # per-chunk G-masked weights, rotating
        o1_ps = pacc.tile([65, T], FP, tag="acc", name="o1_ps")


        for g in range(NC // 8):
            # interp chunks back to natural: 8x [64,128]->[128,64]
            tpi = ptr.tile([128, 512], FP, tag="tp")
            for jc in range(8):
                c = 8 * g + jc
                nc.tensor.transpose(tpi[:, jc * 64:(jc + 1) * 64],
                                    tile12i[:, c * 128:(c + 1) * 128],
                                    ident[0:64, 0:64])
            icomb = work.tile([128, 8, 64], FP, tag="icomb")
            nc.scalar.activation(out=R(icomb[:].rearrange("p c f -> p (c f)")), in_=tpi[:], func=AF.Copy)
            sfx_ps = pmm.tile([128, 8, 64], FP, tag="mm")
            nc.tensor.matmul(sfx_ps[:], R(tri128[:]), R(icomb[:]), start=True, stop=True,
                             skip_group_check=True)
            upw4 = work.tile([128, 8, 64], FP, tag="upw")
            nc.vector.tensor_tensor(out=R(upw4[:]), in0=sfx_ps[:],
                                    in1=_rep_ap(inv3[:, 8 * g:8 * g + 8], 64),
                                    op=OP.mult)
            # nh^T[f, 16-block c] = upw_c^T @ (G * w_all[:,c]) - disjoint out slices
            for jc in range(8):
                c = 8 * g + jc
                nc.gpsimd.tensor_scalar(out=R(wblk[:, jc, :]), in0=G[:],
                                         scalar1=w_all[:, c:c + 1],
                                         scalar2=None, op0=OP.mult)
            nhT_ps = pmm.tile([64, 128], FP, tag="mm")
            for jc in range(8):
                nc.tensor.matmul(nhT_ps[0:64, 16 * jc:16 * jc + 16], R(upw4[:, jc, :]),
                                 R(wblk[:, jc, :]), start=True, stop=True,
                                 skip_group_check=True)
            nhT_sb = work.tile([64, 128], FP, tag="nhT")
            nc.scalar.activation(out=R(nhT_sb[:]), in_=nhT_ps[:], func=AF.Copy)
            nhn_ps = pmm.tile([128, 64], FP, tag="mm")
            nc.tensor.transpose(R(nhn_ps[:]), R(nhT_sb[:]), R(identR[0:64, 0:64]))
            nc.vector.tensor_copy(nh_nat[:, g, 0:64], nhn_ps[:])
            emit_node_acc(g)
            if g + 1 < NC // 8:
                emit_node_score(g + 1)
        # ------- combine + final softmax over F, pipelined in T/2 halves -------
        nc.vector.reciprocal(R(fsr[0:1, :]), o1_ps[64:65, :])
        b1 = pmm.tile([64, T], FP, tag="mm", name="b1")
        for h in range(2):
            hs = slice(h * 512, (h + 1) * 512)
            nc.tensor.matmul(b1[:, hs], R(onesP[0:1, 0:64]), R(fsr[0:1, hs]),
                             start=True, stop=True, skip_group_check=True)
        o1_sb = big.tile([64, T], FP)
        x1 = fin.tile([64, T], FP, tag="x1")
        s12 = fin.tile([64, T], FP, tag="s12")
        e3 = fin.tile([64, T], FP, tag="e3")
        onat_raw = big.tile([128, T // 128, F], FP)
        onat = big.tile([128, T // 128, F], FP)
        zq = fin.tile([128, T // 128], FP, tag="zq")
        rz = fin.tile([128, T // 128], FP, tag="rz")
        for hq in range(4):
            q = slice(hq * TQ, (hq + 1) * TQ)
            nc.scalar.activation(out=o1_sb[:, q], in_=o1_ps[0:64, q], func=AF.Copy)
            nc.vector.tensor_tensor(out=x1[:, q], in0=o1_sb[:, q], in1=b1[:, q],
                                    op=OP.mult)
            nc.vector.tensor_tensor(out=s12[:, q], in0=x1[:, q], in1=x2[:, q],
                                    op=OP.add)
            nc.scalar.activation(out=e3[:, q], in_=s12[:, q], func=AF.Exp,
                                 bias=rootT3[:])
            # unnormalized exp to natural layout; Z is then per-partition
            op_ = ptr.tile([128, 512], FP, tag="tp")
            for k in (2 * hq, 2 * hq + 1):
                nc.tensor.transpose(op_[:, (k % 2) * 64:(k % 2) * 64 + 64],
                                    e3[:, k * 128:(k + 1) * 128],
                                    ident[0:64, 0:64])
            ks = slice(2 * hq, 2 * hq + 2)
            if hq % 2 == 0:
                nc.vector.tensor_copy(onat_raw[:, ks, :].rearrange("p k f -> p (k f)"),
                                      op_[:, 0:128])
            else:
                nc.scalar.activation(out=onat_raw[:, ks, :].rearrange("p k f -> p (k f)"),
                                     in_=op_[:, 0:128], func=AF.Copy)
            nc.vector.tensor_reduce(out=zq[:, ks], in_=onat_raw[:, ks, :],
                                    axis=AX.X, op=OP.add)
            nc.vector.reciprocal(rz[:, ks], zq[:, ks])
            for k in (2 * hq, 2 * hq + 1):
                nc.vector.tensor_scalar(out=onat[:, k, :], in0=onat_raw[:, k, :],
                                        scalar1=rz[:, k:k + 1], scalar2=None,
                                        op0=OP.mult)
            if hq % 2 == 1:
                nc.sync.dma_start(
                    d_out[(hq - 1) * 256:(hq + 1) * 256, :]
                    .rearrange("(k p) f -> p k f", p=128),
                    onat[:, 2 * (hq - 1):2 * (hq + 1), :])


_NC_CACHE = None


def kernel(**inputs):
    global _NC_CACHE
    if _NC_CACHE is None:
        _NC_CACHE = build_nc()
    nc = _NC_CACHE
    shared = {k: np.ascontiguousarray(np.asarray(inputs[k], dtype=np.float32))
              for k in ("Wq", "bq", "Wk", "bk", "Wv", "bv", "Wagg", "bagg")}
    in_maps = []
    for b in range(B):
        m = dict(shared)
        m["root"] = np.ascontiguousarray(np.asarray(inputs["root"][b], dtype=np.float32))
        m["node"] = np.ascontiguousarray(np.asarray(inputs["node"][b], dtype=np.float32))
        m["leaf"] = np.ascontiguousarray(np.asarray(inputs["leaf"][b], dtype=np.float32))
        m["target"] = np.ascontiguousarray(np.asarray(inputs["target"][b], dtype=np.float32))
        in_maps.append(m)
    res = run_bass_kernel_spmd(nc, in_maps, core_ids=list(range(B)))
    return np.stack([r["out"] for r in res.results], axis=0)


# revision 57
# speedup vs baseline: 2.3573x; 1.0076x over previous
"""Trainium2 Bass kernel for nn_DecoderAttention (dual-key tree decoder attention).

Sharding: data-parallel over batch B=8, one batch element per NeuronCore.

Per-core computation (B-slice), fp32 data with fp32r (split-bf16) matmuls:
  q = target @ Wq + bq                     [T,F]   (kept transposed, duplicated on 128 partitions)
  k/v (node, leaf) = x @ {Wk,Wv} + b       (kept transposed [F, *] via PE-transposed inputs)
  logits = leaf @ Wagg + bagg              [L,1]   (PE: leafT-chunk stationary x Wagg column)
  Aqn/Aql softmaxes are computed unnormalized (exp, no max-subtraction: |scores/8| <~ 1.2)
  out_pre = (En^T @ [nh|1])/Z1 + (El^T @ [v|1])/Z2 + root/3
  out = softmax_F(out_pre)                 [T,F]
The tree interpolation's root term commutes through the suffix-mean and the
attention average (softmax weights sum to 1), so root/3 is added once at the end.
Suffix cumsum over L: per-128-chunk triangular matmuls (batched 4 chunks / matmul);
the cross-chunk carries are folded into the LAST ROW of each interp chunk before
the in-chunk suffix (row 127 participates in every suffix sum of its chunk).

Schedule: the leaf-attention score/exp/accumulate stream (the dominant
Activation-engine load) is software-pipelined into the leaf projection loop —
each 512-leaf chunk queues the 4 (score, exp, accumulate) units it unlocks and
drains the queue one chunk behind, so the Activation engine runs continuously
while PE fills its gaps with the next chunk's transposes/projections. Only the
node path (suffix mean -> node_hat -> node attention) waits for the full leaf
pass (it needs the global suffix carries); node attention is interleaved into
the suffix loop, and the final combine/softmax is pipelined in T/4 slices.
PSUM budget (8 banks): pmm 2x[128,1024] + ptr 2x[128,512] + pacc 1x[65,1024],
with the pacc buffer sequenced o2 -> logits -> o1 -> z3.
"""

import os
import sys
from collections import deque

import numpy as np

for _p in ("/opt/trn_rl_repo", "/root/.axon_site/_ro/trn_rl_repo"):
    if os.path.isdir(_p) and _p not in sys.path:
        sys.path.insert(0, _p)

import concourse.bass as bass
import concourse.tile as tile
from concourse import bacc
from concourse import mybir
from concourse.bass_utils import run_bass_kernel_spmd
from concourse.masks import make_identity, make_lower_triangular

FP = mybir.dt.float32
FR = mybir.dt.float32r
BF = mybir.dt.bfloat16
AF = mybir.ActivationFunctionType
OP = mybir.AluOpType
AX = mybir.AxisListType

B, T, N, L, D, F = 8, 1024, 512, 4096, 512, 64
BR = L // N          # 8 leaves per node
NC = L // 128        # 32 leaf chunks of 128
ND = D // 128        # 4 contraction chunks
TQ = T // 4          # final-stage pipeline slice
SCALE = 1.0 / float(np.sqrt(F))


def R(ap):
    """View an fp32 AP as float32r (full-rate PE matmuls, split-bf16 numerics)."""
    return ap.bitcast(FR)


def _rep_ap(ap, rep):
    """Append a step-0 innermost free dim (read each element `rep` times)."""
    return bass.AP(tensor=ap.tensor, offset=ap.offset, ap=list(ap.ap) + [[0, rep]])


def build_nc():
    nc = bacc.Bacc("TRN2", target_bir_lowering=False, debug=False)

    d_root = nc.dram_tensor("root", [1, F], FP, kind="ExternalInput")
    d_node = nc.dram_tensor("node", [N, D], FP, kind="ExternalInput")
    d_leaf = nc.dram_tensor("leaf", [L, D], FP, kind="ExternalInput")
    d_target = nc.dram_tensor("target", [T, D], FP, kind="ExternalInput")
    d_wq = nc.dram_tensor("Wq", [D, F], FP, kind="ExternalInput")
    d_bq = nc.dram_tensor("bq", [F], FP, kind="ExternalInput")
    d_wk = nc.dram_tensor("Wk", [D, F], FP, kind="ExternalInput")
    d_bk = nc.dram_tensor("bk", [F], FP, kind="ExternalInput")
    d_wv = nc.dram_tensor("Wv", [D, F], FP, kind="ExternalInput")
    d_bv = nc.dram_tensor("bv", [F], FP, kind="ExternalInput")
    d_wagg = nc.dram_tensor("Wagg", [D, 1], FP, kind="ExternalInput")
    d_bagg = nc.dram_tensor("bagg", [1], FP, kind="ExternalInput")
    d_out = nc.dram_tensor("out", [T, F], FP, kind="ExternalOutput")

    with tile.TileContext(nc) as tc:
        _emit(nc, tc, d_root, d_node, d_leaf, d_target, d_wq, d_bq, d_wk, d_bk,
              d_wv, d_bv, d_wagg, d_bagg, d_out)
    nc.compile()
    return nc


def _emit(nc, tc, d_root, d_node, d_leaf, d_target, d_wq, d_bq, d_wk, d_bk,
          d_wv, d_bv, d_wagg, d_bagg, d_out):
    from contextlib import ExitStack

    with ExitStack() as ctx:
        ctx.enter_context(nc.allow_low_precision(
            reason="fp32r rounding on matmul operands is intentional"))
        consts = ctx.enter_context(tc.tile_pool(name="consts", bufs=1))
        big = ctx.enter_context(tc.tile_pool(name="big", bufs=1))
        lnat = ctx.enter_context(tc.tile_pool(name="lnat", bufs=3))
        ltp = ctx.enter_context(tc.tile_pool(name="ltp", bufs=2))
        work = ctx.enter_context(tc.tile_pool(name="work", bufs=2))
        fin = ctx.enter_context(tc.tile_pool(name="fin", bufs=1))
        epool = ctx.enter_context(tc.tile_pool(name="epool", bufs=3))
        ptr = ctx.enter_context(tc.tile_pool(name="ptr", bufs=2, space="PSUM"))
        pmm = ctx.enter_context(tc.tile_pool(name="pmm", bufs=2, space="PSUM"))
        pacc = ctx.enter_context(tc.tile_pool(name="pacc", bufs=1, space="PSUM"))

        # ---- identity only (gates all transposes); other consts deferred ----
        ident = consts.tile([128, 128], FP)
        make_identity(nc, ident[:])
        identR = consts.tile([128, 128], FP)   # fp32r-rounded copy for R-transposes
        nc.vector.tensor_copy(R(identR[:]), ident[:])

        # ---------------- stage A: target -> qdual [128, 1024] ----------------
        tns = []
        tn = lnat.tile([128, 4, D], FP, tag="xnat")
        nc.sync.dma_start(R(tn[:]), R(d_target[0:512, :]
                          .rearrange("(j p) d -> p j d", p=128)))
        tns.append(tn)

        # weights / biases; w_qq queued early (gates qdual), w_kv after leaf0
        w_kv = consts.tile([128, ND, 128], FP)     # cols 0:64 Wk, 64:128 Wv per d-chunk
        w_qq = consts.tile([128, ND, 128], FP)     # Wq duplicated
        wagg_raw = consts.tile([128, ND], FP)
        bias_q = consts.tile([128, 1], FP)
        bias_k = consts.tile([128, 1], FP)
        bias_v = consts.tile([64, 1], FP)
        bq2 = d_bq[:].rearrange("(f o) -> f o", o=1)
        bk2 = d_bk[:].rearrange("(f o) -> f o", o=1)
        nc.sync.dma_start(bias_q[0:F, :], bq2)
        nc.sync.dma_start(bias_q[F:128, :], bq2)
        nc.sync.dma_start(bias_k[0:F, :], bk2)
        nc.sync.dma_start(bias_k[F:128, :], bk2)
        nc.sync.dma_start(bias_v[:], d_bv[:].rearrange("(f o) -> f o", o=1))
        bagg_b = consts.tile([128, 1], FP)
        _ba = d_bagg[:]
        nc.gpsimd.dma_start(bagg_b[:], bass.AP(tensor=_ba.tensor, offset=_ba.offset,
                                               ap=[[0, 128], [1, 1]]))
        root_row = consts.tile([1, F], FP)
        nc.sync.dma_start(root_row[:], d_root[:])

        ln_pre = deque()

        def prefetch_leaf(i):
            ln = lnat.tile([128, 4, D], FP, tag="xnat")
            nc.sync.dma_start(R(ln[:]), R(d_leaf[i * 512:(i + 1) * 512, :]
                              .rearrange("(j p) d -> p j d", p=128)))
            ln_pre.append(ln)

        nc.sync.dma_start(R(w_qq[:, :, 0:F]), R(d_wq[:].rearrange("(j p) f -> p j f", p=128)))
        nc.sync.dma_start(R(w_qq[:, :, F:128]), R(d_wq[:].rearrange("(j p) f -> p j f", p=128)))
        tn = lnat.tile([128, 4, D], FP, tag="xnat")
        nc.sync.dma_start(R(tn[:]), R(d_target[512:1024, :]
                          .rearrange("(j p) d -> p j d", p=128)))
        tns.append(tn)
        prefetch_leaf(0)
        nc.sync.dma_start(R(w_kv[:, :, 0:F]), R(d_wk[:].rearrange("(j p) f -> p j f", p=128)))
        nc.sync.dma_start(R(w_kv[:, :, F:128]), R(d_wv[:].rearrange("(j p) f -> p j f", p=128)))
        nc.sync.dma_start(wagg_raw[:], d_wagg[:].rearrange("(j p) o -> p (j o)", p=128))
        q_ps = pmm.tile([128, T], FP, tag="mm", name="q_ps")
        for ib in range(T // 512):
            targT = ltp.tile([128, ND, 512], FP, tag="lt")
            for j in range(4):
                tp = ptr.tile([128, 512], FP, tag="tp")
                for dc in range(ND):
                    nc.tensor.transpose(R(tp[:, dc * 128:(dc + 1) * 128]),
                                        R(tns[ib][:, j, dc * 128:(dc + 1) * 128]),
                                        R(identR[:]))
                if j % 2 == 0:
                    nc.vector.tensor_copy(R(targT[:, 0:ND, j * 128:(j + 1) * 128]),
                                          R(tp[:].rearrange("p (dc b) -> p dc b", b=128)))
                else:
                    nc.scalar.activation(
                        out=R(targT[:, 0:ND, j * 128:(j + 1) * 128]),
                        in_=R(tp[:].rearrange("p (dc b) -> p dc b", b=128)),
                        func=AF.Copy)
            for dc in range(ND):
                nc.tensor.matmul(q_ps[:, ib * 512:(ib + 1) * 512], R(w_qq[:, dc, :]),
                                 R(targT[:, dc, :]), start=(dc == 0), stop=(dc == ND - 1),
                                 skip_group_check=True)
        qdual = big.tile([128, T], FP)
        nc.scalar.activation(out=R(qdual[:]), in_=q_ps[:], func=AF.Identity, bias=bias_q[:])

        # ---------------- stage B: node -> kTn_dual, node_vT ----------------
        prefetch_leaf(1)
        nn = lnat.tile([128, 4, D], FP, tag="xnat")
        nc.sync.dma_start(R(nn[:]), R(d_node[:].rearrange("(j p) d -> p j d", p=128)))
        nodeT = ltp.tile([128, ND, 512], FP, tag="lt")
        for i in range(N // 128):
            tp = ptr.tile([128, 512], FP, tag="tp")
            for dc in range(ND):
                nc.tensor.transpose(R(tp[:, dc * 128:(dc + 1) * 128]),
                                    R(nn[:, i, dc * 128:(dc + 1) * 128]), R(identR[:]))
            if i % 2 == 0:
                nc.vector.tensor_copy(R(nodeT[:, 0:ND, i * 128:(i + 1) * 128]),
                                      R(tp[:].rearrange("p (dc b) -> p dc b", b=128)))
            else:
                nc.scalar.activation(
                    out=R(nodeT[:, 0:ND, i * 128:(i + 1) * 128]),
                    in_=R(tp[:].rearrange("p (dc b) -> p dc b", b=128)),
                    func=AF.Copy)
        kTn_dual = big.tile([128, 256], FP)
        node_vT = big.tile([64, N], FP)            # node_v^T + bias_v (bias pre-folded)
        kvn_ps = pmm.tile([128, 512], FP, tag="mm", name="kvn_ps")
        for dc in range(ND):
            nc.tensor.matmul(kvn_ps[:], R(w_kv[:, dc, :]), R(nodeT[:, dc, :]),
                             start=(dc == 0), stop=(dc == ND - 1))
        for b in range(4):
            ro, co = (b % 2) * 64, (b // 2) * 128
            nc.scalar.activation(out=R(kTn_dual[ro:ro + 64, co:co + 128]),
                                 in_=kvn_ps[0:64, b * 128:(b + 1) * 128],
                                 func=AF.Identity, bias=bias_k[ro:ro + 64, :])
        nc.vector.tensor_scalar(out=node_vT[:], in0=kvn_ps[64:128, :],
                                scalar1=bias_v[:], scalar2=None, op0=OP.add)

        # remaining constants (needed only after the leaf pass); pool/DVE slot
        # these behind stage B's work
        tri_raw = work.tile([128, 128], FP, tag="traw")
        make_lower_triangular(nc, tri_raw[:], val=1.0, diag=True)
        tri128 = consts.tile([128, 128], FP)      # [m,l]=1 iff l<=m  (suffix lhsT)
        nc.vector.tensor_copy(R(tri128[:]), tri_raw[:])
        tri32s = consts.tile([32, 32], FP)        # [k,c]=1 iff k>c   (carry)
        make_lower_triangular(nc, tri32s[:], val=1.0, diag=False)
        G = consts.tile([128, 16], FP)            # G[m,j] = 1 iff m//8 == j
        nc.gpsimd.memset(G[:], 1.0)
        nc.gpsimd.affine_select(out=G[:], in_=G[:], compare_op=OP.is_ge, fill=0.0,
                                base=0, pattern=[[-BR, 16]], channel_multiplier=1)
        nc.gpsimd.affine_select(out=G[:], in_=G[:], compare_op=OP.is_ge, fill=0.0,
                                base=BR - 1, pattern=[[BR, 16]], channel_multiplier=-1)
        GT = consts.tile([16, 128], FP)
        nc.gpsimd.memset(GT[:], 1.0)
        nc.gpsimd.affine_select(out=GT[:], in_=GT[:], compare_op=OP.is_ge, fill=0.0,
                                base=0, pattern=[[1, 128]], channel_multiplier=-BR)
        nc.gpsimd.affine_select(out=GT[:], in_=GT[:], compare_op=OP.is_ge, fill=0.0,
                                base=BR - 1, pattern=[[-1, 128]], channel_multiplier=BR)
        ones_raw = consts.tile([128, 1], FP)
        nc.gpsimd.memset(ones_raw[:], 1.0)
        onesP = consts.tile([128, 64], FP)
        nc.vector.tensor_copy(R(onesP[:]), bass.AP(tensor=ones_raw[:].tensor,
                                                   offset=ones_raw[:].offset,
                                                   ap=[[1, 128], [0, 64]]))
        cnt3 = consts.tile([128, NC], FP)         # 3 * (L - l), l = 128*c + p
        nc.gpsimd.iota(cnt3[:], pattern=[[-3 * 128, NC]], base=3 * L,
                       channel_multiplier=-3, allow_small_or_imprecise_dtypes=True)
        inv3 = consts.tile([128, NC], FP)
        nc.vector.reciprocal(inv3[:], cnt3[:])

        # ------- stage C + F: leaf pass with pipelined leaf attention -------
        # kTdual: 512-chunk i -> rows (i%2)*64, cols (i//2)*512
        kTdual = big.tile([128, L // 2], FP)
        leafT = big.tile([128, ND, L], FP)     # persistent
        lgn = big.tile([128, NC], FP)          # leaf logits, natural layout
        tile12i = big.tile([64, L], FP)        # interp' = leaf_v + node_rep
        vcomb = big.tile([128, NC, 65], BF)    # [v(64) | ones] per 128-leaf chunk
        nc.vector.tensor_copy(vcomb[:, :, 64:65],
                              bass.AP(tensor=ones_raw[:].tensor,
                                      offset=ones_raw[:].offset,
                                      ap=[[1, 128], [0, NC], [1, 1]]))
        o2_ps = pacc.tile([65, T], FP, tag="acc", name="o2_ps")
        totT = big.tile([64, NC], FP)          # per-chunk interp totals (for carries)
        att_q = deque()          # (ct, half) score work not yet emitted
        acc_q = deque()          # (el, b2) exp'd scores awaiting accumulate
        el_state = {"done": 0}

        def emit_score(ct, half):
            ro2 = half * 64
            b2 = 8 * (ct // 4) + ct % 4 + 4 * half
            st = pmm.tile([128, T], FP, tag="mm")
            for h in range(2):
                hs = slice(h * 512, (h + 1) * 512)
                nc.tensor.matmul(st[:, hs],
                                 R(kTdual[ro2:ro2 + 64, ct * 128:(ct + 1) * 128]),
                                 R(qdual[ro2:ro2 + 64, hs]), start=True, stop=True,
                                 skip_group_check=True)
            el = epool.tile([128, T], BF, tag="el")
            nc.scalar.activation(out=el[:], in_=st[:], func=AF.Exp, scale=SCALE)
            acc_q.append((el, b2))

        def emit_acc():
            el, b2 = acc_q.popleft()
            for h in range(2):
                hs = slice(h * 512, (h + 1) * 512)
                nc.tensor.matmul(o2_ps[:, hs], vcomb[:, b2, 0:65], el[:, hs],
                                 start=(el_state["done"] == 0),
                                 stop=(el_state["done"] == 31),
                                 skip_group_check=True)
            el_state["done"] += 1

        for i in range(L // 512):
            ln = ln_pre.popleft() if ln_pre else None
            if ln is None:
                ln = lnat.tile([128, 4, D], FP, tag="xnat")
                nc.sync.dma_start(R(ln[:]), R(d_leaf[i * 512:(i + 1) * 512, :]
                                  .rearrange("(j p) d -> p j d", p=128)))
            if i + 2 < L // 512:
                prefetch_leaf(i + 2)
            for j in range(4):
                tp = ptr.tile([128, 512], FP, tag="tp")
                for dc in range(ND):
                    nc.tensor.transpose(R(tp[:, dc * 128:(dc + 1) * 128]),
                                        R(ln[:, j, dc * 128:(dc + 1) * 128]), R(identR[:]))
                if j % 2 == 0:
                    nc.vector.tensor_copy(
                        R(leafT[:, 0:ND, (4 * i + j) * 128:(4 * i + j + 1) * 128]),
                        R(tp[:].rearrange("p (dc b) -> p dc b", b=128)))
                else:
                    nc.scalar.activation(
                        out=R(leafT[:, 0:ND, (4 * i + j) * 128:(4 * i + j + 1) * 128]),
                        in_=R(tp[:].rearrange("p (dc b) -> p dc b", b=128)),
                        func=AF.Copy)
                if att_q:
                    emit_score(*att_q.popleft())
                if j >= 2 and len(acc_q) >= 3:
                    emit_acc()
            kv_ps = pmm.tile([128, 528], FP, tag="mm")
            for dc in range(ND):
                nc.tensor.matmul(kv_ps[:, 0:512], R(w_kv[:, dc, :]),
                                 R(leafT[:, dc, i * 512:(i + 1) * 512]),
                                 start=(dc == 0), stop=(dc == ND - 1),
                                 skip_group_check=True)
            # logits: one single (non-accumulating) matmul per (column, d-chunk);
            # nested accumulation groups wedge the device, so the 4 d-chunk
            # partials land in separate columns and are summed on DVE below
            for cj in range(4):
                c = 4 * i + cj
                for dc in range(ND):
                    nc.tensor.matmul(kv_ps[:, 512 + 4 * cj + dc:513 + 4 * cj + dc],
                                     leafT[:, dc, c * 128:(c + 1) * 128],
                                     wagg_raw[:, dc:dc + 1],
                                     start=True, stop=True,
                                     skip_group_check=True)
            # drain kv_ps promptly: it shares the PSUM rotation with the scores
            ro, co = (i % 2) * 64, (i // 2) * 512
            sl = slice(i * 512, (i + 1) * 512)
            nc.vector.tensor_scalar(out=R(kTdual[ro:ro + 64, co:co + 512]),
                                    in0=kv_ps[0:64, 0:512], scalar1=bias_k[ro:ro + 64, :],
                                    scalar2=None, op0=OP.add)
            t12v = work.tile([64, 512], FP, tag="t12v")
            nc.vector.tensor_scalar(out=t12v[:], in0=kv_ps[64:128, 0:512],
                                    scalar1=bias_v[:], scalar2=None, op0=OP.add)
            nc.vector.tensor_reduce(
                out=lgn[:, 4 * i:4 * i + 4],
                in_=kv_ps[:, 512:528].rearrange("p (c d) -> p c d", d=4),
                axis=AX.X, op=OP.add)
            while acc_q:
                emit_acc()
            # interp' = leaf_v + node_vT' replicated 8x along l
            nc.gpsimd.tensor_tensor(
                out=tile12i[:, sl].rearrange("f (n c) -> f n c", c=BR),
                in0=t12v[:].rearrange("f (n c) -> f n c", c=BR),
                in1=_rep_ap(node_vT[0:64, 64 * i:64 * (i + 1)], BR), op=OP.add)
            # v back to natural for the attention lhsT: 4x [64,128]->[128,64]
            tpv = ptr.tile([128, 512], FP, tag="tp")
            for c4 in range(4):
                nc.tensor.transpose(tpv[:, c4 * 64:(c4 + 1) * 64],
                                    t12v[:, c4 * 128:(c4 + 1) * 128],
                                    ident[0:64, 0:64])
            nc.vector.tensor_copy(
                vcomb[:, 4 * i:4 * i + 4, 0:64],
                tpv[:, 0:256].rearrange("p (c f) -> p c f", f=64))
            nc.vector.tensor_reduce(out=totT[:, 4 * i:4 * i + 4],
                                    in_=tile12i[:, sl].rearrange("f (c m) -> f c m", m=128),
                                    axis=AX.X, op=OP.add)
            # queue the 4 leaf-attention units this chunk unlocks
            g, half = i // 2, i % 2
            for ct in range(4 * g, 4 * g + 4):
                att_q.append((ct, half))

        node_en = {}

        def emit_node_score(b):
            ro, co = (b % 2) * 64, (b // 2) * 128
            st = pmm.tile([128, T], FP, tag="mm")
            for h in range(2):
                hs = slice(h * 512, (h + 1) * 512)
                nc.tensor.matmul(st[:, hs], R(kTn_dual[ro:ro + 64, co:co + 128]),
                                 R(qdual[ro:ro + 64, hs]), start=True, stop=True,
                                 skip_group_check=True)
            en = epool.tile([128, T], BF, tag="el")
            nc.scalar.activation(out=en[:], in_=st[:], func=AF.Exp, scale=SCALE)
            node_en[b] = en

        def emit_node_acc(b):
            en = node_en.pop(b)
            for h in range(2):
                hs = slice(h * 512, (h + 1) * 512)
                nc.tensor.matmul(o1_ps[:, hs], nh_nat[:, b, 0:65], en[:, hs],
                                 start=(b == 0), stop=(b == 3), skip_group_check=True)

        # ---- flush remaining attention; carries run concurrently on DVE/PE ----
        while att_q or acc_q:
            if att_q:
                emit_score(*att_q.popleft())
            if acc_q:
                emit_acc()
        emit_node_score(0)
        emit_node_score(1)
        tot_ps = ptr.tile([NC, 64], FP, tag="tp")
        nc.tensor.transpose(tot_ps[:], totT[:], ident[0:64, 0:64])
        totals = work.tile([NC, 64], FP, tag="tot")
        nc.scalar.activation(out=totals[:], in_=tot_ps[:], func=AF.Copy)
        carrT_ps = ptr.tile([64, NC], FP, tag="tp")
        nc.tensor.matmul(carrT_ps[:], totals[:], tri32s[:], start=True, stop=True)
        # interpT[f, 128c+127] += carryT[f, c]  (row 127 is in every suffix sum)
        last_rows = tile12i[:, 127::128]
        nc.vector.tensor_tensor(out=last_rows, in0=last_rows, in1=carrT_ps[:], op=OP.add)
        rt_ps = ptr.tile([F, 1], FP, tag="tp")
        nc.tensor.transpose(rt_ps[:], root_row[:], ident[0:1, 0:1])
        rootT3 = consts.tile([F, 1], FP)
        nc.scalar.activation(out=rootT3[:], in_=rt_ps[:], func=AF.Copy, scale=1.0 / 3.0)
        # ---------------- logits -> group-softmax weights ----------------
        e_all = work.tile([128, NC], FP, tag="e_all")
        nc.scalar.activation(out=e_all[:], in_=lgn[:], func=AF.Exp, bias=bagg_b[:])
        s_ps = pmm.tile([16, NC], FP, tag="mm", name="s_ps")
        nc.tensor.matmul(s_ps[:], G[:], e_all[:], start=True, stop=True)
        sinv = work.tile([16, NC], FP, tag="sinv")
        nc.vector.reciprocal(sinv[:], s_ps[:])
        r_ps = pmm.tile([128, NC], FP, tag="mm", name="r_ps")
        nc.tensor.matmul(r_ps[:], GT[:], sinv[:], start=True, stop=True)
        w_all = work.tile([128, NC], FP, tag="w_all")
        nc.vector.tensor_tensor(out=w_all[:], in0=e_all[:], in1=r_ps[:], op=OP.mult)

        o2_sb = big.tile([65, T], FP)
        nc.vector.tensor_copy(o2_sb[:], o2_ps[:])
        fsr = fin.tile([65, T], FP, tag="fsr")  # rows 0/32/64: 1/Z1, 1/Z2, 1/Z3
        nc.vector.reciprocal(R(fsr[32:33, :]), o2_sb[64:65, :])


        # o2-dependent half of the final combine runs during the node phase
        b2 = pmm.tile([64, T], FP, tag="mm", name="b2")
        for h in range(2):
            hs = slice(h * 512, (h + 1) * 512)
            nc.tensor.matmul(b2[:, hs], R(onesP[32:33, 0:64]), R(fsr[32:33, hs]),
                             start=True, stop=True, skip_group_check=True)
        x2 = fin.tile([64, T], FP, tag="x2")
        for h in range(2):
            hs = slice(h * 512, (h + 1) * 512)
            nc.vector.tensor_tensor(out=x2[:, hs], in0=o2_sb[0:64, hs],
                                    in1=b2[:, hs], op=OP.mult)

        # ------- suffix-mean + node_hat, with node attention interleaved -------
        nh_nat = big.tile([128, 4, 65], BF)
        nc.vector.tensor_copy(nh_nat[:, :, 64:65],
                              bass.AP(tensor=ones_raw[:].tensor,
                                      offset=ones_raw[:].offset,
                                      ap=[[1, 128], [0, 4], [1, 1]]))
        wblk = big.tile([128, 8, 16], FP)      # per-chunk G-masked weights, rotating
        o1_ps = pacc.tile([65, T], FP, tag="acc", name="o1_ps")


        for g in range(NC // 8):
            # interp chunks back to natural: 8x [64,128]->[128,64]
            tpi = ptr.tile([128, 512], FP, tag="tp")
            for jc in range(8):
                c = 8 * g + jc
                nc.tensor.transpose(tpi[:, jc * 64:(jc + 1) * 64],
                                    tile12i[:, c * 128:(c + 1) * 128],
                                    ident[0:64, 0:64])
            icomb = work.tile([128, 8, 64], FP, tag="icomb")
            nc.scalar.activation(out=R(icomb[:].rearrange("p c f -> p (c f)")), in_=tpi[:], func=AF.Copy)
            sfx_ps = pmm.tile([128, 8, 64], FP, tag="mm")
            nc.tensor.matmul(sfx_ps[:], R(tri128[:]), R(icomb[:]), start=True, stop=True,
                             skip_group_check=True)
            upw4 = work.tile([128, 8, 64], FP, tag="upw")
            nc.vector.tensor_tensor(out=R(upw4[:]), in0=sfx_ps[:],
                                    in1=_rep_ap(inv3[:, 8 * g:8 * g + 8], 64),
                                    op=OP.mult)
            # nh^T[f, 16-block c] = upw_c^T @ (G * w_all[:,c]) - disjoint out slices
            for jc in range(8):
                c = 8 * g + jc
                nc.gpsimd.tensor_scalar(out=R(wblk[:, jc, :]), in0=G[:],
                                         scalar1=w_all[:, c:c + 1],
                                         scalar2=None, op0=OP.mult)
            nhT_ps = pmm.tile([64, 128], FP, tag="mm")
            for jc in range(8):
                nc.tensor.matmul(nhT_ps[0:64, 16 * jc:16 * jc + 16], R(upw4[:, jc, :]),
                                 R(wblk[:, jc, :]), start=True, stop=True,
                                 skip_group_check=True)
            nhT_sb = work.tile([64, 128], FP, tag="nhT")
            nc.scalar.activation(out=R(nhT_sb[:]), in_=nhT_ps[:], func=AF.Copy)
            nhn_ps = pmm.tile([128, 64], FP, tag="mm")
            nc.tensor.transpose(R(nhn_ps[:]), R(nhT_sb[:]), R(identR[0:64, 0:64]))
            nc.vector.tensor_copy(nh_nat[:, g, 0:64], nhn_ps[:])
            emit_node_acc(g)
            if g + 2 < NC // 8:
                emit_node_score(g + 2)
        # ------- combine + final softmax over F, pipelined in T/2 halves -------
        nc.vector.reciprocal(R(fsr[0:1, :]), o1_ps[64:65, :])
        b1 = pmm.tile([64, T], FP, tag="mm", name="b1")
        for h in range(2):
            hs = slice(h * 512, (h + 1) * 512)
            nc.tensor.matmul(b1[:, hs], R(onesP[0:1, 0:64]), R(fsr[0:1, hs]),
                             start=True, stop=True, skip_group_check=True)
        o1_sb = big.tile([64, T], FP)
        x1 = fin.tile([64, T], FP, tag="x1")
        s12 = fin.tile([64, T], FP, tag="s12")
        e3 = fin.tile([64, T], FP, tag="e3")
        onat_raw = big.tile([128, T // 128, F], FP)
        onat = big.tile([128, T // 128, F], FP)
        zq = fin.tile([128, T // 128], FP, tag="zq")
        rz = fin.tile([128, T // 128], FP, tag="rz")
        for hq in range(4):
            q = slice(hq * TQ, (hq + 1) * TQ)
            nc.scalar.activation(out=o1_sb[:, q], in_=o1_ps[0:64, q], func=AF.Copy)
            nc.vector.tensor_tensor(out=x1[:, q], in0=o1_sb[:, q], in1=b1[:, q],
                                    op=OP.mult)
            nc.vector.tensor_tensor(out=s12[:, q], in0=x1[:, q], in1=x2[:, q],
                                    op=OP.add)
            nc.scalar.activation(out=e3[:, q], in_=s12[:, q], func=AF.Exp,
                                 bias=rootT3[:])
            # unnormalized exp to natural layout; Z is then per-partition
            op_ = ptr.tile([128, 512], FP, tag="tp")
            for k in (2 * hq, 2 * hq + 1):
                nc.tensor.transpose(op_[:, (k % 2) * 64:(k % 2) * 64 + 64],
                                    e3[:, k * 128:(k + 1) * 128],
                                    ident[0:64, 0:64])
            ks = slice(2 * hq, 2 * hq + 2)
            if hq % 2 == 0:
                nc.vector.tensor_copy(onat_raw[:, ks, :].rearrange("p k f -> p (k f)"),
                                      op_[:, 0:128])
            else:
                nc.scalar.activation(out=onat_raw[:, ks, :].rearrange("p k f -> p (k f)"),
                                     in_=op_[:, 0:128], func=AF.Copy)
            nc.vector.tensor_reduce(out=zq[:, ks], in_=onat_raw[:, ks, :],
                                    axis=AX.X, op=OP.add)
            nc.vector.reciprocal(rz[:, ks], zq[:, ks])
            for k in (2 * hq, 2 * hq + 1):
                nc.vector.tensor_scalar(out=onat[:, k, :], in0=onat_raw[:, k, :],
                                        scalar1=rz[:, k:k + 1], scalar2=None,
                                        op0=OP.mult)
            if hq % 2 == 1:
                nc.sync.dma_start(
                    d_out[(hq - 1) * 256:(hq + 1) * 256, :]
                    .rearrange("(k p) f -> p k f", p=128),
                    onat[:, 2 * (hq - 1):2 * (hq + 1), :])


_NC_CACHE = None


def kernel(**inputs):
    global _NC_CACHE
    if _NC_CACHE is None:
        _NC_CACHE = build_nc()
    nc = _NC_CACHE
    shared = {k: np.ascontiguousarray(np.asarray(inputs[k], dtype=np.float32))
              for k in ("Wq", "bq", "Wk", "bk", "Wv", "bv", "Wagg", "bagg")}
    in_maps = []
    for b in range(B):
        m = dict(shared)
        m["root"] = np.ascontiguousarray(np.asarray(inputs["root"][b], dtype=np.float32))
        m["node"] = np.ascontiguousarray(np.asarray(inputs["node"][b], dtype=np.float32))
        m["leaf"] = np.ascontiguousarray(np.asarray(inputs["leaf"][b], dtype=np.float32))
        m["target"] = np.ascontiguousarray(np.asarray(inputs["target"][b], dtype=np.float32))
        in_maps.append(m)
    res = run_bass_kernel_spmd(nc, in_maps, core_ids=list(range(B)))
    return np.stack([r["out"] for r in res.results], axis=0)


# revision 60
# speedup vs baseline: 2.3778x; 1.0087x over previous
"""Trainium2 Bass kernel for nn_DecoderAttention (dual-key tree decoder attention).

Sharding: data-parallel over batch B=8, one batch element per NeuronCore.

Per-core computation (B-slice), fp32 data with fp32r (split-bf16) matmuls:
  q = target @ Wq + bq                     [T,F]   (kept transposed, duplicated on 128 partitions)
  k/v (node, leaf) = x @ {Wk,Wv} + b       (kept transposed [F, *] via PE-transposed inputs)
  logits = leaf @ Wagg + bagg              [L,1]   (PE: leafT-chunk stationary x Wagg column)
  Aqn/Aql softmaxes are computed unnormalized (exp, no max-subtraction: |scores/8| <~ 1.2)
  out_pre = (En^T @ [nh|1])/Z1 + (El^T @ [v|1])/Z2 + root/3
  out = softmax_F(out_pre)                 [T,F]
The tree interpolation's root term commutes through the suffix-mean and the
attention average (softmax weights sum to 1), so root/3 is added once at the end.
Suffix cumsum over L: per-128-chunk triangular matmuls (batched 4 chunks / matmul);
the cross-chunk carries are folded into the LAST ROW of each interp chunk before
the in-chunk suffix (row 127 participates in every suffix sum of its chunk).

Schedule: the leaf-attention score/exp/accumulate stream (the dominant
Activation-engine load) is software-pipelined into the leaf projection loop —
each 512-leaf chunk queues the 4 (score, exp, accumulate) units it unlocks and
drains the queue one chunk behind, so the Activation engine runs continuously
while PE fills its gaps with the next chunk's transposes/projections. Only the
node path (suffix mean -> node_hat -> node attention) waits for the full leaf
pass (it needs the global suffix carries); node attention is interleaved into
the suffix loop, and the final combine/softmax is pipelined in T/4 slices.
PSUM budget (8 banks): pmm 2x[128,1024] + ptr 2x[128,512] + pacc 1x[65,1024],
with the pacc buffer sequenced o2 -> logits -> o1 -> z3.
"""

import os
import sys
from collections import deque

import numpy as np

for _p in ("/opt/trn_rl_repo", "/root/.axon_site/_ro/trn_rl_repo"):
    if os.path.isdir(_p) and _p not in sys.path:
        sys.path.insert(0, _p)

import concourse.bass as bass
import concourse.tile as tile
from concourse import bacc
from concourse import mybir
from concourse.bass_utils import run_bass_kernel_spmd
from concourse.masks import make_identity, make_lower_triangular

FP = mybir.dt.float32
FR = mybir.dt.float32r
BF = mybir.dt.bfloat16
AF = mybir.ActivationFunctionType
OP = mybir.AluOpType
AX = mybir.AxisListType

B, T, N, L, D, F = 8, 1024, 512, 4096, 512, 64
BR = L // N          # 8 leaves per node
NC = L // 128        # 32 leaf chunks of 128
ND = D // 128        # 4 contraction chunks
TQ = T // 4          # final-stage pipeline slice
SCALE = 1.0 / float(np.sqrt(F))


def R(ap):
    """View an fp32 AP as float32r (full-rate PE matmuls, split-bf16 numerics)."""
    return ap.bitcast(FR)


def _rep_ap(ap, rep):
    """Append a step-0 innermost free dim (read each element `rep` times)."""
    return bass.AP(tensor=ap.tensor, offset=ap.offset, ap=list(ap.ap) + [[0, rep]])


def build_nc():
    nc = bacc.Bacc("TRN2", target_bir_lowering=False, debug=False)

    d_root = nc.dram_tensor("root", [1, F], FP, kind="ExternalInput")
    d_node = nc.dram_tensor("node", [N, D], FP, kind="ExternalInput")
    d_leaf = nc.dram_tensor("leaf", [L, D], FP, kind="ExternalInput")
    d_target = nc.dram_tensor("target", [T, D], FP, kind="ExternalInput")
    d_wq = nc.dram_tensor("Wq", [D, F], FP, kind="ExternalInput")
    d_bq = nc.dram_tensor("bq", [F], FP, kind="ExternalInput")
    d_wk = nc.dram_tensor("Wk", [D, F], FP, kind="ExternalInput")
    d_bk = nc.dram_tensor("bk", [F], FP, kind="ExternalInput")
    d_wv = nc.dram_tensor("Wv", [D, F], FP, kind="ExternalInput")
    d_bv = nc.dram_tensor("bv", [F], FP, kind="ExternalInput")
    d_wagg = nc.dram_tensor("Wagg", [D, 1], FP, kind="ExternalInput")
    d_bagg = nc.dram_tensor("bagg", [1], FP, kind="ExternalInput")
    d_out = nc.dram_tensor("out", [T, F], FP, kind="ExternalOutput")

    with tile.TileContext(nc) as tc:
        _emit(nc, tc, d_root, d_node, d_leaf, d_target, d_wq, d_bq, d_wk, d_bk,
              d_wv, d_bv, d_wagg, d_bagg, d_out)
    nc.compile()
    return nc


def _emit(nc, tc, d_root, d_node, d_leaf, d_target, d_wq, d_bq, d_wk, d_bk,
          d_wv, d_bv, d_wagg, d_bagg, d_out):
    from contextlib import ExitStack

    with ExitStack() as ctx:
        ctx.enter_context(nc.allow_low_precision(
            reason="fp32r rounding on matmul operands is intentional"))
        consts = ctx.enter_context(tc.tile_pool(name="consts", bufs=1))
        big = ctx.enter_context(tc.tile_pool(name="big", bufs=1))
        lnat = ctx.enter_context(tc.tile_pool(name="lnat", bufs=3))
        ltp = ctx.enter_context(tc.tile_pool(name="ltp", bufs=2))
        work = ctx.enter_context(tc.tile_pool(name="work", bufs=2))
        fin = ctx.enter_context(tc.tile_pool(name="fin", bufs=1))
        epool = ctx.enter_context(tc.tile_pool(name="epool", bufs=3))
        ptr = ctx.enter_context(tc.tile_pool(name="ptr", bufs=2, space="PSUM"))
        pmm = ctx.enter_context(tc.tile_pool(name="pmm", bufs=2, space="PSUM"))
        pacc = ctx.enter_context(tc.tile_pool(name="pacc", bufs=1, space="PSUM"))

        # ---- identity only (gates all transposes); other consts deferred ----
        ident = consts.tile([128, 128], FP)
        make_identity(nc, ident[:])
        identR = consts.tile([128, 128], FP)   # fp32r-rounded copy for R-transposes
        nc.vector.tensor_copy(R(identR[:]), ident[:])

        # ---------------- stage A: target -> qdual [128, 1024] ----------------
        tns = []
        tn = lnat.tile([128, 4, D], FP, tag="xnat")
        nc.sync.dma_start(R(tn[:]), R(d_target[0:512, :]
                          .rearrange("(j p) d -> p j d", p=128)))
        tns.append(tn)

        # weights / biases; w_qq queued early (gates qdual), w_kv after leaf0
        w_kv = consts.tile([128, ND, 128], FP)     # cols 0:64 Wk, 64:128 Wv per d-chunk
        w_qq = consts.tile([128, ND, 128], FP)     # Wq duplicated
        wagg_raw = consts.tile([128, ND], FP)
        bias_q = consts.tile([128, 1], FP)
        bias_k = consts.tile([128, 1], FP)
        bias_v = consts.tile([64, 1], FP)
        bq2 = d_bq[:].rearrange("(f o) -> f o", o=1)
        bk2 = d_bk[:].rearrange("(f o) -> f o", o=1)
        nc.sync.dma_start(bias_q[0:F, :], bq2)
        nc.sync.dma_start(bias_q[F:128, :], bq2)
        nc.sync.dma_start(bias_k[0:F, :], bk2)
        nc.sync.dma_start(bias_k[F:128, :], bk2)
        nc.sync.dma_start(bias_v[:], d_bv[:].rearrange("(f o) -> f o", o=1))
        bagg_b = consts.tile([128, 1], FP)
        _ba = d_bagg[:]
        nc.gpsimd.dma_start(bagg_b[:], bass.AP(tensor=_ba.tensor, offset=_ba.offset,
                                               ap=[[0, 128], [1, 1]]))
        root_row = consts.tile([1, F], FP)
        nc.sync.dma_start(root_row[:], d_root[:])

        ln_pre = deque()

        def prefetch_leaf(i):
            ln = lnat.tile([128, 4, D], FP, tag="xnat")
            nc.sync.dma_start(R(ln[:]), R(d_leaf[i * 512:(i + 1) * 512, :]
                              .rearrange("(j p) d -> p j d", p=128)))
            ln_pre.append(ln)

        nc.sync.dma_start(R(w_qq[:, :, 0:F]), R(d_wq[:].rearrange("(j p) f -> p j f", p=128)))
        nc.sync.dma_start(R(w_qq[:, :, F:128]), R(d_wq[:].rearrange("(j p) f -> p j f", p=128)))
        tn = lnat.tile([128, 4, D], FP, tag="xnat")
        nc.sync.dma_start(R(tn[:]), R(d_target[512:1024, :]
                          .rearrange("(j p) d -> p j d", p=128)))
        tns.append(tn)
        prefetch_leaf(0)
        nc.sync.dma_start(R(w_kv[:, :, 0:F]), R(d_wk[:].rearrange("(j p) f -> p j f", p=128)))
        nc.sync.dma_start(R(w_kv[:, :, F:128]), R(d_wv[:].rearrange("(j p) f -> p j f", p=128)))
        nc.sync.dma_start(wagg_raw[:], d_wagg[:].rearrange("(j p) o -> p (j o)", p=128))
        q_ps = pmm.tile([128, T], FP, tag="mm", name="q_ps")
        for ib in range(T // 512):
            targT = ltp.tile([128, ND, 512], FP, tag="lt")
            for j in range(4):
                tp = ptr.tile([128, 512], FP, tag="tp")
                for dc in range(ND):
                    nc.tensor.transpose(R(tp[:, dc * 128:(dc + 1) * 128]),
                                        R(tns[ib][:, j, dc * 128:(dc + 1) * 128]),
                                        R(identR[:]))
                if j % 2 == 0:
                    nc.vector.tensor_copy(R(targT[:, 0:ND, j * 128:(j + 1) * 128]),
                                          R(tp[:].rearrange("p (dc b) -> p dc b", b=128)))
                else:
                    nc.scalar.activation(
                        out=R(targT[:, 0:ND, j * 128:(j + 1) * 128]),
                        in_=R(tp[:].rearrange("p (dc b) -> p dc b", b=128)),
                        func=AF.Copy)
            for dc in range(ND):
                nc.tensor.matmul(q_ps[:, ib * 512:(ib + 1) * 512], R(w_qq[:, dc, :]),
                                 R(targT[:, dc, :]), start=(dc == 0), stop=(dc == ND - 1),
                                 skip_group_check=True)
        qdual = big.tile([128, T], FP)
        nc.scalar.activation(out=R(qdual[:]), in_=q_ps[:], func=AF.Identity, bias=bias_q[:])

        # ---------------- stage B: node -> kTn_dual, node_vT ----------------
        prefetch_leaf(1)
        nn = lnat.tile([128, 4, D], FP, tag="xnat")
        nc.sync.dma_start(R(nn[:]), R(d_node[:].rearrange("(j p) d -> p j d", p=128)))
        nodeT = ltp.tile([128, ND, 512], FP, tag="lt")
        for i in range(N // 128):
            tp = ptr.tile([128, 512], FP, tag="tp")
            for dc in range(ND):
                nc.tensor.transpose(R(tp[:, dc * 128:(dc + 1) * 128]),
                                    R(nn[:, i, dc * 128:(dc + 1) * 128]), R(identR[:]))
            if i % 2 == 0:
                nc.vector.tensor_copy(R(nodeT[:, 0:ND, i * 128:(i + 1) * 128]),
                                      R(tp[:].rearrange("p (dc b) -> p dc b", b=128)))
            else:
                nc.scalar.activation(
                    out=R(nodeT[:, 0:ND, i * 128:(i + 1) * 128]),
                    in_=R(tp[:].rearrange("p (dc b) -> p dc b", b=128)),
                    func=AF.Copy)
        kTn_dual = big.tile([128, 256], FP)
        node_vT = big.tile([64, N], FP)            # node_v^T + bias_v (bias pre-folded)
        kvn_ps = pmm.tile([128, 512], FP, tag="mm", name="kvn_ps")
        for dc in range(ND):
            nc.tensor.matmul(kvn_ps[:], R(w_kv[:, dc, :]), R(nodeT[:, dc, :]),
                             start=(dc == 0), stop=(dc == ND - 1))
        for b in range(4):
            ro, co = (b % 2) * 64, (b // 2) * 128
            nc.scalar.activation(out=R(kTn_dual[ro:ro + 64, co:co + 128]),
                                 in_=kvn_ps[0:64, b * 128:(b + 1) * 128],
                                 func=AF.Identity, bias=bias_k[ro:ro + 64, :])
        nc.vector.tensor_scalar(out=node_vT[:], in0=kvn_ps[64:128, :],
                                scalar1=bias_v[:], scalar2=None, op0=OP.add)

        # remaining constants (needed only after the leaf pass); pool/DVE slot
        # these behind stage B's work
        tri_raw = work.tile([128, 128], FP, tag="traw")
        make_lower_triangular(nc, tri_raw[:], val=1.0, diag=True)
        tri128 = consts.tile([128, 128], FP)      # [m,l]=1 iff l<=m  (suffix lhsT)
        nc.vector.tensor_copy(R(tri128[:]), tri_raw[:])
        tri32s = consts.tile([32, 32], FP)        # [k,c]=1 iff k>c   (carry)
        make_lower_triangular(nc, tri32s[:], val=1.0, diag=False)
        G = consts.tile([128, 16], FP)            # G[m,j] = 1 iff m//8 == j
        nc.gpsimd.memset(G[:], 1.0)
        nc.gpsimd.affine_select(out=G[:], in_=G[:], compare_op=OP.is_ge, fill=0.0,
                                base=0, pattern=[[-BR, 16]], channel_multiplier=1)
        nc.gpsimd.affine_select(out=G[:], in_=G[:], compare_op=OP.is_ge, fill=0.0,
                                base=BR - 1, pattern=[[BR, 16]], channel_multiplier=-1)
        GT = consts.tile([16, 128], FP)
        nc.gpsimd.memset(GT[:], 1.0)
        nc.gpsimd.affine_select(out=GT[:], in_=GT[:], compare_op=OP.is_ge, fill=0.0,
                                base=0, pattern=[[1, 128]], channel_multiplier=-BR)
        nc.gpsimd.affine_select(out=GT[:], in_=GT[:], compare_op=OP.is_ge, fill=0.0,
                                base=BR - 1, pattern=[[-1, 128]], channel_multiplier=BR)
        ones_raw = consts.tile([128, 1], FP)
        nc.gpsimd.memset(ones_raw[:], 1.0)
        onesP = consts.tile([128, 64], FP)
        nc.vector.tensor_copy(R(onesP[:]), bass.AP(tensor=ones_raw[:].tensor,
                                                   offset=ones_raw[:].offset,
                                                   ap=[[1, 128], [0, 64]]))
        cnt3 = consts.tile([128, NC], FP)         # 3 * (L - l), l = 128*c + p
        nc.gpsimd.iota(cnt3[:], pattern=[[-3 * 128, NC]], base=3 * L,
                       channel_multiplier=-3, allow_small_or_imprecise_dtypes=True)
        inv3 = consts.tile([128, NC], FP)
        nc.vector.reciprocal(inv3[:], cnt3[:])

        # ------- stage C + F: leaf pass with pipelined leaf attention -------
        # kTdual: 512-chunk i -> rows (i%2)*64, cols (i//2)*512
        kTdual = big.tile([128, L // 2], FP)
        leafT = big.tile([128, ND, L], FP)     # persistent
        lgn = big.tile([128, NC], FP)          # leaf logits, natural layout
        tile12i = big.tile([64, L], FP)        # interp' = leaf_v + node_rep
        vcomb = big.tile([128, NC, 65], BF)    # [v(64) | ones] per 128-leaf chunk
        nc.vector.tensor_copy(vcomb[:, :, 64:65],
                              bass.AP(tensor=ones_raw[:].tensor,
                                      offset=ones_raw[:].offset,
                                      ap=[[1, 128], [0, NC], [1, 1]]))
        o2_ps = pacc.tile([65, T], FP, tag="acc", name="o2_ps")
        totT = big.tile([64, NC], FP)          # per-chunk interp totals (for carries)
        att_q = deque()          # (ct, half) score work not yet emitted
        acc_q = deque()          # (el, b2) exp'd scores awaiting accumulate
        el_state = {"done": 0}

        def emit_score(ct, half):
            ro2 = half * 64
            b2 = 8 * (ct // 4) + ct % 4 + 4 * half
            st = pmm.tile([128, T], FP, tag="mm")
            for h in range(2):
                hs = slice(h * 512, (h + 1) * 512)
                nc.tensor.matmul(st[:, hs],
                                 R(kTdual[ro2:ro2 + 64, ct * 128:(ct + 1) * 128]),
                                 R(qdual[ro2:ro2 + 64, hs]), start=True, stop=True,
                                 skip_group_check=True)
            el = epool.tile([128, T], BF, tag="el")
            nc.scalar.activation(out=el[:], in_=st[:], func=AF.Exp, scale=SCALE)
            acc_q.append((el, b2))

        def emit_acc():
            el, b2 = acc_q.popleft()
            for h in range(2):
                hs = slice(h * 512, (h + 1) * 512)
                nc.tensor.matmul(o2_ps[:, hs], vcomb[:, b2, 0:65], el[:, hs],
                                 start=(el_state["done"] == 0),
                                 stop=(el_state["done"] == 31),
                                 skip_group_check=True)
            el_state["done"] += 1

        for i in range(L // 512):
            ln = ln_pre.popleft() if ln_pre else None
            if ln is None:
                ln = lnat.tile([128, 4, D], FP, tag="xnat")
                nc.sync.dma_start(R(ln[:]), R(d_leaf[i * 512:(i + 1) * 512, :]
                                  .rearrange("(j p) d -> p j d", p=128)))
            if i + 2 < L // 512:
                prefetch_leaf(i + 2)
            for j in range(4):
                tp = ptr.tile([128, 512], FP, tag="tp")
                for dc in range(ND):
                    nc.tensor.transpose(R(tp[:, dc * 128:(dc + 1) * 128]),
                                        R(ln[:, j, dc * 128:(dc + 1) * 128]), R(identR[:]))
                if j % 2 == 0:
                    nc.vector.tensor_copy(
                        R(leafT[:, 0:ND, (4 * i + j) * 128:(4 * i + j + 1) * 128]),
                        R(tp[:].rearrange("p (dc b) -> p dc b", b=128)))
                else:
                    nc.scalar.activation(
                        out=R(leafT[:, 0:ND, (4 * i + j) * 128:(4 * i + j + 1) * 128]),
                        in_=R(tp[:].rearrange("p (dc b) -> p dc b", b=128)),
                        func=AF.Copy)
                if att_q:
                    emit_score(*att_q.popleft())
                if j >= 2 and len(acc_q) >= 3:
                    emit_acc()
            kv_ps = pmm.tile([128, 528], FP, tag="mm")
            for dc in range(ND):
                nc.tensor.matmul(kv_ps[:, 0:512], R(w_kv[:, dc, :]),
                                 R(leafT[:, dc, i * 512:(i + 1) * 512]),
                                 start=(dc == 0), stop=(dc == ND - 1),
                                 skip_group_check=True)
            # logits: one single (non-accumulating) matmul per (column, d-chunk);
            # nested accumulation groups wedge the device, so the 4 d-chunk
            # partials land in separate columns and are summed on DVE below
            for cj in range(4):
                c = 4 * i + cj
                for dc in range(ND):
                    nc.tensor.matmul(kv_ps[:, 512 + 4 * cj + dc:513 + 4 * cj + dc],
                                     leafT[:, dc, c * 128:(c + 1) * 128],
                                     wagg_raw[:, dc:dc + 1],
                                     start=True, stop=True,
                                     skip_group_check=True)
            # drain kv_ps promptly: it shares the PSUM rotation with the scores
            ro, co = (i % 2) * 64, (i // 2) * 512
            sl = slice(i * 512, (i + 1) * 512)
            nc.vector.tensor_scalar(out=R(kTdual[ro:ro + 64, co:co + 512]),
                                    in0=kv_ps[0:64, 0:512], scalar1=bias_k[ro:ro + 64, :],
                                    scalar2=None, op0=OP.add)
            t12v = work.tile([64, 512], FP, tag="t12v")
            nc.vector.tensor_scalar(out=t12v[:], in0=kv_ps[64:128, 0:512],
                                    scalar1=bias_v[:], scalar2=None, op0=OP.add)
            nc.vector.tensor_reduce(
                out=lgn[:, 4 * i:4 * i + 4],
                in_=kv_ps[:, 512:528].rearrange("p (c d) -> p c d", d=4),
                axis=AX.X, op=OP.add)
            while acc_q:
                emit_acc()
            # interp' = leaf_v + node_vT' replicated 8x along l
            nc.gpsimd.tensor_tensor(
                out=tile12i[:, sl].rearrange("f (n c) -> f n c", c=BR),
                in0=t12v[:].rearrange("f (n c) -> f n c", c=BR),
                in1=_rep_ap(node_vT[0:64, 64 * i:64 * (i + 1)], BR), op=OP.add)
            # v back to natural for the attention lhsT: 4x [64,128]->[128,64]
            tpv = ptr.tile([128, 512], FP, tag="tp")
            for c4 in range(4):
                nc.tensor.transpose(tpv[:, c4 * 64:(c4 + 1) * 64],
                                    t12v[:, c4 * 128:(c4 + 1) * 128],
                                    ident[0:64, 0:64])
            nc.vector.tensor_copy(
                vcomb[:, 4 * i:4 * i + 4, 0:64],
                tpv[:, 0:256].rearrange("p (c f) -> p c f", f=64))
            nc.vector.tensor_reduce(out=totT[:, 4 * i:4 * i + 4],
                                    in_=tile12i[:, sl].rearrange("f (c m) -> f c m", m=128),
                                    axis=AX.X, op=OP.add)
            # queue the 4 leaf-attention units this chunk unlocks
            g, half = i // 2, i % 2
            for ct in range(4 * g, 4 * g + 4):
                att_q.append((ct, half))

        node_en = {}

        def emit_node_score(b):
            ro, co = (b % 2) * 64, (b // 2) * 128
            st = pmm.tile([128, T], FP, tag="mm")
            for h in range(2):
                hs = slice(h * 512, (h + 1) * 512)
                nc.tensor.matmul(st[:, hs], R(kTn_dual[ro:ro + 64, co:co + 128]),
                                 R(qdual[ro:ro + 64, hs]), start=True, stop=True,
                                 skip_group_check=True)
            en = epool.tile([128, T], BF, tag="el")
            nc.scalar.activation(out=en[:], in_=st[:], func=AF.Exp, scale=SCALE)
            node_en[b] = en

        def emit_node_acc(b):
            en = node_en.pop(b)
            for h in range(2):
                hs = slice(h * 512, (h + 1) * 512)
                nc.tensor.matmul(o1_ps[:, hs], nh_nat[:, b, 0:65], en[:, hs],
                                 start=(b == 0), stop=(b == 3), skip_group_check=True)

        # ---- flush remaining attention; carries run concurrently on DVE/PE ----
        while att_q or acc_q:
            if att_q:
                emit_score(*att_q.popleft())
            if acc_q:
                emit_acc()
        emit_node_score(0)
        emit_node_score(1)
        tot_ps = ptr.tile([NC, 64], FP, tag="tp")
        nc.tensor.transpose(tot_ps[:], totT[:], ident[0:64, 0:64])
        totals = work.tile([NC, 64], FP, tag="tot")
        nc.scalar.activation(out=totals[:], in_=tot_ps[:], func=AF.Copy)
        carrT_ps = ptr.tile([64, NC], FP, tag="tp")
        nc.tensor.matmul(carrT_ps[:], totals[:], tri32s[:], start=True, stop=True)
        # interpT[f, 128c+127] += carryT[f, c]  (row 127 is in every suffix sum)
        last_rows = tile12i[:, 127::128]
        nc.vector.tensor_tensor(out=last_rows, in0=last_rows, in1=carrT_ps[:], op=OP.add)
        rt_ps = ptr.tile([F, 1], FP, tag="tp")
        nc.tensor.transpose(rt_ps[:], root_row[:], ident[0:1, 0:1])
        rootT3 = consts.tile([F, 1], FP)
        nc.scalar.activation(out=rootT3[:], in_=rt_ps[:], func=AF.Copy, scale=1.0 / 3.0)
        # ---------------- logits -> group-softmax weights ----------------
        e_all = work.tile([128, NC], FP, tag="e_all")
        nc.scalar.activation(out=e_all[:], in_=lgn[:], func=AF.Exp, bias=bagg_b[:])
        s_ps = pmm.tile([16, NC], FP, tag="mm", name="s_ps")
        nc.tensor.matmul(s_ps[:], G[:], e_all[:], start=True, stop=True)
        sinv = work.tile([16, NC], FP, tag="sinv")
        nc.vector.reciprocal(sinv[:], s_ps[:])
        r_ps = pmm.tile([128, NC], FP, tag="mm", name="r_ps")
        nc.tensor.matmul(r_ps[:], GT[:], sinv[:], start=True, stop=True)
        w_all = work.tile([128, NC], FP, tag="w_all")
        nc.vector.tensor_tensor(out=w_all[:], in0=e_all[:], in1=r_ps[:], op=OP.mult)

        o2_sb = big.tile([65, T], FP)
        nc.vector.tensor_copy(o2_sb[:], o2_ps[:])
        fsr = fin.tile([65, T], FP, tag="fsr")  # rows 0/32/64: 1/Z1, 1/Z2, 1/Z3
        nc.vector.reciprocal(R(fsr[32:33, :]), o2_sb[64:65, :])


        # o2-dependent half of the final combine runs during the node phase
        b2 = pmm.tile([64, T], FP, tag="mm", name="b2")
        for h in range(2):
            hs = slice(h * 512, (h + 1) * 512)
            nc.tensor.matmul(b2[:, hs], R(onesP[32:33, 0:64]), R(fsr[32:33, hs]),
                             start=True, stop=True, skip_group_check=True)
        x2 = fin.tile([64, T], FP, tag="x2")
        for h in range(2):
            hs = slice(h * 512, (h + 1) * 512)
            nc.vector.tensor_tensor(out=x2[:, hs], in0=o2_sb[0:64, hs],
                                    in1=b2[:, hs], op=OP.mult)

        # ------- suffix-mean + node_hat, with node attention interleaved -------
        nh_nat = big.tile([128, 4, 65], BF)
        nc.vector.tensor_copy(nh_nat[:, :, 64:65],
                              bass.AP(tensor=ones_raw[:].tensor,
                                      offset=ones_raw[:].offset,
                                      ap=[[1, 128], [0, 4], [1, 1]]))
        wblk = big.tile([128, 8, 16], FP)      # per-chunk G-masked weights, rotating
        o1_ps = pacc.tile([65, T], FP, tag="acc", name="o1_ps")


        for g in range(NC // 8):
            # interp chunks back to natural: 8x [64,128]->[128,64]
            tpi = ptr.tile([128, 512], FP, tag="tp")
            for jc in range(8):
                c = 8 * g + jc
                nc.tensor.transpose(tpi[:, jc * 64:(jc + 1) * 64],
                                    tile12i[:, c * 128:(c + 1) * 128],
                                    ident[0:64, 0:64])
            icomb = work.tile([128, 8, 64], FP, tag="icomb")
            nc.scalar.activation(out=R(icomb[:].rearrange("p c f -> p (c f)")), in_=tpi[:], func=AF.Copy)
            sfx_ps = pmm.tile([128, 8, 64], FP, tag="mm")
            nc.tensor.matmul(sfx_ps[:], R(tri128[:]), R(icomb[:]), start=True, stop=True,
                             skip_group_check=True)
            upw4 = work.tile([128, 8, 64], FP, tag="upw")
            nc.vector.tensor_tensor(out=R(upw4[:]), in0=sfx_ps[:],
                                    in1=_rep_ap(inv3[:, 8 * g:8 * g + 8], 64),
                                    op=OP.mult)
            # nh^T[f, 16-block c] = upw_c^T @ (G * w_all[:,c]) - disjoint out slices
            for jc in range(8):
                c = 8 * g + jc
                nc.gpsimd.tensor_scalar(out=R(wblk[:, jc, :]), in0=G[:],
                                         scalar1=w_all[:, c:c + 1],
                                         scalar2=None, op0=OP.mult)
            nhT_ps = pmm.tile([64, 128], FP, tag="mm")
            for jc in range(8):
                nc.tensor.matmul(nhT_ps[0:64, 16 * jc:16 * jc + 16], R(upw4[:, jc, :]),
                                 R(wblk[:, jc, :]), start=True, stop=True,
                                 skip_group_check=True)
            nhT_sb = work.tile([64, 128], FP, tag="nhT")
            nc.scalar.activation(out=R(nhT_sb[:]), in_=nhT_ps[:], func=AF.Copy)
            nhn_ps = pmm.tile([128, 64], FP, tag="mm")
            nc.tensor.transpose(R(nhn_ps[:]), R(nhT_sb[:]), R(identR[0:64, 0:64]))
            nc.vector.tensor_copy(nh_nat[:, g, 0:64], nhn_ps[:])
            emit_node_acc(g)
            if g + 2 < NC // 8:
                emit_node_score(g + 2)
        # ------- combine + final softmax over F, pipelined in T/2 halves -------
        nc.vector.reciprocal(R(fsr[0:1, :]), o1_ps[64:65, :])
        b1 = pmm.tile([64, T], FP, tag="mm", name="b1")
        for h in range(2):
            hs = slice(h * 512, (h + 1) * 512)
            nc.tensor.matmul(b1[:, hs], R(onesP[0:1, 0:64]), R(fsr[0:1, hs]),
                             start=True, stop=True, skip_group_check=True)
        o1_sb = big.tile([64, T], FP)
        x1 = fin.tile([64, T], FP, tag="x1")
        s12 = fin.tile([64, T], FP, tag="s12")
        e3 = fin.tile([64, T], FP, tag="e3")
        onat_raw = big.tile([128, T // 128, F], FP)
        onat = big.tile([128, T // 128, F], FP)
        zq = fin.tile([128, T // 128], FP, tag="zq")
        rz = fin.tile([128, T // 128], FP, tag="rz")
        for hq in range(4):
            q = slice(hq * TQ, (hq + 1) * TQ)
            nc.scalar.activation(out=o1_sb[:, q], in_=o1_ps[0:64, q], func=AF.Copy)
            nc.vector.tensor_tensor(out=x1[:, q], in0=o1_sb[:, q], in1=b1[:, q],
                                    op=OP.mult)
            es = nc.vector if hq % 2 == 0 else nc.gpsimd
            es.tensor_tensor(out=s12[:, q], in0=x1[:, q], in1=x2[:, q],
                             op=OP.add)
            nc.scalar.activation(out=e3[:, q], in_=s12[:, q], func=AF.Exp,
                                 bias=rootT3[:])
            # unnormalized exp to natural layout; Z is then per-partition
            op_ = ptr.tile([128, 512], FP, tag="tp")
            for k in (2 * hq, 2 * hq + 1):
                nc.tensor.transpose(op_[:, (k % 2) * 64:(k % 2) * 64 + 64],
                                    e3[:, k * 128:(k + 1) * 128],
                                    ident[0:64, 0:64])
            ks = slice(2 * hq, 2 * hq + 2)
            if hq % 2 == 0:
                nc.vector.tensor_copy(onat_raw[:, ks, :].rearrange("p k f -> p (k f)"),
                                      op_[:, 0:128])
            else:
                nc.scalar.activation(out=onat_raw[:, ks, :].rearrange("p k f -> p (k f)"),
                                     in_=op_[:, 0:128], func=AF.Copy)
            nc.vector.tensor_reduce(out=zq[:, ks], in_=onat_raw[:, ks, :],
                                    axis=AX.X, op=OP.add)
            nc.vector.reciprocal(rz[:, ks], zq[:, ks])
            for k in (2 * hq, 2 * hq + 1):
                nc.gpsimd.tensor_scalar(out=onat[:, k, :], in0=onat_raw[:, k, :],
                                        scalar1=rz[:, k:k + 1], scalar2=None,
                                        op0=OP.mult)
            nc.sync.dma_start(
                d_out[hq * 256:(hq + 1) * 256, :]
                .rearrange("(k p) f -> p k f", p=128),
                onat[:, 2 * hq:2 * hq + 2, :])


_NC_CACHE = None


def kernel(**inputs):
    global _NC_CACHE
    if _NC_CACHE is None:
        _NC_CACHE = build_nc()
    nc = _NC_CACHE
    shared = {k: np.ascontiguousarray(np.asarray(inputs[k], dtype=np.float32))
              for k in ("Wq", "bq", "Wk", "bk", "Wv", "bv", "Wagg", "bagg")}
    in_maps = []
    for b in range(B):
        m = dict(shared)
        m["root"] = np.ascontiguousarray(np.asarray(inputs["root"][b], dtype=np.float32))
        m["node"] = np.ascontiguousarray(np.asarray(inputs["node"][b], dtype=np.float32))
        m["leaf"] = np.ascontiguousarray(np.asarray(inputs["leaf"][b], dtype=np.float32))
        m["target"] = np.ascontiguousarray(np.asarray(inputs["target"][b], dtype=np.float32))
        in_maps.append(m)
    res = run_bass_kernel_spmd(nc, in_maps, core_ids=list(range(B)))
    return np.stack([r["out"] for r in res.results], axis=0)


# revision 65
# speedup vs baseline: 2.4008x; 1.0097x over previous
"""Trainium2 Bass kernel for nn_DecoderAttention (dual-key tree decoder attention).

Sharding: data-parallel over batch B=8, one batch element per NeuronCore.

Per-core computation (B-slice), fp32 data with fp32r (split-bf16) matmuls:
  q = target @ Wq + bq                     [T,F]   (kept transposed, duplicated on 128 partitions)
  k/v (node, leaf) = x @ {Wk,Wv} + b       (kept transposed [F, *] via PE-transposed inputs)
  logits = leaf @ Wagg + bagg              [L,1]   (PE: leafT-chunk stationary x Wagg column)
  Aqn/Aql softmaxes are computed unnormalized (exp, no max-subtraction: |scores/8| <~ 1.2)
  out_pre = (En^T @ [nh|1])/Z1 + (El^T @ [v|1])/Z2 + root/3
  out = softmax_F(out_pre)                 [T,F]
The tree interpolation's root term commutes through the suffix-mean and the
attention average (softmax weights sum to 1), so root/3 is added once at the end.
Suffix cumsum over L: per-128-chunk triangular matmuls (batched 4 chunks / matmul);
the cross-chunk carries are folded into the LAST ROW of each interp chunk before
the in-chunk suffix (row 127 participates in every suffix sum of its chunk).

Schedule: the leaf-attention score/exp/accumulate stream (the dominant
Activation-engine load) is software-pipelined into the leaf projection loop —
each 512-leaf chunk queues the 4 (score, exp, accumulate) units it unlocks and
drains the queue one chunk behind, so the Activation engine runs continuously
while PE fills its gaps with the next chunk's transposes/projections. Only the
node path (suffix mean -> node_hat -> node attention) waits for the full leaf
pass (it needs the global suffix carries); node attention is interleaved into
the suffix loop, and the final combine/softmax is pipelined in T/4 slices.
PSUM budget (8 banks): pmm 2x[128,1024] + ptr 2x[128,512] + pacc 1x[65,1024],
with the pacc buffer sequenced o2 -> logits -> o1 -> z3.
"""

import os
import sys
from collections import deque

import numpy as np

for _p in ("/opt/trn_rl_repo", "/root/.axon_site/_ro/trn_rl_repo"):
    if os.path.isdir(_p) and _p not in sys.path:
        sys.path.insert(0, _p)

import concourse.bass as bass
import concourse.tile as tile
from concourse import bacc
from concourse import mybir
from concourse.bass_utils import run_bass_kernel_spmd
from concourse.masks import make_identity, make_lower_triangular

FP = mybir.dt.float32
FR = mybir.dt.float32r
BF = mybir.dt.bfloat16
AF = mybir.ActivationFunctionType
OP = mybir.AluOpType
AX = mybir.AxisListType

B, T, N, L, D, F = 8, 1024, 512, 4096, 512, 64
BR = L // N          # 8 leaves per node
NC = L // 128        # 32 leaf chunks of 128
ND = D // 128        # 4 contraction chunks
TQ = T // 4          # final-stage pipeline slice
SCALE = 1.0 / float(np.sqrt(F))


def R(ap):
    """View an fp32 AP as float32r (full-rate PE matmuls, split-bf16 numerics)."""
    return ap.bitcast(FR)


def _rep_ap(ap, rep):
    """Append a step-0 innermost free dim (read each element `rep` times)."""
    return bass.AP(tensor=ap.tensor, offset=ap.offset, ap=list(ap.ap) + [[0, rep]])


def build_nc():
    nc = bacc.Bacc("TRN2", target_bir_lowering=False, debug=False)

    d_root = nc.dram_tensor("root", [1, F], FP, kind="ExternalInput")
    d_node = nc.dram_tensor("node", [N, D], FP, kind="ExternalInput")
    d_leaf = nc.dram_tensor("leaf", [L, D], FP, kind="ExternalInput")
    d_target = nc.dram_tensor("target", [T, D], FP, kind="ExternalInput")
    d_wq = nc.dram_tensor("Wq", [D, F], FP, kind="ExternalInput")
    d_bq = nc.dram_tensor("bq", [F], FP, kind="ExternalInput")
    d_wk = nc.dram_tensor("Wk", [D, F], FP, kind="ExternalInput")
    d_bk = nc.dram_tensor("bk", [F], FP, kind="ExternalInput")
    d_wv = nc.dram_tensor("Wv", [D, F], FP, kind="ExternalInput")
    d_bv = nc.dram_tensor("bv", [F], FP, kind="ExternalInput")
    d_wagg = nc.dram_tensor("Wagg", [D, 1], FP, kind="ExternalInput")
    d_bagg = nc.dram_tensor("bagg", [1], FP, kind="ExternalInput")
    d_out = nc.dram_tensor("out", [T, F], FP, kind="ExternalOutput")

    with tile.TileContext(nc) as tc:
        _emit(nc, tc, d_root, d_node, d_leaf, d_target, d_wq, d_bq, d_wk, d_bk,
              d_wv, d_bv, d_wagg, d_bagg, d_out)
    nc.compile()
    return nc


def _emit(nc, tc, d_root, d_node, d_leaf, d_target, d_wq, d_bq, d_wk, d_bk,
          d_wv, d_bv, d_wagg, d_bagg, d_out):
    from contextlib import ExitStack

    with ExitStack() as ctx:
        ctx.enter_context(nc.allow_low_precision(
            reason="fp32r rounding on matmul operands is intentional"))
        consts = ctx.enter_context(tc.tile_pool(name="consts", bufs=1))
        big = ctx.enter_context(tc.tile_pool(name="big", bufs=1))
        lnat = ctx.enter_context(tc.tile_pool(name="lnat", bufs=3))
        ltp = ctx.enter_context(tc.tile_pool(name="ltp", bufs=2))
        work = ctx.enter_context(tc.tile_pool(name="work", bufs=2))
        fin = ctx.enter_context(tc.tile_pool(name="fin", bufs=1))
        epool = ctx.enter_context(tc.tile_pool(name="epool", bufs=3))
        ptr = ctx.enter_context(tc.tile_pool(name="ptr", bufs=2, space="PSUM"))
        pmm = ctx.enter_context(tc.tile_pool(name="pmm", bufs=2, space="PSUM"))
        pacc = ctx.enter_context(tc.tile_pool(name="pacc", bufs=1, space="PSUM"))

        # ---- identity only (gates all transposes); other consts deferred ----
        ident = consts.tile([128, 128], FP)
        make_identity(nc, ident[:])
        identR = consts.tile([128, 128], FP)   # fp32r-rounded copy for R-transposes
        nc.vector.tensor_copy(R(identR[:]), ident[:])

        # ---------------- stage A: target -> qdual [128, 1024] ----------------
        tns = []
        tn = lnat.tile([128, 4, D], FP, tag="xnat")
        nc.sync.dma_start(R(tn[:]), R(d_target[0:512, :]
                          .rearrange("(j p) d -> p j d", p=128)))
        tns.append(tn)

        # weights / biases; w_qq queued early (gates qdual), w_kv after leaf0
        w_kv = consts.tile([128, ND, 128], FP)     # cols 0:64 Wk, 64:128 Wv per d-chunk
        w_qq = consts.tile([128, ND, 128], FP)     # Wq duplicated
        wagg_raw = consts.tile([128, ND], FP)
        bias_q = consts.tile([128, 1], FP)
        bias_k = consts.tile([128, 1], FP)
        bias_v = consts.tile([64, 1], FP)
        bq2 = d_bq[:].rearrange("(f o) -> f o", o=1)
        bk2 = d_bk[:].rearrange("(f o) -> f o", o=1)
        nc.sync.dma_start(bias_q[0:F, :], bq2)
        nc.sync.dma_start(bias_q[F:128, :], bq2)
        nc.sync.dma_start(bias_k[0:F, :], bk2)
        nc.sync.dma_start(bias_k[F:128, :], bk2)
        nc.sync.dma_start(bias_v[:], d_bv[:].rearrange("(f o) -> f o", o=1))
        bagg_b = consts.tile([128, 1], FP)
        _ba = d_bagg[:]
        nc.gpsimd.dma_start(bagg_b[:], bass.AP(tensor=_ba.tensor, offset=_ba.offset,
                                               ap=[[0, 128], [1, 1]]))
        root_row = consts.tile([1, F], FP)
        nc.sync.dma_start(root_row[:], d_root[:])

        ln_pre = deque()

        def prefetch_leaf(i):
            ln = lnat.tile([128, 4, D], FP, tag="xnat")
            nc.sync.dma_start(R(ln[:]), R(d_leaf[i * 512:(i + 1) * 512, :]
                              .rearrange("(j p) d -> p j d", p=128)))
            ln_pre.append(ln)

        nc.sync.dma_start(R(w_qq[:, :, 0:F]), R(d_wq[:].rearrange("(j p) f -> p j f", p=128)))
        nc.sync.dma_start(R(w_qq[:, :, F:128]), R(d_wq[:].rearrange("(j p) f -> p j f", p=128)))
        tn = lnat.tile([128, 4, D], FP, tag="xnat")
        nc.sync.dma_start(R(tn[:]), R(d_target[512:1024, :]
                          .rearrange("(j p) d -> p j d", p=128)))
        tns.append(tn)
        prefetch_leaf(0)
        nc.sync.dma_start(R(w_kv[:, :, 0:F]), R(d_wk[:].rearrange("(j p) f -> p j f", p=128)))
        nc.sync.dma_start(R(w_kv[:, :, F:128]), R(d_wv[:].rearrange("(j p) f -> p j f", p=128)))
        nc.sync.dma_start(wagg_raw[:], d_wagg[:].rearrange("(j p) o -> p (j o)", p=128))
        q_ps = pmm.tile([128, T], FP, tag="mm", name="q_ps")
        for ib in range(T // 512):
            targT = ltp.tile([128, ND, 512], FP, tag="lt")
            for j in range(4):
                tp = ptr.tile([128, 512], FP, tag="tp")
                for dc in range(ND):
                    nc.tensor.transpose(R(tp[:, dc * 128:(dc + 1) * 128]),
                                        R(tns[ib][:, j, dc * 128:(dc + 1) * 128]),
                                        R(identR[:]))
                if j % 2 == 0:
                    nc.vector.tensor_copy(R(targT[:, 0:ND, j * 128:(j + 1) * 128]),
                                          R(tp[:].rearrange("p (dc b) -> p dc b", b=128)))
                else:
                    nc.scalar.activation(
                        out=R(targT[:, 0:ND, j * 128:(j + 1) * 128]),
                        in_=R(tp[:].rearrange("p (dc b) -> p dc b", b=128)),
                        func=AF.Copy)
            for dc in range(ND):
                nc.tensor.matmul(q_ps[:, ib * 512:(ib + 1) * 512], R(w_qq[:, dc, :]),
                                 R(targT[:, dc, :]), start=(dc == 0), stop=(dc == ND - 1),
                                 skip_group_check=True)
        qdual = big.tile([128, T], FP)
        nc.scalar.activation(out=R(qdual[:]), in_=q_ps[:], func=AF.Identity, bias=bias_q[:])

        # ---------------- stage B: node -> kTn_dual, node_vT ----------------
        prefetch_leaf(1)
        nn = lnat.tile([128, 4, D], FP, tag="xnat")
        nc.sync.dma_start(R(nn[:]), R(d_node[:].rearrange("(j p) d -> p j d", p=128)))
        nodeT = ltp.tile([128, ND, 512], FP, tag="lt")
        for i in range(N // 128):
            tp = ptr.tile([128, 512], FP, tag="tp")
            for dc in range(ND):
                nc.tensor.transpose(R(tp[:, dc * 128:(dc + 1) * 128]),
                                    R(nn[:, i, dc * 128:(dc + 1) * 128]), R(identR[:]))
            if i % 2 == 0:
                nc.vector.tensor_copy(R(nodeT[:, 0:ND, i * 128:(i + 1) * 128]),
                                      R(tp[:].rearrange("p (dc b) -> p dc b", b=128)))
            else:
                nc.scalar.activation(
                    out=R(nodeT[:, 0:ND, i * 128:(i + 1) * 128]),
                    in_=R(tp[:].rearrange("p (dc b) -> p dc b", b=128)),
                    func=AF.Copy)
        kTn_dual = big.tile([128, 256], FP)
        node_vT = big.tile([64, N], FP)            # node_v^T + bias_v (bias pre-folded)
        kvn_ps = pmm.tile([128, 512], FP, tag="mm", name="kvn_ps")
        for dc in range(ND):
            nc.tensor.matmul(kvn_ps[:], R(w_kv[:, dc, :]), R(nodeT[:, dc, :]),
                             start=(dc == 0), stop=(dc == ND - 1))
        for b in range(4):
            ro, co = (b % 2) * 64, (b // 2) * 128
            nc.scalar.activation(out=R(kTn_dual[ro:ro + 64, co:co + 128]),
                                 in_=kvn_ps[0:64, b * 128:(b + 1) * 128],
                                 func=AF.Identity, bias=bias_k[ro:ro + 64, :])
        nc.vector.tensor_scalar(out=node_vT[:], in0=kvn_ps[64:128, :],
                                scalar1=bias_v[:], scalar2=None, op0=OP.add)

        # remaining constants (needed only after the leaf pass); pool/DVE slot
        # these behind stage B's work
        tri_raw = work.tile([128, 128], FP, tag="traw")
        make_lower_triangular(nc, tri_raw[:], val=1.0, diag=True)
        tri128 = consts.tile([128, 128], FP)      # [m,l]=1 iff l<=m  (suffix lhsT)
        nc.vector.tensor_copy(R(tri128[:]), tri_raw[:])
        tri32s = consts.tile([32, 32], FP)        # [k,c]=1 iff k>c   (carry)
        make_lower_triangular(nc, tri32s[:], val=1.0, diag=False)
        G = consts.tile([128, 16], FP)            # G[m,j] = 1 iff m//8 == j
        nc.gpsimd.memset(G[:], 1.0)
        nc.gpsimd.affine_select(out=G[:], in_=G[:], compare_op=OP.is_ge, fill=0.0,
                                base=0, pattern=[[-BR, 16]], channel_multiplier=1)
        nc.gpsimd.affine_select(out=G[:], in_=G[:], compare_op=OP.is_ge, fill=0.0,
                                base=BR - 1, pattern=[[BR, 16]], channel_multiplier=-1)
        GT = consts.tile([16, 128], FP)
        nc.gpsimd.memset(GT[:], 1.0)
        nc.gpsimd.affine_select(out=GT[:], in_=GT[:], compare_op=OP.is_ge, fill=0.0,
                                base=0, pattern=[[1, 128]], channel_multiplier=-BR)
        nc.gpsimd.affine_select(out=GT[:], in_=GT[:], compare_op=OP.is_ge, fill=0.0,
                                base=BR - 1, pattern=[[-1, 128]], channel_multiplier=BR)
        ones_raw = consts.tile([128, 1], FP)
        nc.gpsimd.memset(ones_raw[:], 1.0)
        onesP = consts.tile([128, 64], FP)
        nc.vector.tensor_copy(R(onesP[:]), bass.AP(tensor=ones_raw[:].tensor,
                                                   offset=ones_raw[:].offset,
                                                   ap=[[1, 128], [0, 64]]))
        cnt3 = consts.tile([128, NC], FP)         # 3 * (L - l), l = 128*c + p
        nc.gpsimd.iota(cnt3[:], pattern=[[-3 * 128, NC]], base=3 * L,
                       channel_multiplier=-3, allow_small_or_imprecise_dtypes=True)
        inv3 = consts.tile([128, NC], FP)
        nc.vector.reciprocal(inv3[:], cnt3[:])

        # ------- stage C + F: leaf pass with pipelined leaf attention -------
        # kTdual: 512-chunk i -> rows (i%2)*64, cols (i//2)*512
        kTdual = big.tile([128, L // 2], FP)
        leafT = big.tile([128, ND, L], FP)     # persistent
        lgn = big.tile([128, NC], FP)          # leaf logits, natural layout
        tile12i = big.tile([64, L], FP)        # interp' = leaf_v + node_rep
        vcomb = big.tile([128, NC, 65], BF)    # [v(64) | ones] per 128-leaf chunk
        nc.vector.tensor_copy(vcomb[:, :, 64:65],
                              bass.AP(tensor=ones_raw[:].tensor,
                                      offset=ones_raw[:].offset,
                                      ap=[[1, 128], [0, NC], [1, 1]]))
        o2_ps = pacc.tile([65, T], FP, tag="acc", name="o2_ps")
        totT = big.tile([64, NC], FP)          # per-chunk interp totals (for carries)
        att_q = deque()          # (ct, half) score work not yet emitted
        acc_q = deque()          # (el, b2) exp'd scores awaiting accumulate
        el_state = {"done": 0}

        def emit_score(ct, half):
            ro2 = half * 64
            b2 = 8 * (ct // 4) + ct % 4 + 4 * half
            st = pmm.tile([128, T], FP, tag="mm")
            for h in range(2):
                hs = slice(h * 512, (h + 1) * 512)
                nc.tensor.matmul(st[:, hs],
                                 R(kTdual[ro2:ro2 + 64, ct * 128:(ct + 1) * 128]),
                                 R(qdual[ro2:ro2 + 64, hs]), start=True, stop=True,
                                 skip_group_check=True)
            el = epool.tile([128, T], BF, tag="el")
            nc.scalar.activation(out=el[:], in_=st[:], func=AF.Exp, scale=SCALE)
            acc_q.append((el, b2))

        def emit_acc():
            el, b2 = acc_q.popleft()
            for h in range(2):
                hs = slice(h * 512, (h + 1) * 512)
                nc.tensor.matmul(o2_ps[:, hs], vcomb[:, b2, 0:65], el[:, hs],
                                 start=(el_state["done"] == 0),
                                 stop=(el_state["done"] == 31),
                                 skip_group_check=True)
            el_state["done"] += 1

        for i in range(L // 512):
            ln = ln_pre.popleft() if ln_pre else None
            if ln is None:
                ln = lnat.tile([128, 4, D], FP, tag="xnat")
                nc.sync.dma_start(R(ln[:]), R(d_leaf[i * 512:(i + 1) * 512, :]
                                  .rearrange("(j p) d -> p j d", p=128)))
            if i + 2 < L // 512:
                prefetch_leaf(i + 2)
            for j in range(4):
                tp = ptr.tile([128, 512], FP, tag="tp")
                for dc in range(ND):
                    nc.tensor.transpose(R(tp[:, dc * 128:(dc + 1) * 128]),
                                        R(ln[:, j, dc * 128:(dc + 1) * 128]), R(identR[:]))
                nc.vector.tensor_copy(
                    R(leafT[:, 0:ND, (4 * i + j) * 128:(4 * i + j + 1) * 128]),
                    R(tp[:].rearrange("p (dc b) -> p dc b", b=128)))
                if att_q:
                    emit_score(*att_q.popleft())
                if j >= 2 and len(acc_q) >= 3:
                    emit_acc()
            kv_ps = pmm.tile([128, 528], FP, tag="mm")
            for dc in range(ND):
                nc.tensor.matmul(kv_ps[:, 0:512], R(w_kv[:, dc, :]),
                                 R(leafT[:, dc, i * 512:(i + 1) * 512]),
                                 start=(dc == 0), stop=(dc == ND - 1),
                                 skip_group_check=True)
            # logits: one single (non-accumulating) matmul per (column, d-chunk);
            # nested accumulation groups wedge the device, so the 4 d-chunk
            # partials land in separate columns and are summed on DVE below
            for cj in range(4):
                c = 4 * i + cj
                for dc in range(ND):
                    nc.tensor.matmul(kv_ps[:, 512 + 4 * cj + dc:513 + 4 * cj + dc],
                                     leafT[:, dc, c * 128:(c + 1) * 128],
                                     wagg_raw[:, dc:dc + 1],
                                     start=True, stop=True,
                                     skip_group_check=True)
            # drain kv_ps promptly: it shares the PSUM rotation with the scores
            ro, co = (i % 2) * 64, (i // 2) * 512
            sl = slice(i * 512, (i + 1) * 512)
            nc.vector.tensor_scalar(out=R(kTdual[ro:ro + 64, co:co + 512]),
                                    in0=kv_ps[0:64, 0:512], scalar1=bias_k[ro:ro + 64, :],
                                    scalar2=None, op0=OP.add)
            t12v = work.tile([64, 512], FP, tag="t12v")
            nc.vector.tensor_scalar(out=t12v[:], in0=kv_ps[64:128, 0:512],
                                    scalar1=bias_v[:], scalar2=None, op0=OP.add)
            nc.vector.tensor_reduce(
                out=lgn[:, 4 * i:4 * i + 4],
                in_=kv_ps[:, 512:528].rearrange("p (c d) -> p c d", d=4),
                axis=AX.X, op=OP.add)
            while acc_q:
                emit_acc()
            # interp' = leaf_v + node_vT' replicated 8x along l
            nc.gpsimd.tensor_tensor(
                out=tile12i[:, sl].rearrange("f (n c) -> f n c", c=BR),
                in0=t12v[:].rearrange("f (n c) -> f n c", c=BR),
                in1=_rep_ap(node_vT[0:64, 64 * i:64 * (i + 1)], BR), op=OP.add)
            # v back to natural for the attention lhsT: 4x [64,128]->[128,64]
            tpv = ptr.tile([128, 512], FP, tag="tp")
            for c4 in range(4):
                nc.tensor.transpose(tpv[:, c4 * 64:(c4 + 1) * 64],
                                    t12v[:, c4 * 128:(c4 + 1) * 128],
                                    ident[0:64, 0:64])
            nc.vector.tensor_copy(
                vcomb[:, 4 * i:4 * i + 4, 0:64],
                tpv[:, 0:256].rearrange("p (c f) -> p c f", f=64))
            nc.vector.tensor_reduce(out=totT[:, 4 * i:4 * i + 4],
                                    in_=tile12i[:, sl].rearrange("f (c m) -> f c m", m=128),
                                    axis=AX.X, op=OP.add)
            # queue the 4 leaf-attention units this chunk unlocks
            g, half = i // 2, i % 2
            for ct in range(4 * g, 4 * g + 4):
                att_q.append((ct, half))

        node_en = {}

        def emit_node_score(b):
            ro, co = (b % 2) * 64, (b // 2) * 128
            st = pmm.tile([128, T], FP, tag="mm")
            for h in range(2):
                hs = slice(h * 512, (h + 1) * 512)
                nc.tensor.matmul(st[:, hs], R(kTn_dual[ro:ro + 64, co:co + 128]),
                                 R(qdual[ro:ro + 64, hs]), start=True, stop=True,
                                 skip_group_check=True)
            en = epool.tile([128, T], BF, tag="el")
            nc.scalar.activation(out=en[:], in_=st[:], func=AF.Exp, scale=SCALE)
            node_en[b] = en

        def emit_node_acc(b):
            en = node_en.pop(b)
            for h in range(2):
                hs = slice(h * 512, (h + 1) * 512)
                nc.tensor.matmul(o1_ps[:, hs], nh_nat[:, b, 0:65], en[:, hs],
                                 start=(b == 0), stop=(b == 3), skip_group_check=True)

        # ---- flush remaining attention; carries run concurrently on DVE/PE ----
        while att_q or acc_q:
            if att_q:
                emit_score(*att_q.popleft())
            if acc_q:
                emit_acc()
        emit_node_score(0)
        emit_node_score(1)
        tot_ps = ptr.tile([NC, 64], FP, tag="tp")
        nc.tensor.transpose(tot_ps[:], totT[:], ident[0:64, 0:64])
        totals = work.tile([NC, 64], FP, tag="tot")
        nc.scalar.activation(out=totals[:], in_=tot_ps[:], func=AF.Copy)
        carrT_ps = ptr.tile([64, NC], FP, tag="tp")
        nc.tensor.matmul(carrT_ps[:], totals[:], tri32s[:], start=True, stop=True)
        # interpT[f, 128c+127] += carryT[f, c]  (row 127 is in every suffix sum)
        last_rows = tile12i[:, 127::128]
        nc.vector.tensor_tensor(out=last_rows, in0=last_rows, in1=carrT_ps[:], op=OP.add)
        rt_ps = ptr.tile([F, 1], FP, tag="tp")
        nc.tensor.transpose(rt_ps[:], root_row[:], ident[0:1, 0:1])
        rootT3 = consts.tile([F, 1], FP)
        nc.scalar.activation(out=rootT3[:], in_=rt_ps[:], func=AF.Copy, scale=1.0 / 3.0)
        # ---------------- logits -> group-softmax weights ----------------
        e_all = work.tile([128, NC], FP, tag="e_all")
        nc.scalar.activation(out=e_all[:], in_=lgn[:], func=AF.Exp, bias=bagg_b[:])
        s_ps = pmm.tile([16, NC], FP, tag="mm", name="s_ps")
        nc.tensor.matmul(s_ps[:], G[:], e_all[:], start=True, stop=True)
        sinv = work.tile([16, NC], FP, tag="sinv")
        nc.vector.reciprocal(sinv[:], s_ps[:])
        r_ps = pmm.tile([128, NC], FP, tag="mm", name="r_ps")
        nc.tensor.matmul(r_ps[:], GT[:], sinv[:], start=True, stop=True)
        w_all = work.tile([128, NC], FP, tag="w_all")
        nc.vector.tensor_tensor(out=w_all[:], in0=e_all[:], in1=r_ps[:], op=OP.mult)

        o2_sb = big.tile([65, T], FP)
        nc.vector.tensor_copy(o2_sb[:], o2_ps[:])
        fsr = fin.tile([65, T], FP, tag="fsr")  # rows 0/32/64: 1/Z1, 1/Z2, 1/Z3
        nc.vector.reciprocal(R(fsr[32:33, :]), o2_sb[64:65, :])


        # o2-dependent half of the final combine runs during the node phase
        b2 = pmm.tile([64, T], FP, tag="mm", name="b2")
        for h in range(2):
            hs = slice(h * 512, (h + 1) * 512)
            nc.tensor.matmul(b2[:, hs], R(onesP[32:33, 0:64]), R(fsr[32:33, hs]),
                             start=True, stop=True, skip_group_check=True)
        x2 = fin.tile([64, T], FP, tag="x2")
        for h in range(2):
            hs = slice(h * 512, (h + 1) * 512)
            nc.vector.tensor_tensor(out=x2[:, hs], in0=o2_sb[0:64, hs],
                                    in1=b2[:, hs], op=OP.mult)

        # ------- suffix-mean + node_hat, with node attention interleaved -------
        nh_nat = big.tile([128, 4, 65], BF)
        nc.vector.tensor_copy(nh_nat[:, :, 64:65],
                              bass.AP(tensor=ones_raw[:].tensor,
                                      offset=ones_raw[:].offset,
                                      ap=[[1, 128], [0, 4], [1, 1]]))
        wblk = big.tile([128, 8, 16], FP)      # per-chunk G-masked weights, rotating
        o1_ps = pacc.tile([65, T], FP, tag="acc", name="o1_ps")


        for g in range(NC // 8):
            # interp chunks back to natural: 8x [64,128]->[128,64]
            tpi = ptr.tile([128, 512], FP, tag="tp")
            for jc in range(8):
                c = 8 * g + jc
                nc.tensor.transpose(tpi[:, jc * 64:(jc + 1) * 64],
                                    tile12i[:, c * 128:(c + 1) * 128],
                                    ident[0:64, 0:64])
            icomb = work.tile([128, 8, 64], FP, tag="icomb")
            nc.scalar.activation(out=R(icomb[:].rearrange("p c f -> p (c f)")), in_=tpi[:], func=AF.Copy)
            sfx_ps = pmm.tile([128, 8, 64], FP, tag="mm")
            nc.tensor.matmul(sfx_ps[:], R(tri128[:]), R(icomb[:]), start=True, stop=True,
                             skip_group_check=True)
            upw4 = work.tile([128, 8, 64], FP, tag="upw")
            nc.vector.tensor_tensor(out=R(upw4[:]), in0=sfx_ps[:],
                                    in1=_rep_ap(inv3[:, 8 * g:8 * g + 8], 64),
                                    op=OP.mult)
            # nh^T[f, 16-block c] = upw_c^T @ (G * w_all[:,c]) - disjoint out slices
            for jc in range(8):
                c = 8 * g + jc
                nc.gpsimd.tensor_scalar(out=R(wblk[:, jc, :]), in0=G[:],
                                         scalar1=w_all[:, c:c + 1],
                                         scalar2=None, op0=OP.mult)
            nhT_ps = pmm.tile([64, 128], FP, tag="mm")
            for jc in range(8):
                nc.tensor.matmul(nhT_ps[0:64, 16 * jc:16 * jc + 16], R(upw4[:, jc, :]),
                                 R(wblk[:, jc, :]), start=True, stop=True,
                                 skip_group_check=True)
            nhT_sb = work.tile([64, 128], FP, tag="nhT")
            nc.scalar.activation(out=R(nhT_sb[:]), in_=nhT_ps[:], func=AF.Copy)
            nhn_ps = pmm.tile([128, 64], FP, tag="mm")
            nc.tensor.transpose(R(nhn_ps[:]), R(nhT_sb[:]), R(identR[0:64, 0:64]))
            nc.vector.tensor_copy(nh_nat[:, g, 0:64], nhn_ps[:])
            emit_node_acc(g)
            if g + 2 < NC // 8:
                emit_node_score(g + 2)
        # ------- combine + final softmax over F, pipelined in T/2 halves -------
        nc.vector.reciprocal(R(fsr[0:1, :]), o1_ps[64:65, :])
        b1 = pmm.tile([64, T], FP, tag="mm", name="b1")
        for h in range(2):
            hs = slice(h * 512, (h + 1) * 512)
            nc.tensor.matmul(b1[:, hs], R(onesP[0:1, 0:64]), R(fsr[0:1, hs]),
                             start=True, stop=True, skip_group_check=True)
        o1_sb = big.tile([64, T], FP)
        x1 = fin.tile([64, T], FP, tag="x1")
        s12 = fin.tile([64, T], FP, tag="s12")
        e3 = fin.tile([64, T], FP, tag="e3")
        onat_raw = big.tile([128, T // 128, F], FP)
        onat = big.tile([128, T // 128, F], FP)
        zq = fin.tile([128, T // 128], FP, tag="zq")
        rz = fin.tile([128, T // 128], FP, tag="rz")
        for hq in range(4):
            q = slice(hq * TQ, (hq + 1) * TQ)
            nc.scalar.activation(out=o1_sb[:, q], in_=o1_ps[0:64, q], func=AF.Copy)
            nc.vector.tensor_tensor(out=x1[:, q], in0=o1_sb[:, q], in1=b1[:, q],
                                    op=OP.mult)
            es = nc.vector if hq % 2 == 0 else nc.gpsimd
            es.tensor_tensor(out=s12[:, q], in0=x1[:, q], in1=x2[:, q],
                             op=OP.add)
            nc.scalar.activation(out=e3[:, q], in_=s12[:, q], func=AF.Exp,
                                 bias=rootT3[:])
            # unnormalized exp to natural layout; Z is then per-partition
            op_ = ptr.tile([128, 512], FP, tag="tp")
            for k in (2 * hq, 2 * hq + 1):
                nc.tensor.transpose(op_[:, (k % 2) * 64:(k % 2) * 64 + 64],
                                    e3[:, k * 128:(k + 1) * 128],
                                    ident[0:64, 0:64])
            ks = slice(2 * hq, 2 * hq + 2)
            if hq % 2 == 0:
                nc.vector.tensor_copy(onat_raw[:, ks, :].rearrange("p k f -> p (k f)"),
                                      op_[:, 0:128])
            else:
                nc.scalar.activation(out=onat_raw[:, ks, :].rearrange("p k f -> p (k f)"),
                                     in_=op_[:, 0:128], func=AF.Copy)
            nc.vector.tensor_reduce(out=zq[:, ks], in_=onat_raw[:, ks, :],
                                    axis=AX.X, op=OP.add)
            nc.vector.reciprocal(rz[:, ks], zq[:, ks])
            for k in (2 * hq, 2 * hq + 1):
                nc.gpsimd.tensor_scalar(out=onat[:, k, :], in0=onat_raw[:, k, :],
                                        scalar1=rz[:, k:k + 1], scalar2=None,
                                        op0=OP.mult)
            nc.sync.dma_start(
                d_out[hq * 256:(hq + 1) * 256, :]
                .rearrange("(k p) f -> p k f", p=128),
                onat[:, 2 * hq:2 * hq + 2, :])


_NC_CACHE = None


def kernel(**inputs):
    global _NC_CACHE
    if _NC_CACHE is None:
        _NC_CACHE = build_nc()
    nc = _NC_CACHE
    shared = {k: np.ascontiguousarray(np.asarray(inputs[k], dtype=np.float32))
              for k in ("Wq", "bq", "Wk", "bk", "Wv", "bv", "Wagg", "bagg")}
    in_maps = []
    for b in range(B):
        m = dict(shared)
        m["root"] = np.ascontiguousarray(np.asarray(inputs["root"][b], dtype=np.float32))
        m["node"] = np.ascontiguousarray(np.asarray(inputs["node"][b], dtype=np.float32))
        m["leaf"] = np.ascontiguousarray(np.asarray(inputs["leaf"][b], dtype=np.float32))
        m["target"] = np.ascontiguousarray(np.asarray(inputs["target"][b], dtype=np.float32))
        in_maps.append(m)
    res = run_bass_kernel_spmd(nc, in_maps, core_ids=list(range(B)))
    return np.stack([r["out"] for r in res.results], axis=0)


# revision 77
# speedup vs baseline: 2.4055x; 1.0019x over previous
"""Trainium2 Bass kernel for nn_DecoderAttention (dual-key tree decoder attention).

Sharding: data-parallel over batch B=8, one batch element per NeuronCore.

Per-core computation (B-slice), fp32 data with fp32r (split-bf16) matmuls:
  q = target @ Wq + bq                     [T,F]   (kept transposed, duplicated on 128 partitions)
  k/v (node, leaf) = x @ {Wk,Wv} + b       (kept transposed [F, *] via PE-transposed inputs)
  logits = leaf @ Wagg + bagg              [L,1]   (PE: leafT-chunk stationary x Wagg column)
  Aqn/Aql softmaxes are computed unnormalized (exp, no max-subtraction: |scores/8| <~ 1.2)
  out_pre = (En^T @ [nh|1])/Z1 + (El^T @ [v|1])/Z2 + root/3
  out = softmax_F(out_pre)                 [T,F]
The tree interpolation's root term commutes through the suffix-mean and the
attention average (softmax weights sum to 1), so root/3 is added once at the end.
Suffix cumsum over L: per-128-chunk triangular matmuls (batched 4 chunks / matmul);
the cross-chunk carries are folded into the LAST ROW of each interp chunk before
the in-chunk suffix (row 127 participates in every suffix sum of its chunk).

Schedule: the leaf-attention score/exp/accumulate stream (the dominant
Activation-engine load) is software-pipelined into the leaf projection loop —
each 512-leaf chunk queues the 4 (score, exp, accumulate) units it unlocks and
drains the queue one chunk behind, so the Activation engine runs continuously
while PE fills its gaps with the next chunk's transposes/projections. Only the
node path (suffix mean -> node_hat -> node attention) waits for the full leaf
pass (it needs the global suffix carries); node attention is interleaved into
the suffix loop, and the final combine/softmax is pipelined in T/4 slices.
PSUM budget (8 banks): pmm 2x[128,1024] + ptr 2x[128,512] + pacc 1x[65,1024],
with the pacc buffer sequenced o2 -> logits -> o1 -> z3.
"""

import os
import sys
from collections import deque

import numpy as np

for _p in ("/opt/trn_rl_repo", "/root/.axon_site/_ro/trn_rl_repo"):
    if os.path.isdir(_p) and _p not in sys.path:
        sys.path.insert(0, _p)

import concourse.bass as bass
import concourse.tile as tile
from concourse import bacc
from concourse import mybir
from concourse.bass_utils import run_bass_kernel_spmd
from concourse.masks import make_identity, make_lower_triangular

FP = mybir.dt.float32
FR = mybir.dt.float32r
BF = mybir.dt.bfloat16
AF = mybir.ActivationFunctionType
OP = mybir.AluOpType
AX = mybir.AxisListType

B, T, N, L, D, F = 8, 1024, 512, 4096, 512, 64
BR = L // N          # 8 leaves per node
NC = L // 128        # 32 leaf chunks of 128
ND = D // 128        # 4 contraction chunks
TQ = T // 4          # final-stage pipeline slice
SCALE = 1.0 / float(np.sqrt(F))


def R(ap):
    """View an fp32 AP as float32r (full-rate PE matmuls, split-bf16 numerics)."""
    return ap.bitcast(FR)


def _rep_ap(ap, rep):
    """Append a step-0 innermost free dim (read each element `rep` times)."""
    return bass.AP(tensor=ap.tensor, offset=ap.offset, ap=list(ap.ap) + [[0, rep]])


def build_nc():
    nc = bacc.Bacc("TRN2", target_bir_lowering=False, debug=False)

    d_root = nc.dram_tensor("root", [1, F], FP, kind="ExternalInput")
    d_node = nc.dram_tensor("node", [N, D], FP, kind="ExternalInput")
    d_leaf = nc.dram_tensor("leaf", [L, D], FP, kind="ExternalInput")
    d_target = nc.dram_tensor("target", [T, D], FP, kind="ExternalInput")
    d_wq = nc.dram_tensor("Wq", [D, F], FP, kind="ExternalInput")
    d_bq = nc.dram_tensor("bq", [F], FP, kind="ExternalInput")
    d_wk = nc.dram_tensor("Wk", [D, F], FP, kind="ExternalInput")
    d_bk = nc.dram_tensor("bk", [F], FP, kind="ExternalInput")
    d_wv = nc.dram_tensor("Wv", [D, F], FP, kind="ExternalInput")
    d_bv = nc.dram_tensor("bv", [F], FP, kind="ExternalInput")
    d_wagg = nc.dram_tensor("Wagg", [D, 1], FP, kind="ExternalInput")
    d_bagg = nc.dram_tensor("bagg", [1], FP, kind="ExternalInput")
    d_out = nc.dram_tensor("out", [T, F], FP, kind="ExternalOutput")

    with tile.TileContext(nc) as tc:
        _emit(nc, tc, d_root, d_node, d_leaf, d_target, d_wq, d_bq, d_wk, d_bk,
              d_wv, d_bv, d_wagg, d_bagg, d_out)
    nc.compile()
    return nc


def _emit(nc, tc, d_root, d_node, d_leaf, d_target, d_wq, d_bq, d_wk, d_bk,
          d_wv, d_bv, d_wagg, d_bagg, d_out):
    from contextlib import ExitStack

    with ExitStack() as ctx:
        ctx.enter_context(nc.allow_low_precision(
            reason="fp32r rounding on matmul operands is intentional"))
        consts = ctx.enter_context(tc.tile_pool(name="consts", bufs=1))
        big = ctx.enter_context(tc.tile_pool(name="big", bufs=1))
        lnat = ctx.enter_context(tc.tile_pool(name="lnat", bufs=3))
        ltp = ctx.enter_context(tc.tile_pool(name="ltp", bufs=2))
        work = ctx.enter_context(tc.tile_pool(name="work", bufs=2))
        fin = ctx.enter_context(tc.tile_pool(name="fin", bufs=1))
        epool = ctx.enter_context(tc.tile_pool(name="epool", bufs=3))
        ptr = ctx.enter_context(tc.tile_pool(name="ptr", bufs=2, space="PSUM"))
        pmm = ctx.enter_context(tc.tile_pool(name="pmm", bufs=2, space="PSUM"))
        pacc = ctx.enter_context(tc.tile_pool(name="pacc", bufs=1, space="PSUM"))

        # ---- identity only (gates all transposes); other consts deferred ----
        ident = consts.tile([128, 128], FP)
        make_identity(nc, ident[:])
        identR = consts.tile([128, 128], FP)   # fp32r-rounded copy for R-transposes
        nc.vector.tensor_copy(R(identR[:]), ident[:])

        # ---------------- stage A: target -> qdual [128, 1024] ----------------
        tns = []
        tn = lnat.tile([128, 4, D], FP, tag="xnat")
        nc.sync.dma_start(R(tn[:]), R(d_target[0:512, :]
                          .rearrange("(j p) d -> p j d", p=128)))
        tns.append(tn)

        # weights / biases; w_qq queued early (gates qdual), w_kv after leaf0
        w_kv = consts.tile([128, ND, 128], FP)     # cols 0:64 Wk, 64:128 Wv per d-chunk
        w_qq = consts.tile([128, ND, 128], FP)     # Wq duplicated
        wagg_raw = consts.tile([128, ND], FP)
        bias_q = consts.tile([128, 1], FP)
        bias_k = consts.tile([128, 1], FP)
        bias_v = consts.tile([64, 1], FP)
        bq2 = d_bq[:].rearrange("(f o) -> f o", o=1)
        bk2 = d_bk[:].rearrange("(f o) -> f o", o=1)
        nc.sync.dma_start(bias_q[0:F, :], bq2)
        nc.sync.dma_start(bias_q[F:128, :], bq2)
        nc.sync.dma_start(bias_k[0:F, :], bk2)
        nc.sync.dma_start(bias_k[F:128, :], bk2)
        nc.sync.dma_start(bias_v[:], d_bv[:].rearrange("(f o) -> f o", o=1))
        bagg_b = consts.tile([128, 1], FP)
        _ba = d_bagg[:]
        nc.gpsimd.dma_start(bagg_b[:], bass.AP(tensor=_ba.tensor, offset=_ba.offset,
                                               ap=[[0, 128], [1, 1]]))
        root_row = consts.tile([1, F], FP)
        nc.sync.dma_start(root_row[:], d_root[:])

        ln_pre = deque()

        def prefetch_leaf(i):
            ln = lnat.tile([128, 4, D], FP, tag="xnat")
            nc.sync.dma_start(R(ln[:]), R(d_leaf[i * 512:(i + 1) * 512, :]
                              .rearrange("(j p) d -> p j d", p=128)))
            ln_pre.append(ln)

        nc.sync.dma_start(R(w_qq[:, :, 0:F]), R(d_wq[:].rearrange("(j p) f -> p j f", p=128)))
        nc.sync.dma_start(R(w_qq[:, :, F:128]), R(d_wq[:].rearrange("(j p) f -> p j f", p=128)))
        tn = lnat.tile([128, 4, D], FP, tag="xnat")
        nc.sync.dma_start(R(tn[:]), R(d_target[512:1024, :]
                          .rearrange("(j p) d -> p j d", p=128)))
        tns.append(tn)
        prefetch_leaf(0)
        nc.sync.dma_start(R(w_kv[:, :, 0:F]), R(d_wk[:].rearrange("(j p) f -> p j f", p=128)))
        nc.sync.dma_start(R(w_kv[:, :, F:128]), R(d_wv[:].rearrange("(j p) f -> p j f", p=128)))
        nc.sync.dma_start(wagg_raw[:], d_wagg[:].rearrange("(j p) o -> p (j o)", p=128))
        q_ps = pmm.tile([128, T], FP, tag="mm", name="q_ps")
        for ib in range(T // 512):
            targT = ltp.tile([128, ND, 512], FP, tag="lt")
            for j in range(4):
                tp = ptr.tile([128, 512], FP, tag="tp")
                for dc in range(ND):
                    nc.tensor.transpose(R(tp[:, dc * 128:(dc + 1) * 128]),
                                        R(tns[ib][:, j, dc * 128:(dc + 1) * 128]),
                                        R(identR[:]))
                if j % 2 == 0:
                    nc.vector.tensor_copy(R(targT[:, 0:ND, j * 128:(j + 1) * 128]),
                                          R(tp[:].rearrange("p (dc b) -> p dc b", b=128)))
                else:
                    nc.scalar.activation(
                        out=R(targT[:, 0:ND, j * 128:(j + 1) * 128]),
                        in_=R(tp[:].rearrange("p (dc b) -> p dc b", b=128)),
                        func=AF.Copy)
            for dc in range(ND):
                nc.tensor.matmul(q_ps[:, ib * 512:(ib + 1) * 512], R(w_qq[:, dc, :]),
                                 R(targT[:, dc, :]), start=(dc == 0), stop=(dc == ND - 1),
                                 skip_group_check=True)
        qdual = big.tile([128, T], FP)
        nc.scalar.activation(out=R(qdual[:]), in_=q_ps[:], func=AF.Identity, bias=bias_q[:])

        # ---------------- stage B: node -> kTn_dual, node_vT ----------------
        prefetch_leaf(1)
        nn = lnat.tile([128, 4, D], FP, tag="xnat")
        nc.sync.dma_start(R(nn[:]), R(d_node[:].rearrange("(j p) d -> p j d", p=128)))
        nodeT = ltp.tile([128, ND, 512], FP, tag="lt")
        for i in range(N // 128):
            tp = ptr.tile([128, 512], FP, tag="tp")
            for dc in range(ND):
                nc.tensor.transpose(R(tp[:, dc * 128:(dc + 1) * 128]),
                                    R(nn[:, i, dc * 128:(dc + 1) * 128]), R(identR[:]))
            if i % 2 == 0:
                nc.vector.tensor_copy(R(nodeT[:, 0:ND, i * 128:(i + 1) * 128]),
                                      R(tp[:].rearrange("p (dc b) -> p dc b", b=128)))
            else:
                nc.scalar.activation(
                    out=R(nodeT[:, 0:ND, i * 128:(i + 1) * 128]),
                    in_=R(tp[:].rearrange("p (dc b) -> p dc b", b=128)),
                    func=AF.Copy)
        kTn_dual = big.tile([128, 256], FP)
        node_vT = big.tile([64, N], FP)            # node_v^T + bias_v (bias pre-folded)
        kvn_ps = pmm.tile([128, 512], FP, tag="mm", name="kvn_ps")
        for dc in range(ND):
            nc.tensor.matmul(kvn_ps[:], R(w_kv[:, dc, :]), R(nodeT[:, dc, :]),
                             start=(dc == 0), stop=(dc == ND - 1))
        for b in range(4):
            ro, co = (b % 2) * 64, (b // 2) * 128
            nc.scalar.activation(out=R(kTn_dual[ro:ro + 64, co:co + 128]),
                                 in_=kvn_ps[0:64, b * 128:(b + 1) * 128],
                                 func=AF.Identity, bias=bias_k[ro:ro + 64, :])
        nc.vector.tensor_scalar(out=node_vT[:], in0=kvn_ps[64:128, :],
                                scalar1=bias_v[:], scalar2=None, op0=OP.add)

        # remaining constants (needed only after the leaf pass); pool/DVE slot
        # these behind stage B's work
        tri_raw = work.tile([128, 128], FP, tag="traw")
        make_lower_triangular(nc, tri_raw[:], val=1.0, diag=True)
        tri128 = consts.tile([128, 128], FP)      # [m,l]=1 iff l<=m  (suffix lhsT)
        nc.vector.tensor_copy(R(tri128[:]), tri_raw[:])
        tri32s = consts.tile([32, 32], FP)        # [k,c]=1 iff k>c   (carry)
        make_lower_triangular(nc, tri32s[:], val=1.0, diag=False)
        G = consts.tile([128, 16], FP)            # G[m,j] = 1 iff m//8 == j
        nc.gpsimd.memset(G[:], 1.0)
        nc.gpsimd.affine_select(out=G[:], in_=G[:], compare_op=OP.is_ge, fill=0.0,
                                base=0, pattern=[[-BR, 16]], channel_multiplier=1)
        nc.gpsimd.affine_select(out=G[:], in_=G[:], compare_op=OP.is_ge, fill=0.0,
                                base=BR - 1, pattern=[[BR, 16]], channel_multiplier=-1)
        GT = consts.tile([16, 128], FP)
        nc.gpsimd.memset(GT[:], 1.0)
        nc.gpsimd.affine_select(out=GT[:], in_=GT[:], compare_op=OP.is_ge, fill=0.0,
                                base=0, pattern=[[1, 128]], channel_multiplier=-BR)
        nc.gpsimd.affine_select(out=GT[:], in_=GT[:], compare_op=OP.is_ge, fill=0.0,
                                base=BR - 1, pattern=[[-1, 128]], channel_multiplier=BR)
        ones_raw = consts.tile([128, 1], FP)
        nc.gpsimd.memset(ones_raw[:], 1.0)
        onesP = consts.tile([128, 64], FP)
        nc.vector.tensor_copy(R(onesP[:]), bass.AP(tensor=ones_raw[:].tensor,
                                                   offset=ones_raw[:].offset,
                                                   ap=[[1, 128], [0, 64]]))
        cnt3 = consts.tile([128, NC], FP)         # 3 * (L - l), l = 128*c + p
        nc.gpsimd.iota(cnt3[:], pattern=[[-3 * 128, NC]], base=3 * L,
                       channel_multiplier=-3, allow_small_or_imprecise_dtypes=True)
        inv3 = consts.tile([128, NC], FP)
        nc.vector.reciprocal(inv3[:], cnt3[:])

        # ------- stage C + F: leaf pass with pipelined leaf attention -------
        # kTdual: 512-chunk i -> rows (i%2)*64, cols (i//2)*512
        kTdual = big.tile([128, L // 2], FP)
        leafT = big.tile([128, ND, L], FP)     # persistent
        lgn = big.tile([128, NC], FP)          # leaf logits, natural layout
        tile12i = big.tile([64, L], FP)        # interp' = leaf_v + node_rep
        vcomb = big.tile([128, NC, 65], BF)    # [v(64) | ones] per 128-leaf chunk
        nc.vector.tensor_copy(vcomb[:, :, 64:65],
                              bass.AP(tensor=ones_raw[:].tensor,
                                      offset=ones_raw[:].offset,
                                      ap=[[1, 128], [0, NC], [1, 1]]))
        o2_ps = pacc.tile([65, T], FP, tag="acc", name="o2_ps")
        totT = big.tile([64, NC], FP)          # per-chunk interp totals (for carries)
        att_q = deque()          # (ct, half) score work not yet emitted
        acc_q = deque()          # (el, b2) exp'd scores awaiting accumulate
        el_state = {"done": 0}

        def emit_score(ct, half):
            ro2 = half * 64
            b2 = 8 * (ct // 4) + ct % 4 + 4 * half
            st = pmm.tile([128, T], FP, tag="mm")
            for h in range(2):
                hs = slice(h * 512, (h + 1) * 512)
                nc.tensor.matmul(st[:, hs],
                                 R(kTdual[ro2:ro2 + 64, ct * 128:(ct + 1) * 128]),
                                 R(qdual[ro2:ro2 + 64, hs]), start=True, stop=True,
                                 skip_group_check=True)
            el = epool.tile([128, T], BF, tag="el")
            nc.scalar.activation(out=el[:], in_=st[:], func=AF.Exp, scale=SCALE)
            acc_q.append((el, b2))

        def emit_acc():
            el, b2 = acc_q.popleft()
            for h in range(2):
                hs = slice(h * 512, (h + 1) * 512)
                nc.tensor.matmul(o2_ps[:, hs], vcomb[:, b2, 0:65], el[:, hs],
                                 start=(el_state["done"] == 0),
                                 stop=(el_state["done"] == 31),
                                 skip_group_check=True)
            el_state["done"] += 1

        for i in range(L // 512):
            ln = ln_pre.popleft() if ln_pre else None
            if ln is None:
                ln = lnat.tile([128, 4, D], FP, tag="xnat")
                nc.sync.dma_start(R(ln[:]), R(d_leaf[i * 512:(i + 1) * 512, :]
                                  .rearrange("(j p) d -> p j d", p=128)))
            if i + 2 < L // 512:
                prefetch_leaf(i + 2)
            for j in range(4):
                tp = ptr.tile([128, 512], FP, tag="tp")
                for dc in range(ND):
                    nc.tensor.transpose(R(tp[:, dc * 128:(dc + 1) * 128]),
                                        R(ln[:, j, dc * 128:(dc + 1) * 128]), R(identR[:]))
                nc.vector.tensor_copy(
                    R(leafT[:, 0:ND, (4 * i + j) * 128:(4 * i + j + 1) * 128]),
                    R(tp[:].rearrange("p (dc b) -> p dc b", b=128)))
                if att_q:
                    emit_score(*att_q.popleft())

            kv_ps = pmm.tile([128, 528], FP, tag="mm")
            for dc in range(ND):
                nc.tensor.matmul(kv_ps[:, 0:512], R(w_kv[:, dc, :]),
                                 R(leafT[:, dc, i * 512:(i + 1) * 512]),
                                 start=(dc == 0), stop=(dc == ND - 1),
                                 skip_group_check=True)
            # logits: one single (non-accumulating) matmul per (column, d-chunk);
            # nested accumulation groups wedge the device, so the 4 d-chunk
            # partials land in separate columns and are summed on DVE below
            for cj in range(4):
                c = 4 * i + cj
                for dc in range(ND):
                    nc.tensor.matmul(kv_ps[:, 512 + 4 * cj + dc:513 + 4 * cj + dc],
                                     leafT[:, dc, c * 128:(c + 1) * 128],
                                     wagg_raw[:, dc:dc + 1],
                                     start=True, stop=True,
                                     skip_group_check=True)
            # drain kv_ps promptly: it shares the PSUM rotation with the scores
            ro, co = (i % 2) * 64, (i // 2) * 512
            sl = slice(i * 512, (i + 1) * 512)
            nc.vector.tensor_scalar(out=R(kTdual[ro:ro + 64, co:co + 512]),
                                    in0=kv_ps[0:64, 0:512], scalar1=bias_k[ro:ro + 64, :],
                                    scalar2=None, op0=OP.add)
            t12v = work.tile([64, 512], FP, tag="t12v")
            nc.vector.tensor_scalar(out=t12v[:], in0=kv_ps[64:128, 0:512],
                                    scalar1=bias_v[:], scalar2=None, op0=OP.add)
            while acc_q:
                emit_acc()
            nc.vector.tensor_reduce(
                out=lgn[:, 4 * i:4 * i + 4],
                in_=kv_ps[:, 512:528].rearrange("p (c d) -> p c d", d=4),
                axis=AX.X, op=OP.add)
            # interp' = leaf_v + node_vT' replicated 8x along l
            nc.gpsimd.tensor_tensor(
                out=tile12i[:, sl].rearrange("f (n c) -> f n c", c=BR),
                in0=t12v[:].rearrange("f (n c) -> f n c", c=BR),
                in1=_rep_ap(node_vT[0:64, 64 * i:64 * (i + 1)], BR), op=OP.add)
            # v back to natural for the attention lhsT: 4x [64,128]->[128,64]
            tpv = ptr.tile([128, 512], FP, tag="tp")
            for c4 in range(4):
                nc.tensor.transpose(tpv[:, c4 * 64:(c4 + 1) * 64],
                                    t12v[:, c4 * 128:(c4 + 1) * 128],
                                    ident[0:64, 0:64])
            nc.vector.tensor_copy(
                vcomb[:, 4 * i:4 * i + 4, 0:64],
                tpv[:, 0:256].rearrange("p (c f) -> p c f", f=64))
            nc.vector.tensor_reduce(out=totT[:, 4 * i:4 * i + 4],
                                    in_=tile12i[:, sl].rearrange("f (c m) -> f c m", m=128),
                                    axis=AX.X, op=OP.add)
            # queue the 4 leaf-attention units this chunk unlocks
            g, half = i // 2, i % 2
            for ct in range(4 * g, 4 * g + 4):
                att_q.append((ct, half))

        node_en = {}

        def emit_node_score(b):
            ro, co = (b % 2) * 64, (b // 2) * 128
            st = pmm.tile([128, T], FP, tag="mm")
            for h in range(2):
                hs = slice(h * 512, (h + 1) * 512)
                nc.tensor.matmul(st[:, hs], R(kTn_dual[ro:ro + 64, co:co + 128]),
                                 R(qdual[ro:ro + 64, hs]), start=True, stop=True,
                                 skip_group_check=True)
            en = epool.tile([128, T], BF, tag="el")
            nc.scalar.activation(out=en[:], in_=st[:], func=AF.Exp, scale=SCALE)
            node_en[b] = en

        def emit_node_acc(b):
            en = node_en.pop(b)
            for h in range(2):
                hs = slice(h * 512, (h + 1) * 512)
                nc.tensor.matmul(o1_ps[:, hs], nh_nat[:, b, 0:65], en[:, hs],
                                 start=(b == 0), stop=(b == 3), skip_group_check=True)

        # ---- flush remaining attention; carries run concurrently on DVE/PE ----
        while att_q or acc_q:
            if att_q:
                emit_score(*att_q.popleft())
            if acc_q:
                emit_acc()
        emit_node_score(0)
        emit_node_score(1)
        tot_ps = ptr.tile([NC, 64], FP, tag="tp")
        nc.tensor.transpose(tot_ps[:], totT[:], ident[0:64, 0:64])
        totals = work.tile([NC, 64], FP, tag="tot")
        nc.scalar.activation(out=totals[:], in_=tot_ps[:], func=AF.Copy)
        carrT_ps = ptr.tile([64, NC], FP, tag="tp")
        nc.tensor.matmul(carrT_ps[:], totals[:], tri32s[:], start=True, stop=True)
        # interpT[f, 128c+127] += carryT[f, c]  (row 127 is in every suffix sum)
        last_rows = tile12i[:, 127::128]
        nc.vector.tensor_tensor(out=last_rows, in0=last_rows, in1=carrT_ps[:], op=OP.add)
        rt_ps = ptr.tile([F, 1], FP, tag="tp")
        nc.tensor.transpose(rt_ps[:], root_row[:], ident[0:1, 0:1])
        rootT3 = consts.tile([F, 1], FP)
        nc.scalar.activation(out=rootT3[:], in_=rt_ps[:], func=AF.Copy, scale=1.0 / 3.0)
        # ---------------- logits -> group-softmax weights ----------------
        e_all = work.tile([128, NC], FP, tag="e_all")
        nc.scalar.activation(out=e_all[:], in_=lgn[:], func=AF.Exp, bias=bagg_b[:])
        s_ps = pmm.tile([16, NC], FP, tag="mm", name="s_ps")
        nc.tensor.matmul(s_ps[:], G[:], e_all[:], start=True, stop=True)
        sinv = work.tile([16, NC], FP, tag="sinv")
        nc.vector.reciprocal(sinv[:], s_ps[:])
        r_ps = pmm.tile([128, NC], FP, tag="mm", name="r_ps")
        nc.tensor.matmul(r_ps[:], GT[:], sinv[:], start=True, stop=True)
        w_all = work.tile([128, NC], FP, tag="w_all")
        nc.vector.tensor_tensor(out=w_all[:], in0=e_all[:], in1=r_ps[:], op=OP.mult)

        o2_sb = big.tile([65, T], FP)
        nc.vector.tensor_copy(o2_sb[:], o2_ps[:])
        fsr = fin.tile([65, T], FP, tag="fsr")  # rows 0/32/64: 1/Z1, 1/Z2, 1/Z3
        nc.vector.reciprocal(R(fsr[32:33, :]), o2_sb[64:65, :])


        # o2-dependent half of the final combine runs during the node phase
        b2 = pmm.tile([64, T], FP, tag="mm", name="b2")
        for h in range(2):
            hs = slice(h * 512, (h + 1) * 512)
            nc.tensor.matmul(b2[:, hs], R(onesP[32:33, 0:64]), R(fsr[32:33, hs]),
                             start=True, stop=True, skip_group_check=True)
        x2 = fin.tile([64, T], FP, tag="x2")
        for h in range(2):
            hs = slice(h * 512, (h + 1) * 512)
            nc.vector.tensor_tensor(out=x2[:, hs], in0=o2_sb[0:64, hs],
                                    in1=b2[:, hs], op=OP.mult)

        # ------- suffix-mean + node_hat, with node attention interleaved -------
        nh_nat = big.tile([128, 4, 65], BF)
        nc.vector.tensor_copy(nh_nat[:, :, 64:65],
                              bass.AP(tensor=ones_raw[:].tensor,
                                      offset=ones_raw[:].offset,
                                      ap=[[1, 128], [0, 4], [1, 1]]))
        wblk = big.tile([128, 8, 16], FP)      # per-chunk G-masked weights, rotating
        o1_ps = pacc.tile([65, T], FP, tag="acc", name="o1_ps")


        for g in range(NC // 8):
            # interp chunks back to natural: 8x [64,128]->[128,64]
            tpi = ptr.tile([128, 512], FP, tag="tp")
            for jc in range(8):
                c = 8 * g + jc
                nc.tensor.transpose(tpi[:, jc * 64:(jc + 1) * 64],
                                    tile12i[:, c * 128:(c + 1) * 128],
                                    ident[0:64, 0:64])
            icomb = work.tile([128, 8, 64], FP, tag="icomb")
            nc.scalar.activation(out=R(icomb[:].rearrange("p c f -> p (c f)")), in_=tpi[:], func=AF.Copy)
            sfx_ps = pmm.tile([128, 8, 64], FP, tag="mm")
            nc.tensor.matmul(sfx_ps[:], R(tri128[:]), R(icomb[:]), start=True, stop=True,
                             skip_group_check=True)
            upw4 = work.tile([128, 8, 64], FP, tag="upw")
            nc.vector.tensor_tensor(out=R(upw4[:]), in0=sfx_ps[:],
                                    in1=_rep_ap(inv3[:, 8 * g:8 * g + 8], 64),
                                    op=OP.mult)
            # nh^T[f, 16-block c] = upw_c^T @ (G * w_all[:,c]) - disjoint out slices
            for jc in range(8):
                c = 8 * g + jc
                nc.gpsimd.tensor_scalar(out=R(wblk[:, jc, :]), in0=G[:],
                                         scalar1=w_all[:, c:c + 1],
                                         scalar2=None, op0=OP.mult)
            nhT_ps = pmm.tile([64, 128], FP, tag="mm")
            for jc in range(8):
                nc.tensor.matmul(nhT_ps[0:64, 16 * jc:16 * jc + 16], R(upw4[:, jc, :]),
                                 R(wblk[:, jc, :]), start=True, stop=True,
                                 skip_group_check=True)
            nhT_sb = work.tile([64, 128], FP, tag="nhT")
            nc.scalar.activation(out=R(nhT_sb[:]), in_=nhT_ps[:], func=AF.Copy)
            nhn_ps = pmm.tile([128, 64], FP, tag="mm")
            nc.tensor.transpose(R(nhn_ps[:]), R(nhT_sb[:]), R(identR[0:64, 0:64]))
            nc.vector.tensor_copy(nh_nat[:, g, 0:64], nhn_ps[:])
            emit_node_acc(g)
            if g + 2 < NC // 8:
                emit_node_score(g + 2)
        # ------- combine + final softmax over F, pipelined in T/2 halves -------
        nc.vector.reciprocal(R(fsr[0:1, :]), o1_ps[64:65, :])
        b1 = pmm.tile([64, T], FP, tag="mm", name="b1")
        for h in range(2):
            hs = slice(h * 512, (h + 1) * 512)
            nc.tensor.matmul(b1[:, hs], R(onesP[0:1, 0:64]), R(fsr[0:1, hs]),
                             start=True, stop=True, skip_group_check=True)
        o1_sb = big.tile([64, T], FP)
        x1 = fin.tile([64, T], FP, tag="x1")
        s12 = fin.tile([64, T], FP, tag="s12")
        e3 = fin.tile([64, T], FP, tag="e3")
        onat_raw = big.tile([128, T // 128, F], FP)
        onat = big.tile([128, T // 128, F], FP)
        zq = fin.tile([128, T // 128], FP, tag="zq")
        rz = fin.tile([128, T // 128], FP, tag="rz")
        for hq in range(4):
            q = slice(hq * TQ, (hq + 1) * TQ)
            nc.scalar.activation(out=o1_sb[:, q], in_=o1_ps[0:64, q], func=AF.Copy)
            nc.vector.tensor_tensor(out=x1[:, q], in0=o1_sb[:, q], in1=b1[:, q],
                                    op=OP.mult)
            es = nc.vector if hq % 2 == 0 else nc.gpsimd
            es.tensor_tensor(out=s12[:, q], in0=x1[:, q], in1=x2[:, q],
                             op=OP.add)
            nc.scalar.activation(out=e3[:, q], in_=s12[:, q], func=AF.Exp,
                                 bias=rootT3[:])
            # unnormalized exp to natural layout; Z is then per-partition
            op_ = ptr.tile([128, 512], FP, tag="tp")
            for k in (2 * hq, 2 * hq + 1):
                nc.tensor.transpose(op_[:, (k % 2) * 64:(k % 2) * 64 + 64],
                                    e3[:, k * 128:(k + 1) * 128],
                                    ident[0:64, 0:64])
            ks = slice(2 * hq, 2 * hq + 2)
            if hq % 2 == 0:
                nc.vector.tensor_copy(onat_raw[:, ks, :].rearrange("p k f -> p (k f)"),
                                      op_[:, 0:128])
            else:
                nc.scalar.activation(out=onat_raw[:, ks, :].rearrange("p k f -> p (k f)"),
                                     in_=op_[:, 0:128], func=AF.Copy)
            nc.vector.tensor_reduce(out=zq[:, ks], in_=onat_raw[:, ks, :],
                                    axis=AX.X, op=OP.add)
            nc.vector.reciprocal(rz[:, ks], zq[:, ks])
            for k in (2 * hq, 2 * hq + 1):
                nc.gpsimd.tensor_scalar(out=onat[:, k, :], in0=onat_raw[:, k, :],
                                        scalar1=rz[:, k:k + 1], scalar2=None,
                                        op0=OP.mult)
            nc.sync.dma_start(
                d_out[hq * 256:(hq + 1) * 256, :]
                .rearrange("(k p) f -> p k f", p=128),
                onat[:, 2 * hq:2 * hq + 2, :])


_NC_CACHE = None


def kernel(**inputs):
    global _NC_CACHE
    if _NC_CACHE is None:
        _NC_CACHE = build_nc()
    nc = _NC_CACHE
    shared = {k: np.ascontiguousarray(np.asarray(inputs[k], dtype=np.float32))
              for k in ("Wq", "bq", "Wk", "bk", "Wv", "bv", "Wagg", "bagg")}
    in_maps = []
    for b in range(B):
        m = dict(shared)
        m["root"] = np.ascontiguousarray(np.asarray(inputs["root"][b], dtype=np.float32))
        m["node"] = np.ascontiguousarray(np.asarray(inputs["node"][b], dtype=np.float32))
        m["leaf"] = np.ascontiguousarray(np.asarray(inputs["leaf"][b], dtype=np.float32))
        m["target"] = np.ascontiguousarray(np.asarray(inputs["target"][b], dtype=np.float32))
        in_maps.append(m)
    res = run_bass_kernel_spmd(nc, in_maps, core_ids=list(range(B)))
    return np.stack([r["out"] for r in res.results], axis=0)
